# revision 1
# baseline (speedup 1.0000x reference)
"""Encoder-decoder GQA attention block (B=4, L=S=1024, H=2048, 32 Q heads,
8 KV heads, head_dim 64) + output projection + residual + layernorm, on 8
Trainium2 NeuronCores.

Sharding: rows. Core c handles batch c//2, L-half c%2 (512 query rows).
K/V projections are computed per-batch (duplicated on the 2 cores sharing a
batch — cheap), attention over all 32 heads for the core's rows, output
projection, residual + LN. No collectives.

Layout strategy: activations live feature-major ("transposed", H on the
partition dim) so every matmul uses naturally-laid-out weights as lhsT and
never needs an on-chip transpose:
  qT = Wq.T @ xT   (feature-major, per-partition bias add)
  kT = Wk.T @ eT   (feature-major)
  v  = (eT.T chunks as lhsT) @ Wv  (natural [s, 512]; free-dim bias add)
  scoresT[s, t] per head = kT_h.T @ qT_h  -> softmax over the partition (s)
  dim: exp via ACT (scale 1/8 fused), rowsum via an all-ones 65th column
  appended to V in the attn@V matmul; the reciprocal is broadcast across 64
  partitions with a K=1 matmul into the free upper half of the same PSUM
  bank. ctxT (feature-major) = V|1.T @ expT, then out = ctxT.T @ Wo lands
  back in natural [tok, H] layout for residual + layernorm (free-dim
  reductions).

Scheduling: Q-projection blocks are interleaved with attention per kv-head
pair so the ACT engine (softmax exp, ~133us) streams behind the PE the whole
time instead of saturating a separate attention phase; the last attn@V pair
of each kv-head pair is software-pipelined into the next block so its exp
tiles get a full production window. K/V projections are ordered to match
DMA arrival (K-low, V-low, K-high, V-high) on a dedicated 4-buffer PSUM
pool; input DMAs are k-granular so dependent chains start as chunks land;
the first Wo block and layernorm constants prefetch during attention; the
gamma/beta elementwise work runs on the otherwise-idle GpSimd engine in
phase C (except the final row block, where DVE is free). DMA issue order is
strict needed-first: the cost model (and aggregate HBM bandwidth on HW)
serializes transfers, so gamma/beta/Wo prefetches are deferred behind the
projection inputs. Cost model: ~322us/core; PE-busy floor ~280us (attention
matmuls are output-rate-bound at K=64/M=65, so the 280us floor is tight).

All matmuls bf16 (inputs pre-cast on host), fp32 PSUM accumulation, softmax
in fp32 (no max-subtraction: |scores| <= ~7 by construction, exp is safe in
fp32).
"""

from contextlib import ExitStack

import numpy as np
import ml_dtypes

import concourse.bass as bass  # noqa: F401  (bass.AP used via handles)
import concourse.mybir as mybir
import concourse.tile as tile
from concourse import bacc
from concourse.bass_utils import run_bass_kernel_spmd

BF16 = ml_dtypes.bfloat16

H = 2048
NH = 32
KVH = 8
G = 4           # query-head groups per kv head
HD = 64
B, L, S = 4, 1024, 1024
TOK = 512       # decoder rows per core
KC = H // 128   # 16 contraction chunks
SC = S // 128   # 8 s chunks
EPS = 1e-6

FP = mybir.dt.float32
BF = mybir.dt.bfloat16

_CACHE: dict = {}


def _build(use_mask: bool):
    nc = bacc.Bacc("TRN2", target_bir_lowering=False)

    xT = nc.dram_tensor("xT", [H, TOK], BF, kind="ExternalInput")
    xres = nc.dram_tensor("xres", [TOK, H], FP, kind="ExternalInput")
    eT = nc.dram_tensor("eT", [H, S], BF, kind="ExternalInput")
    wq = nc.dram_tensor("wq", [H, H], BF, kind="ExternalInput")
    wk = nc.dram_tensor("wk", [H, KVH * HD], BF, kind="ExternalInput")
    wv = nc.dram_tensor("wv", [H, KVH * HD], BF, kind="ExternalInput")
    wo = nc.dram_tensor("wo", [H, H], BF, kind="ExternalInput")
    bq2 = nc.dram_tensor("bq2", [128, KC], FP, kind="ExternalInput")
    bk2 = nc.dram_tensor("bk2", [128, 4], FP, kind="ExternalInput")
    bvr = nc.dram_tensor("bvr", [128, KVH * HD], FP, kind="ExternalInput")
    gamr = nc.dram_tensor("gamr", [128, H], BF, kind="ExternalInput")
    betr = nc.dram_tensor("betr", [128, H], BF, kind="ExternalInput")
    if use_mask:
        maskT = nc.dram_tensor("maskT", [S, TOK], BF, kind="ExternalInput")
    out = nc.dram_tensor("out", [TOK, H], FP, kind="ExternalOutput")

    Exp = mybir.ActivationFunctionType.Exp
    Sqrt = mybir.ActivationFunctionType.Sqrt

    eT4 = eT.rearrange("(j a p) s -> j p a s", a=4, p=128)
    xT4 = xT.rearrange("(j a p) s -> j p a s", a=4, p=128)
    wk4 = wk.rearrange("(j a p) n -> j p a n", a=4, p=128)
    wv4 = wv.rearrange("(j a p) n -> j p a n", a=4, p=128)
    wq4 = wq.rearrange("(j a p) n -> j p a n", a=4, p=128)
    wo4 = wo.rearrange("(j a p) n -> j p a n", a=4, p=128)

    with tile.TileContext(nc) as tc:
      with (
          tc.tile_pool(name="ctxT", bufs=KC) as ctxp,
          tc.tile_pool(name="cc", bufs=1) as ccp,
          tc.tile_pool(name="ln", bufs=10) as lnp,
      ):
        gam_sb = ccp.tile([128, H], BF, name="gam_sb")
        bet_sb = ccp.tile([128, H], BF, name="bet_sb")
        eps_sb = ccp.tile([128, 1], FP, name="eps_sb")
        nc.vector.memset(eps_sb[:], EPS)
        ctx_sb = [ctxp.tile([128, TOK], BF, tag="ctx", name="ctx")
                  for _ in range(KC)]

        wqp_cm = tc.tile_pool(name="wq", bufs=8)
        wqp = wqp_cm.__enter__()
        _stk = ExitStack()
        psA = _stk.enter_context(tc.tile_pool(name="psA", bufs=2, space="PSUM"))
        constp = _stk.enter_context(tc.tile_pool(name="const", bufs=1))
        xtp = _stk.enter_context(tc.tile_pool(name="xTp", bufs=4))
        qtp = _stk.enter_context(tc.tile_pool(name="qT", bufs=NH))
        ktp = _stk.enter_context(tc.tile_pool(name="kT", bufs=KVH))
        vvp = _stk.enter_context(tc.tile_pool(name="vv", bufs=SC))
        mkp = _stk.enter_context(
            tc.tile_pool(name="maskp", bufs=SC if use_mask else 1))

        qT_sb = [None] * NH
        kT_sb = [ktp.tile([64, S], BF, tag="kt", name="kt") for _ in range(KVH)]
        vv_sb = []

        with (
            tc.tile_pool(name="eTp", bufs=4) as etp,
            tc.tile_pool(name="wk", bufs=4) as wkp,
            tc.tile_pool(name="wv", bufs=4) as wvp,
            tc.tile_pool(name="psKV", bufs=4, space="PSUM") as psKV,
        ):
            # input DMAs, needed-first; 2-chunk granularity balances the
            # 565ns/DMA SP-SEQ issue cost against arrival granularity, and
            # the (tiny but 565ns each) bias DMAs issue after the first
            # critical transfers
            bq_sb = constp.tile([128, KC], FP, name="bq_sb")
            bk_sb = constp.tile([128, 4], FP, name="bk_sb")
            bv_sb = constp.tile([128, KVH * HD], FP, name="bv_sb")
            wk_sb, wv_sb, xT_sb = [], [], []
            et_tiles = []
            for j in range(4):
                t = wkp.tile([128, 4, 512], BF, tag="wkt", name="wkt")
                e = etp.tile([128, 4, S], BF, tag="et", name="et")
                for a in (0, 2):
                    nc.sync.dma_start(t[:, a:a + 2, :], wk4[j][:, a:a + 2, :])
                    nc.sync.dma_start(
                        e[:, a:a + 2, 0:512], eT4[j][:, a:a + 2, 0:512]
                    )
                wk_sb.extend(t[:, a, :] for a in range(4))
                et_tiles.append(e)
            nc.sync.dma_start(bq_sb[:], bq2[:])
            nc.sync.dma_start(bk_sb[:], bk2[:])
            nc.sync.dma_start(bv_sb[:], bvr[:])
            for j in range(4):
                t = wvp.tile([128, 4, 512], BF, tag="wvt", name="wvt")
                nc.sync.dma_start(t[:], wv4[j])
                wv_sb.extend(t[:, a, :] for a in range(4))
            for j in range(4):
                nc.sync.dma_start(et_tiles[j][:, :, 512:S], eT4[j][:, :, 512:S])
            for j in range(4):
                t = xtp.tile([128, 4, TOK], BF, tag="xt", name="xt")
                nc.scalar.dma_start(t[:], xT4[j])
                xT_sb.extend(t[:, a, :] for a in range(4))
            wq_blk = []
            for j in range(4):
                t = wqp.tile([128, 4, 512], BF, tag="wq", name="wqt")
                nc.scalar.dma_start(t[:], wq4[j][:, :, 0:512])
                wq_blk.append(t)
            nc.sync.dma_start(gam_sb[:], gamr[:])
            nc.sync.dma_start(bet_sb[:], betr[:])
            if use_mask:
                mask_sb = []
                for sc in range(SC):
                    t = mkp.tile([128, TOK], BF, tag="mk", name="mk")
                    nc.sync.dma_start(t[:], maskT[sc * 128:(sc + 1) * 128, :])
                    mask_sb.append(t)

            def k_proj(sh):
                # k-major across 4 concurrent PSUM chains: the PE consumes
                # each arriving (wk, eT) chunk with 4 matmuls, so the first
                # projection is DMA-paced with no per-chain stalls
                pss = [psKV.tile([128, 512], FP, tag="psKV", name="psKV")
                       for _ in range(4)]
                for k in range(KC):
                    for m in range(4):
                        nc.tensor.matmul(
                            pss[m][:],
                            wk_sb[k][:, m * 128:(m + 1) * 128],
                            et_tiles[k // 4][:, k % 4, sh * 512:(sh + 1) * 512],
                            start=(k == 0),
                            stop=(k == KC - 1),
                        )
                for m in range(4):
                    for hi in range(2):
                        h = 2 * m + hi
                        pb = hi * 64
                        nc.vector.tensor_scalar_add(
                            kT_sb[h][:, sh * 512:(sh + 1) * 512],
                            pss[m][pb:pb + 64, :],
                            bk_sb[pb:pb + 64, m:m + 1],
                        )

            def v_proj(scs):
                scs = list(scs)
                pss = [psKV.tile([128, 512], FP, tag="psKV", name="psKV")
                       for _ in scs]
                for k in range(KC):
                    for i, sc in enumerate(scs):
                        nc.tensor.matmul(
                            pss[i][:],
                            et_tiles[k // 4][:, k % 4, sc * 128:(sc + 1) * 128],
                            wv_sb[k],
                            start=(k == 0),
                            stop=(k == KC - 1),
                        )
                for i, sc in enumerate(scs):
                    v = vvp.tile([128, KVH, HD + 1], BF, tag="vv", name="vv")
                    nc.vector.tensor_add(
                        v[:, :, 0:HD],
                        pss[i].rearrange("p (h d) -> p h d", d=HD),
                        bv_sb.rearrange("p (h d) -> p h d", d=HD),
                    )
                    nc.vector.memset(v[:, :, HD:HD + 1], 1.0)
                    vv_sb.append(v)

            # DMA-arrival-paced: K(lo) -> V(lo) -> K(hi) -> V(hi)
            k_proj(0)
            v_proj(range(0, 4))
            k_proj(1)
            v_proj(range(4, SC))


        # inputs eT/wk/wv released; open attention pools in their space
        psS = _stk.enter_context(tc.tile_pool(name="psS", bufs=2, space="PSUM"))
        psO = _stk.enter_context(tc.tile_pool(name="psO", bufs=2, space="PSUM"))
        expp = _stk.enter_context(tc.tile_pool(name="expp", bufs=26))
        recp = _stk.enter_context(tc.tile_pool(name="rec", bufs=4))
        bcp = _stk.enter_context(tc.tile_pool(name="bc", bufs=4))
        ones_sb = constp.tile([1, 64], BF, name="ones_sb")
        nc.vector.memset(ones_sb[:], 1.0)

        def q_proj(m):
            q = m % 4
            ps = psA.tile([128, TOK], FP, tag="psA", name="psA")
            for k in range(KC):
                nc.tensor.matmul(
                    ps[:],
                    wq_blk[k // 4][:, k % 4, q * 128:(q + 1) * 128],
                    xT_sb[k][:],
                    start=(k == 0),
                    stop=(k == KC - 1),
                )
            for hi in range(2):
                qt = qtp.tile([64, TOK], BF, tag="qt", name="qt")
                nc.vector.tensor_scalar_add(
                    qt[:], ps[hi * 64:hi * 64 + 64, :],
                    bq_sb[hi * 64:hi * 64 + 64, m:m + 1],
                )
                qT_sb[2 * m + hi] = qt

        def scores_gp(h, gp):
            tiles = []
            for sc in range(SC):
                lhs = kT_sb[h][:, sc * 128:(sc + 1) * 128]
                ps = psS.tile([128, 2, TOK], FP, tag="psS", name="psS")
                for gi in range(2):
                    hh = h * G + gp * 2 + gi
                    nc.tensor.matmul(
                        ps[:, gi, :], lhs, qT_sb[hh][:], start=True, stop=True
                    )
                    if use_mask:
                        nc.vector.tensor_add(
                            ps[:, gi, :], ps[:, gi, :], mask_sb[sc][:]
                        )
                ex = expp.tile([128, 2, TOK], BF, tag="ex", name="ex")
                nc.scalar.activation(ex[:], ps[:], func=Exp, scale=0.125)
                tiles.append(ex)
            return tiles

        def attn_v_pair(h, gp, tiles):
            for gi in range(2):
                g = gp * 2 + gi
                hh = h * G + g
                po = psO.tile([128, TOK], FP, tag="psO", name="psO")
                for sc in range(SC):
                    nc.tensor.matmul(
                        po[0:HD + 1, :],
                        vv_sb[sc][:, h, :],
                        tiles[sc][:, gi, :],
                        start=(sc == 0),
                        stop=(sc == SC - 1),
                    )
                recb = recp.tile([1, TOK], BF, tag="recb", name="recb")
                with nc.allow_low_precision(reason="softmax recip rounds to bf16"):
                    nc.vector.reciprocal(recb[:], po[HD:HD + 1, :])
                po_sb = bcp.tile([64, TOK], FP, tag="posb", name="posb")
                nc.vector.tensor_copy(po_sb[:], po[0:HD, :])
                # broadcast recip across 64 partitions with a K=1 matmul into
                # the free upper half of the same PSUM bank
                nc.tensor.matmul(
                    po[64:128, :], ones_sb[:], recb[:], start=True, stop=True
                )
                nc.vector.tensor_mul(
                    ctx_sb[hh // 2][(hh % 2) * 64:(hh % 2) * 64 + 64, :],
                    po_sb[:],
                    po[64:128, :],
                )

        wo_blk0 = []
        pending = None
        for nb in range(4):
            h0, h1 = 2 * nb, 2 * nb + 1
            if pending is not None:
                attn_v_pair(*pending)  # (h1, gp1) of the previous nb
                pending = None
            q_proj(4 * nb + 0)
            q_proj(4 * nb + 1)
            t0 = scores_gp(h0, 0)
            q_proj(4 * nb + 2)
            q_proj(4 * nb + 3)
            if nb < 3:
                nxt = []
                for j in range(4):
                    t = wqp.tile([128, 4, 512], BF, tag="wq", name="wqt")
                    nc.scalar.dma_start(
                        t[:], wq4[j][:, :, (nb + 1) * 512:(nb + 2) * 512]
                    )
                    nxt.append(t)
            t1 = scores_gp(h0, 1)
            u0 = scores_gp(h1, 0)
            attn_v_pair(h0, 0, t0)
            if nb == 3:
                for nb2 in range(2):  # prefetch Wo blocks 0 and 1
                    for j in range(4):
                        t = wqp.tile([128, 4, 512], BF, tag="wq", name="wqt")
                        nc.scalar.dma_start(
                            t[:], wo4[j][:, :, nb2 * 512:(nb2 + 1) * 512]
                        )
                        wo_blk0.append(t)
            u1 = scores_gp(h1, 1)
            attn_v_pair(h0, 1, t1)
            attn_v_pair(h1, 0, u0)
            pending = (h1, 1, u1)
            if nb < 3:
                wq_blk = nxt
        attn_v_pair(*pending)

        _stk.close()  # release attention-phase pools

        # ---- Phase C: output projection + residual + layernorm ------------
        # tt-outer so each row-block's LN/store overlaps the next block's mms
        with (
            tc.tile_pool(name="psC", bufs=4, space="PSUM") as psC,
            tc.tile_pool(name="wC2", bufs=12) as wcp2,
            tc.tile_pool(name="xr", bufs=6) as xrp,
            tc.tile_pool(name="outp", bufs=2) as outp,
        ):
            def _ln(tt, ob):
                stats = lnp.tile([128, 4, 6], FP, tag="st", name="st")
                for sg in range(4):
                    nc.vector.bn_stats(
                        stats[:, sg, :], ob[:, sg * 512:(sg + 1) * 512]
                    )
                mv = lnp.tile([128, 2], FP, tag="mv", name="mv")
                nc.vector.bn_aggr(mv[:], stats[:])
                std = lnp.tile([128, 1], FP, tag="sd", name="sd")
                nc.scalar.activation(
                    std[:], mv[:, 1:2], func=Sqrt, bias=eps_sb[:], scale=1.0
                )
                rstd = lnp.tile([128, 1], FP, tag="rs", name="rs")
                nc.vector.reciprocal(rstd[:], std[:])
                nc.vector.tensor_scalar(
                    ob[:],
                    ob[:],
                    scalar1=mv[:, 0:1],
                    scalar2=rstd[:],
                    op0=mybir.AluOpType.subtract,
                    op1=mybir.AluOpType.mult,
                )
                eng = nc.vector if tt == 3 else nc.gpsimd
                eng.tensor_mul(ob[:], ob[:], gam_sb[:])
                eng.tensor_add(ob[:], ob[:], bet_sb[:])
                nc.sync.dma_start(out[tt * 128:(tt + 1) * 128, :], ob[:])

            wo_blks = [
                [wo_blk0[k // 4][:, k % 4, :] for k in range(KC)],
                [wo_blk0[4 + k // 4][:, k % 4, :] for k in range(KC)],
            ]
            for nb in range(2, 4):
                blk = []
                for j in range(4):
                    t = wcp2.tile([128, 4, 512], BF, tag="wo", name="wot")
                    nc.scalar.dma_start(
                        t[:], wo4[j][:, :, nb * 512:(nb + 1) * 512]
                    )
                    blk.extend(t[:, a, :] for a in range(4))
                wo_blks.append(blk)

            for tt in range(4):
                ob = outp.tile([128, H], FP, tag="ob", name="ob")
                for nb in range(4):
                    xt = xrp.tile([128, 512], FP, tag="xr", name="xr")
                    nc.sync.dma_start(
                        xt[:],
                        xres[tt * 128:(tt + 1) * 128, nb * 512:(nb + 1) * 512],
                    )
                    ps = psC.tile([128, 512], FP, tag="psC", name="psC")
                    for k in range(KC):
                        nc.tensor.matmul(
                            ps[:],
                            ctx_sb[k][:, tt * 128:(tt + 1) * 128],
                            wo_blks[nb][k][:],
                            start=(k == 0),
                            stop=(k == KC - 1),
                        )
                    sl = slice(nb * 512, (nb + 1) * 512)
                    nc.vector.tensor_add(ob[:, sl], ps[:], xt[:])
                _ln(tt, ob)


        wqp_cm.__exit__(None, None, None)

    nc.compile()
    return nc


def _get_nc(use_mask: bool):
    if use_mask not in _CACHE:
        _CACHE[use_mask] = _build(use_mask)
    return _CACHE[use_mask]


def kernel(
    hidden_state,
    encoder_hidden_state,
    encoder_attention_mask,
    Wq, bq, Wk, bk, Wv, bv, Wo, bo, gamma, beta,
):
    hidden_state = np.asarray(hidden_state, dtype=np.float32)
    encoder_hidden_state = np.asarray(encoder_hidden_state, dtype=np.float32)
    encoder_attention_mask = np.asarray(encoder_attention_mask, dtype=np.float32)
    Wq = np.asarray(Wq, dtype=np.float32)
    bq = np.asarray(bq, dtype=np.float32)
    Wk = np.asarray(Wk, dtype=np.float32)
    bk = np.asarray(bk, dtype=np.float32)
    Wv = np.asarray(Wv, dtype=np.float32)
    bv = np.asarray(bv, dtype=np.float32)
    Wo = np.asarray(Wo, dtype=np.float32)
    bo = np.asarray(bo, dtype=np.float32)
    gamma = np.asarray(gamma, dtype=np.float32)
    beta = np.asarray(beta, dtype=np.float32)

    use_mask = bool(np.any(encoder_attention_mask))
    nc = _get_nc(use_mask)
    in_maps = _prepare_in_maps(
        hidden_state, encoder_hidden_state, encoder_attention_mask,
        Wq, bq, Wk, bk, Wv, bv, Wo, bo, gamma, beta, use_mask,
    )

    res = run_bass_kernel_spmd(nc, in_maps, core_ids=list(range(8)))
    kernel._last_results = res

    output = np.empty((B, L, H), dtype=np.float32)
    for c in range(8):
        b, lh = c // 2, c % 2
        output[b, lh * TOK:(lh + 1) * TOK, :] = res.results[c]["out"]
    return output


def _prepare_in_maps(
    hidden_state, encoder_hidden_state, encoder_attention_mask,
    Wq, bq, Wk, bk, Wv, bv, Wo, bo, gamma, beta, use_mask,
):
    wq_bf = np.ascontiguousarray(Wq.astype(BF16))
    wk_bf = np.ascontiguousarray(Wk.astype(BF16))
    wv_bf = np.ascontiguousarray(Wv.astype(BF16))
    wo_bf = np.ascontiguousarray(Wo.astype(BF16))
    bq2 = np.ascontiguousarray(bq.reshape(KC, 128).T)
    bk2 = np.ascontiguousarray(bk.reshape(4, 128).T)
    bvr = np.ascontiguousarray(np.tile(bv[None, :], (128, 1)))
    gamr = np.ascontiguousarray(np.tile(gamma[None, :].astype(BF16), (128, 1)))
    betr = np.ascontiguousarray(np.tile(beta[None, :].astype(BF16), (128, 1)))

    eT_by_b = [
        np.ascontiguousarray(encoder_hidden_state[b].T.astype(BF16)) for b in range(B)
    ]

    in_maps = []
    for c in range(8):
        b, lh = c // 2, c % 2
        rows = hidden_state[b, lh * TOK:(lh + 1) * TOK, :]
        m = {
            "xT": np.ascontiguousarray(rows.T.astype(BF16)),
            "xres": rows + bo[None, :],
            "eT": eT_by_b[b],
            "wq": wq_bf, "wk": wk_bf, "wv": wv_bf, "wo": wo_bf,
            "bq2": bq2, "bk2": bk2, "bvr": bvr,
            "gamr": gamr, "betr": betr,
        }
        if use_mask:
            mslice = encoder_attention_mask[b, 0, lh * TOK:(lh + 1) * TOK, :]
            m["maskT"] = np.ascontiguousarray((mslice.T * 8.0).astype(BF16))
        in_maps.append(m)
    return in_maps



# revision 14
# speedup vs baseline: 1.4701x; 1.4701x over previous
"""Encoder-decoder GQA attention block (B=4, L=S=1024, H=2048, 32 Q heads,
8 KV heads, head_dim 64) + output projection + residual + layernorm, on 8
Trainium2 NeuronCores.

Sharding: rows. Core c handles batch c//2, L-half c%2 (512 query rows).
K/V projections are computed per-batch (duplicated on the 2 cores sharing a
batch), attention over all 32 heads for the core's rows, output projection,
residual + LN. No collectives.

v2: all large matmuls run fp8(e4m3) with perf_mode=DoubleRow (0.5 cyc/row,
two 128-deep K slices per instruction -> 4x the bf16 MAC rate). Weights are
pre-scaled x32 on the host so their ~N(0, 0.02) entries land in e4m3's
normal range; activations stay unscaled (~N(0,1)). Scale bookkeeping:
  q,k carry x32 each -> scores PSUM x1024 -> exp scale 2^-13 (=0.125/1024)
  exp is shifted by bias=-2 (values ~e^[-9,3]) to sit in e4m3 range; the
  V "ones column" is 1/32 so rowsum carries x(1/32) while ctx carries v's
  x32 -> ctx = 32*ctx_true, exactly what fp8 O-proj wants.
  O-proj PSUM = 1024*(ctx@Wo); residual is pre-scaled x1024 on the host and
  layernorm is scale-invariant (eps shift negligible), so no unscale op.
Scores keep K=64 contraction: DoubleRow's second K-slice is a zero pad in
kT (j=1 memset once per buffer), so the pair contributes k.T@q + 0*junk.
attn@V pairs s-chunks: exp tiles are [128s, 2sc, 512t], V is [128s, 4sc,
8h, 72] (72 = 64+1 rowsum col + 7 pad so the DoubleRow pair stride is
16B-aligned).

Engine budget (cost model): ACT exp is the pole (~133us: 128 ops x
[128,2,512]); PE ~90us total (fp8-DR everywhere, bf16 only for the K=1
recip broadcast); DVE ~90us (PSUM->SBUF casts, recip, ctx mul, LN stats);
Pool takes the zero-pad memsets, residual adds and the LN normalize mul;
DMA ~60us serialized. Schedule: scores for heads 0-7 (s-lo) interleave
with the K/V projections in phase A so ACT starts ~12us in; attn@V lags
one head behind exp; Wq streams in 512-col blocks, Wo prefetches late in
phase B; phase C is dense PE (O-proj) with per-row-block LN + store.

LayerNorm gamma/beta ops and their DMAs are skipped when gamma==1 and
beta==0 (detected at runtime; separate build variant), same for the mask.
"""

from contextlib import ExitStack

import numpy as np
import ml_dtypes

import concourse.bass as bass  # noqa: F401  (bass.AP used via handles)
import concourse.mybir as mybir
import concourse.tile as tile
from concourse import bacc
from concourse.bass_utils import run_bass_kernel_spmd

BF16 = ml_dtypes.bfloat16
FP8 = ml_dtypes.float8_e4m3fn

H = 2048
NH = 32
KVH = 8
G = 4           # query-head groups per kv head
HD = 64
B, L, S = 4, 1024, 1024
TOK = 512       # decoder rows per core
KC = 8          # contraction chunk-pairs (8 x (2x128) = 2048)
SC = S // 128   # 8 s chunks
EPS = 1e-6
WS = 32.0       # host-side fp8 weight scale

FP = mybir.dt.float32
BF = mybir.dt.bfloat16
F8 = mybir.dt.float8e4
DR = mybir.MatmulPerfMode.DoubleRow

_CACHE: dict = {}


def _build(use_mask: bool, triv_ln: bool):
    nc = bacc.Bacc("TRN2", target_bir_lowering=False)

    xT = nc.dram_tensor("xT", [128, KC, 2, TOK], F8, kind="ExternalInput")
    xres = nc.dram_tensor("xres", [TOK, H], FP, kind="ExternalInput")
    eT = nc.dram_tensor("eT", [128, KC, 2, S], F8, kind="ExternalInput")
    wq = nc.dram_tensor("wq", [128, KC, 2, H], F8, kind="ExternalInput")
    wk = nc.dram_tensor("wk", [128, KC, 2, 512], F8, kind="ExternalInput")
    wv = nc.dram_tensor("wv", [128, KC, 2, 512], F8, kind="ExternalInput")
    wo = nc.dram_tensor("wo", [128, KC, 2, H], F8, kind="ExternalInput")
    bq2 = nc.dram_tensor("bq2", [128, 16], FP, kind="ExternalInput")
    bk2 = nc.dram_tensor("bk2", [128, 4], FP, kind="ExternalInput")
    bvr = nc.dram_tensor("bvr", [128, KVH * HD], FP, kind="ExternalInput")
    if not triv_ln:
        gamr = nc.dram_tensor("gamr", [128, H], BF, kind="ExternalInput")
        betr = nc.dram_tensor("betr", [128, H], BF, kind="ExternalInput")
    if use_mask:
        maskT = nc.dram_tensor("maskT", [S, TOK], BF, kind="ExternalInput")
    out = nc.dram_tensor("out", [TOK, H], FP, kind="ExternalOutput")

    Exp = mybir.ActivationFunctionType.Exp
    Sqrt = mybir.ActivationFunctionType.Sqrt

    with tile.TileContext(nc) as tc:
      with (
          tc.tile_pool(name="ctxT", bufs=1) as ctxp,
          tc.tile_pool(name="cc", bufs=1) as ccp,
          tc.tile_pool(name="ln", bufs=10) as lnp,
          tc.tile_pool(name="qT", bufs=NH) as qtp,
          tc.tile_pool(name="kT", bufs=KVH) as ktp,
          tc.tile_pool(name="vv", bufs=2) as vvp,
          tc.tile_pool(name="expp", bufs=16) as expp,
          tc.tile_pool(name="rec", bufs=4) as recp,
          tc.tile_pool(name="bc", bufs=4) as bcp,
          tc.tile_pool(name="const", bufs=1) as constp,
          tc.tile_pool(name="wq", bufs=2) as wqp,
          tc.tile_pool(name="xTp", bufs=1) as xtp,
          tc.tile_pool(name="wo", bufs=4) as wop,
          tc.tile_pool(name="maskp", bufs=SC if use_mask else 1) as mkp,
      ):
        eps_sb = ccp.tile([128, 1], FP, name="eps_sb")
        ones_sb = ccp.tile([1, 64], BF, name="ones_sb")
        nb2_sb = ccp.tile([128, 1], FP, name="nb2_sb")
        nc.vector.memset(eps_sb[:], EPS)
        nc.vector.memset(ones_sb[:], 1.0)
        nc.vector.memset(nb2_sb[:], -2.0)
        if not triv_ln:
            gam_sb = ccp.tile([128, H], BF, name="gam_sb")
            bet_sb = ccp.tile([128, H], BF, name="bet_sb")

        ctx_sb = ctxp.tile([128, 16, TOK], F8, name="ctx")
        # qt/kt hold the DoubleRow K-pair on dim1; slice [:,1,:] is a zero
        # pad (memset once per buffer below) so K=64 contractions are legal.
        qT_sb = [qtp.tile([64, 2, TOK], F8, name="qt") for _ in range(NH)]
        kT_sb = [ktp.tile([64, 2, S], F8, name="kt") for _ in range(KVH)]
        # V: [s-part, sc-in-quad, kv-head, 64+1(rowsum)+7(pad to 16B)]
        vv_sb = [vvp.tile([128, 4, KVH, 72], F8, name="vv") for _ in range(2)]

        for t in qT_sb:
            nc.gpsimd.memset(t[:, 1, :], 0.0)
        for t in kT_sb:
            nc.gpsimd.memset(t[:, 1, :], 0.0)
        for t in vv_sb:
            nc.gpsimd.memset(t[:, :, :, 64:65], 1.0)

        _stk = ExitStack()
        psA = _stk.enter_context(tc.tile_pool(name="psA", bufs=2, space="PSUM"))
        psS = _stk.enter_context(tc.tile_pool(name="psS", bufs=2, space="PSUM"))

        bq_sb = constp.tile([128, 16], FP, name="bq_sb")
        bk_sb = constp.tile([128, 4], FP, name="bk_sb")
        bv_sb = constp.tile([128, KVH * HD], FP, name="bv_sb")

        def q_proj(m):
            blk, base = wq_blk[0]
            q = m - base
            assert 0 <= q < 4, (m, base)
            ps = psA.tile([128, TOK], FP, tag="psA", name="psA")
            for c in range(KC):
                nc.tensor.matmul(
                    ps[:],
                    blk[:, c, :, q * 128:(q + 1) * 128],
                    xt_t[:, c],
                    start=(c == 0),
                    stop=(c == KC - 1),
                    perf_mode=DR,
                )
            with nc.allow_low_precision(reason="q cast to fp8 for scores"):
                for hi in range(2):
                    nc.vector.tensor_scalar_add(
                        qT_sb[2 * m + hi][:, 0, :],
                        ps[hi * 64:hi * 64 + 64, :],
                        bq_sb[hi * 64:hi * 64 + 64, m:m + 1],
                    )

        def scores_quad(hh, qd):
            h = hh // G
            ps = psS.tile([128, 2, TOK], FP, tag="psS", name="psS")
            for i in range(2):
                sc = 2 * qd + i
                nc.tensor.matmul(
                    ps[:, i, :],
                    kT_sb[h][:, :, sc * 128:(sc + 1) * 128],
                    qT_sb[hh][:],
                    start=True,
                    stop=True,
                    perf_mode=DR,
                )
                if use_mask:
                    nc.vector.tensor_add(ps[:, i, :], ps[:, i, :],
                                         mask_sb[sc][:])
            ex = expp.tile([128, 2, TOK], F8, tag="ex", name="ex")
            nc.scalar.activation(ex[:], ps[:], func=Exp,
                                 scale=1.0 / 8192.0, bias=nb2_sb[:])
            return ex

        def attn_v(hh, exs):
            h = hh // G
            po = psO.tile([128, TOK], FP, tag="psO", name="psO")
            for qd in range(4):
                vq = vv_sb[qd // 2][:, (qd % 2) * 2:(qd % 2) * 2 + 2, h, 0:65]
                nc.tensor.matmul(
                    po[0:65, :],
                    vq,
                    exs[qd][:],
                    start=(qd == 0),
                    stop=(qd == 3),
                    perf_mode=DR,
                )
            recb = recp.tile([1, TOK], BF, tag="recb", name="recb")
            with nc.allow_low_precision(reason="softmax recip rounds to bf16"):
                nc.vector.reciprocal(recb[:], po[64:65, :])
            # broadcast recip across 64 partitions on the idle Pool engine
            rb = bcp.tile([64, TOK], BF, tag="rb", name="rb")
            nc.gpsimd.partition_broadcast(rb[:], recb[:])
            with nc.allow_low_precision(reason="ctx cast to fp8 for O-proj"):
                nc.vector.tensor_mul(
                    ctx_sb[(hh % 2) * 64:(hh % 2) * 64 + 64, hh // 2, :],
                    po[0:64, :],
                    rb[:],
                )

        # ---- Phase A: input DMAs, K/V/Q projections, s-lo scores ---------
        with (
            tc.tile_pool(name="eTp", bufs=1) as etp,
            tc.tile_pool(name="wk", bufs=1) as wkp,
            tc.tile_pool(name="wv", bufs=1) as wvp,
            tc.tile_pool(name="psKV", bufs=2, space="PSUM") as psKV,
        ):
            wk_t = wkp.tile([128, KC, 2, 512], F8, name="wkt")
            et_t = etp.tile([128, KC, 2, S], F8, name="et")
            wv_t = wvp.tile([128, KC, 2, 512], F8, name="wvt")
            xt_t = xtp.tile([128, KC, 2, TOK], F8, name="xt")
            for c in range(KC):
                nc.sync.dma_start(wk_t[:, c], wk[:, c])
                nc.sync.dma_start(et_t[:, c, :, 0:512], eT[:, c, :, 0:512])
            nc.sync.dma_start(bq_sb[:], bq2[:])
            nc.sync.dma_start(bk_sb[:], bk2[:])
            nc.sync.dma_start(bv_sb[:], bvr[:])
            nc.sync.dma_start(xt_t[:], xT[:])
            t = wqp.tile([128, KC, 2, 512], F8, tag="wq", name="wqt")
            nc.scalar.dma_start(t[:], wq[:, :, :, 0:512])
            wq_blk = [(t, 0)]
            for c in range(KC):
                nc.sync.dma_start(et_t[:, c, :, 512:S], eT[:, c, :, 512:S])
            nc.sync.dma_start(wv_t[:], wv[:])
            if use_mask:
                mask_sb = []
                for sc in range(SC):
                    mt = mkp.tile([128, TOK], BF, tag="mk", name="mk")
                    nc.sync.dma_start(mt[:], maskT[sc * 128:(sc + 1) * 128, :])
                    mask_sb.append(mt)

            def k_proj(sh, ms):
                pss = [psKV.tile([128, 512], FP, tag="psKV", name="psKV")
                       for _ in ms]
                for c in range(KC):
                    for i, m in enumerate(ms):
                        nc.tensor.matmul(
                            pss[i][:],
                            wk_t[:, c, :, m * 128:(m + 1) * 128],
                            et_t[:, c, :, sh * 512:(sh + 1) * 512],
                            start=(c == 0),
                            stop=(c == KC - 1),
                            perf_mode=DR,
                        )
                with nc.allow_low_precision(reason="k cast to fp8"):
                    for i, m in enumerate(ms):
                        for hi in range(2):
                            nc.vector.tensor_scalar_add(
                                kT_sb[2 * m + hi][:, 0,
                                                  sh * 512:(sh + 1) * 512],
                                pss[i][hi * 64:hi * 64 + 64, :],
                                bk_sb[hi * 64:hi * 64 + 64, m:m + 1],
                            )

            def v_proj(scs):
                pss = [psKV.tile([128, 512], FP, tag="psKV", name="psKV")
                       for _ in scs]
                for c in range(KC):
                    for i, sc in enumerate(scs):
                        nc.tensor.matmul(
                            pss[i][:],
                            et_t[:, c, :, sc * 128:(sc + 1) * 128],
                            wv_t[:, c],
                            start=(c == 0),
                            stop=(c == KC - 1),
                            perf_mode=DR,
                        )
                with nc.allow_low_precision(reason="v cast to fp8"):
                    for i, sc in enumerate(scs):
                        nc.vector.tensor_add(
                            vv_sb[sc // 4][:, sc % 4, :, 0:64],
                            pss[i].rearrange("p (h d) -> p h d", d=HD),
                            bv_sb.rearrange("p (h d) -> p h d", d=HD),
                        )

            # interleave s-lo scores with the projections so ACT (the pole)
            # starts as soon as kT h0/h1 and qT 0-3 exist
            stash = {}
            k_proj(0, (0, 1))
            q_proj(0)
            q_proj(1)
            stash[(0, 0)] = scores_quad(0, 0)
            stash[(0, 1)] = scores_quad(0, 1)
            k_proj(0, (2, 3))
            stash[(1, 0)] = scores_quad(1, 0)
            stash[(1, 1)] = scores_quad(1, 1)
            v_proj((0, 1))
            q_proj(2)
            q_proj(3)
            stash[(2, 0)] = scores_quad(2, 0)
            stash[(2, 1)] = scores_quad(2, 1)
            v_proj((2, 3))
            stash[(3, 0)] = scores_quad(3, 0)
            stash[(3, 1)] = scores_quad(3, 1)
            k_proj(1, (0, 1))
            stash[(4, 0)] = scores_quad(4, 0)
            stash[(4, 1)] = scores_quad(4, 1)
            k_proj(1, (2, 3))
            v_proj((4, 5))
            v_proj((6, 7))

        # psKV released; open attention output pool in its banks
        psO = _stk.enter_context(tc.tile_pool(name="psO", bufs=2, space="PSUM"))

        wo_blks = []

        # ---- Phase B: per-head scores+exp / lagged attn@V ----------------
        pending = None
        for hh in range(NH):
            if hh % 2 == 0 and 2 <= hh <= 24:
                m = 4 + (hh - 2) // 2
                if m % 4 == 0:  # new 512-col Wq block
                    t = wqp.tile([128, KC, 2, 512], F8, tag="wq", name="wqt")
                    nc.scalar.dma_start(
                        t[:], wq[:, :, :, (m // 4) * 512:(m // 4 + 1) * 512])
                    wq_blk[0] = (t, m)
                q_proj(m)
            if hh >= 24 and hh % 2 == 0:  # prefetch Wo blocks
                nb = (hh - 24) // 2
                t = wop.tile([128, KC, 2, 512], F8, tag="wo", name="wot")
                nc.scalar.dma_start(
                    t[:], wo[:, :, :, nb * 512:(nb + 1) * 512])
                wo_blks.append(t)
            exs = []
            for qd in range(4):
                if qd < 2 and (hh, qd) in stash:
                    exs.append(stash.pop((hh, qd)))
                else:
                    exs.append(scores_quad(hh, qd))
            if pending is not None:
                attn_v(*pending)
            pending = (hh, exs)
        attn_v(*pending)

        _stk.close()

        # ---- Phase C: output projection + residual + layernorm -----------
        with (
            tc.tile_pool(name="psC", bufs=4, space="PSUM") as psC,
            tc.tile_pool(name="xr", bufs=6) as xrp,
            tc.tile_pool(name="outp", bufs=2) as outp,
        ):
            if not triv_ln:
                nc.sync.dma_start(gam_sb[:], gamr[:])
                nc.sync.dma_start(bet_sb[:], betr[:])

            def _ln(tt, ob):
                stats = lnp.tile([128, 4, 6], FP, tag="st", name="st")
                for sg in range(4):
                    nc.vector.bn_stats(
                        stats[:, sg, :], ob[:, sg * 512:(sg + 1) * 512]
                    )
                mv = lnp.tile([128, 2], FP, tag="mv", name="mv")
                nc.vector.bn_aggr(mv[:], stats[:])
                std = lnp.tile([128, 1], FP, tag="sd", name="sd")
                nc.scalar.activation(
                    std[:], mv[:, 1:2], func=Sqrt, bias=eps_sb[:], scale=1.0
                )
                rstd = lnp.tile([128, 1], FP, tag="rs", name="rs")
                nc.vector.reciprocal(rstd[:], std[:])
                nc.gpsimd.tensor_scalar(
                    ob[:],
                    ob[:],
                    scalar1=mv[:, 0:1],
                    scalar2=rstd[:],
                    op0=mybir.AluOpType.subtract,
                    op1=mybir.AluOpType.mult,
                )
                if not triv_ln:
                    nc.gpsimd.tensor_mul(ob[:], ob[:], gam_sb[:])
                    nc.gpsimd.tensor_add(ob[:], ob[:], bet_sb[:])
                nc.sync.dma_start(out[tt * 128:(tt + 1) * 128, :], ob[:])

            for tt in range(4):
                ob = outp.tile([128, H], FP, tag="ob", name="ob")
                for nb in range(4):
                    xt = xrp.tile([128, 512], FP, tag="xr", name="xr")
                    nc.sync.dma_start(
                        xt[:],
                        xres[tt * 128:(tt + 1) * 128, nb * 512:(nb + 1) * 512],
                    )
                    ps = psC.tile([128, 512], FP, tag="psC", name="psC")
                    for c in range(KC):
                        nc.tensor.matmul(
                            ps[:],
                            ctx_sb[:, 2 * c:2 * c + 2, tt * 128:(tt + 1) * 128],
                            wo_blks[nb][:, c],
                            start=(c == 0),
                            stop=(c == KC - 1),
                            perf_mode=DR,
                        )
                    sl = slice(nb * 512, (nb + 1) * 512)
                    nc.vector.tensor_add(ob[:, sl], ps[:], xt[:])
                _ln(tt, ob)

    nc.compile()
    return nc


def _get_nc(use_mask: bool, triv_ln: bool | None = None):
    if triv_ln is None:
        triv_ln = _LAST_TRIV[0]
    key = (use_mask, triv_ln)
    if key not in _CACHE:
        _CACHE[key] = _build(use_mask, triv_ln)
    return _CACHE[key]


_LAST_TRIV = [True]


def _pack_w(w, cols):
    """[2048, cols] fp32 -> [128, 8, 2, cols] fp8 with K-pair layout."""
    return np.ascontiguousarray(
        (w * WS).reshape(KC, 2, 128, cols).transpose(2, 0, 1, 3)
    ).astype(FP8)


def _pack_act(xT, cols):
    """[2048, cols] fp32 (feature-major) -> [128, 8, 2, cols] fp8."""
    return np.ascontiguousarray(
        xT.reshape(KC, 2, 128, cols).transpose(2, 0, 1, 3)
    ).astype(FP8)


def kernel(
    hidden_state,
    encoder_hidden_state,
    encoder_attention_mask,
    Wq, bq, Wk, bk, Wv, bv, Wo, bo, gamma, beta,
):
    hidden_state = np.asarray(hidden_state, dtype=np.float32)
    encoder_hidden_state = np.asarray(encoder_hidden_state, dtype=np.float32)
    encoder_attention_mask = np.asarray(encoder_attention_mask, dtype=np.float32)
    Wq = np.asarray(Wq, dtype=np.float32)
    bq = np.asarray(bq, dtype=np.float32)
    Wk = np.asarray(Wk, dtype=np.float32)
    bk = np.asarray(bk, dtype=np.float32)
    Wv = np.asarray(Wv, dtype=np.float32)
    bv = np.asarray(bv, dtype=np.float32)
    Wo = np.asarray(Wo, dtype=np.float32)
    bo = np.asarray(bo, dtype=np.float32)
    gamma = np.asarray(gamma, dtype=np.float32)
    beta = np.asarray(beta, dtype=np.float32)

    use_mask = bool(np.any(encoder_attention_mask))
    triv_ln = bool(np.all(gamma == 1.0) and np.all(beta == 0.0))
    _LAST_TRIV[0] = triv_ln
    nc = _get_nc(use_mask, triv_ln)
    in_maps = _prepare_in_maps(
        hidden_state, encoder_hidden_state, encoder_attention_mask,
        Wq, bq, Wk, bk, Wv, bv, Wo, bo, gamma, beta, use_mask,
    )

    res = run_bass_kernel_spmd(nc, in_maps, core_ids=list(range(8)))
    kernel._last_results = res

    output = np.empty((B, L, H), dtype=np.float32)
    for c in range(8):
        b, lh = c // 2, c % 2
        output[b, lh * TOK:(lh + 1) * TOK, :] = res.results[c]["out"]
    return output


def _prepare_in_maps(
    hidden_state, encoder_hidden_state, encoder_attention_mask,
    Wq, bq, Wk, bk, Wv, bv, Wo, bo, gamma, beta, use_mask,
):
    triv_ln = bool(np.all(gamma == 1.0) and np.all(beta == 0.0))
    wq_p = _pack_w(Wq, H)
    wk_p = _pack_w(Wk, KVH * HD)
    wv_p = _pack_w(Wv, KVH * HD)
    wo_p = _pack_w(Wo, H)
    bq2 = np.ascontiguousarray(bq.reshape(16, 128).T) * WS
    bk2 = np.ascontiguousarray(bk.reshape(4, 128).T) * WS
    bvr = np.ascontiguousarray(np.tile(bv[None, :], (128, 1))) * WS
    if not triv_ln:
        gamr = np.ascontiguousarray(
            np.tile(gamma[None, :].astype(BF16), (128, 1)))
        betr = np.ascontiguousarray(
            np.tile(beta[None, :].astype(BF16), (128, 1)))

    eT_by_b = [
        _pack_act(encoder_hidden_state[b].T, S) for b in range(B)
    ]

    in_maps = []
    for c in range(8):
        b, lh = c // 2, c % 2
        rows = hidden_state[b, lh * TOK:(lh + 1) * TOK, :]
        m = {
            "xT": _pack_act(rows.T, TOK),
            "xres": (rows + bo[None, :]) * (WS * WS),
            "eT": eT_by_b[b],
            "wq": wq_p, "wk": wk_p, "wv": wv_p, "wo": wo_p,
            "bq2": bq2, "bk2": bk2, "bvr": bvr,
        }
        if not triv_ln:
            m["gamr"] = gamr
            m["betr"] = betr
        if use_mask:
            mslice = encoder_attention_mask[b, 0, lh * TOK:(lh + 1) * TOK, :]
            m["maskT"] = np.ascontiguousarray(
                (mslice.T * (8.0 * WS * WS)).astype(BF16))
        in_maps.append(m)
    return in_maps


# revision 32
# speedup vs baseline: 1.5965x; 1.0860x over previous
"""Encoder-decoder GQA attention block (B=4, L=S=1024, H=2048, 32 Q heads,
8 KV heads, head_dim 64) + output projection + residual + layernorm, on 8
Trainium2 NeuronCores.

Sharding: rows. Core c handles batch c//2, L-half c%2 (512 query rows).
K/V projections are computed per-batch (duplicated on the 2 cores sharing a
batch), attention over all 32 heads for the core's rows, output projection,
residual + LN. No collectives.

v2: all large matmuls run fp8(e4m3) with perf_mode=DoubleRow (0.5 cyc/row,
two 128-deep K slices per instruction -> 4x the bf16 MAC rate). Weights are
pre-scaled x32 on the host so their ~N(0, 0.02) entries land in e4m3's
normal range; activations stay unscaled (~N(0,1)). Scale bookkeeping:
  q,k carry x32 each -> scores PSUM x1024 -> exp scale 2^-13 (=0.125/1024)
  exp is shifted by bias=-2 (values ~e^[-9,3]) to sit in e4m3 range; the
  V "ones column" is 1/32 so rowsum carries x(1/32) while ctx carries v's
  x32 -> ctx = 32*ctx_true, exactly what fp8 O-proj wants.
  O-proj PSUM = 1024*(ctx@Wo); residual is pre-scaled x1024 on the host and
  layernorm is scale-invariant (eps shift negligible), so no unscale op.
Scores keep K=64 contraction: DoubleRow's second K-slice is a zero pad in
kT (j=1 memset once per buffer), so the pair contributes k.T@q + 0*junk.
attn@V pairs s-chunks: exp tiles are [128s, 2sc, 512t], V is [128s, 4sc,
8h, 72] (72 = 64+1 rowsum col + 7 pad so the DoubleRow pair stride is
16B-aligned).

Engine budget (cost model): ACT exp is the pole (~133us: 128 ops x
[128,2,512]); PE ~90us total (fp8-DR everywhere, bf16 only for the K=1
recip broadcast); DVE ~90us (PSUM->SBUF casts, recip, ctx mul, LN stats);
Pool takes the zero-pad memsets, residual adds and the LN normalize mul;
DMA ~60us serialized. Schedule: scores for heads 0-7 (s-lo) interleave
with the K/V projections in phase A so ACT starts ~12us in; attn@V lags
one head behind exp; Wq streams in 512-col blocks, Wo prefetches late in
phase B; phase C is dense PE (O-proj) with per-row-block LN + store.

LayerNorm gamma/beta ops and their DMAs are skipped when gamma==1 and
beta==0 (detected at runtime; separate build variant), same for the mask.
"""

from contextlib import ExitStack

import numpy as np
import ml_dtypes

import concourse.bass as bass  # noqa: F401  (bass.AP used via handles)
import concourse.mybir as mybir
import concourse.tile as tile
from concourse import bacc
from concourse.bass_utils import run_bass_kernel_spmd

BF16 = ml_dtypes.bfloat16
FP8 = ml_dtypes.float8_e4m3fn

H = 2048
NH = 32
KVH = 8
G = 4           # query-head groups per kv head
HD = 64
B, L, S = 4, 1024, 1024
TOK = 512       # decoder rows per core
KC = 8          # contraction chunk-pairs (8 x (2x128) = 2048)
SC = S // 128   # 8 s chunks
EPS = 1e-6
WS = 32.0       # host-side fp8 weight scale

FP = mybir.dt.float32
BF = mybir.dt.bfloat16
F8 = mybir.dt.float8e4
DR = mybir.MatmulPerfMode.DoubleRow

_CACHE: dict = {}


def _build(use_mask: bool, triv_ln: bool, triv_bias: bool):
    nc = bacc.Bacc("TRN2", target_bir_lowering=False)

    xT = nc.dram_tensor("xT", [128, KC, 2, TOK], F8, kind="ExternalInput")
    xres = nc.dram_tensor("xres", [TOK, H], BF, kind="ExternalInput")
    eT = nc.dram_tensor("eT", [128, KC, 2, S], F8, kind="ExternalInput")
    wq = nc.dram_tensor("wq", [128, KC, 2, H], F8, kind="ExternalInput")
    wk = nc.dram_tensor("wk", [128, KC, 2, 512], F8, kind="ExternalInput")
    wv = nc.dram_tensor("wv", [128, KC, 2, 512], F8, kind="ExternalInput")
    wo = nc.dram_tensor("wo", [128, KC, 2, H], F8, kind="ExternalInput")
    # packed biases: [bq(16) | bk(4) | bv(512)] -> one DMA
    if not triv_bias:
        bias_all = nc.dram_tensor("bias_all", [128, 532], FP,
                                  kind="ExternalInput")
    ident = nc.dram_tensor("ident", [128, 128], BF, kind="ExternalInput")
    if not triv_ln:
        gamr = nc.dram_tensor("gamr", [128, H], BF, kind="ExternalInput")
        betr = nc.dram_tensor("betr", [128, H], BF, kind="ExternalInput")
    if use_mask:
        maskT = nc.dram_tensor("maskT", [S, TOK], BF, kind="ExternalInput")
    out = nc.dram_tensor("out", [TOK, H], FP, kind="ExternalOutput")

    Exp = mybir.ActivationFunctionType.Exp
    Ident = mybir.ActivationFunctionType.Identity
    Sqrt = mybir.ActivationFunctionType.Sqrt

    with tile.TileContext(nc) as tc:
      with (
          tc.tile_pool(name="ctxT", bufs=1) as ctxp,
          tc.tile_pool(name="cc", bufs=1) as ccp,
          tc.tile_pool(name="ln", bufs=10) as lnp,
          tc.tile_pool(name="qT", bufs=NH) as qtp,
          tc.tile_pool(name="kT", bufs=KVH) as ktp,
          tc.tile_pool(name="vv", bufs=2) as vvp,
          tc.tile_pool(name="expp", bufs=12) as expp,
          tc.tile_pool(name="rec", bufs=4) as recp,
          tc.tile_pool(name="bc", bufs=4) as bcp,
          tc.tile_pool(name="const", bufs=1) as constp,
          tc.tile_pool(name="wq", bufs=2) as wqp,
          tc.tile_pool(name="xTp", bufs=1) as xtp,
          tc.tile_pool(name="wo", bufs=2) as wop,
          tc.tile_pool(name="xr", bufs=4) as xrp,
          tc.tile_pool(name="maskp", bufs=SC if use_mask else 1) as mkp,
      ):
        eps_sb = ccp.tile([128, 1], FP, name="eps_sb")
        nb2_sb = ccp.tile([128, 1], FP, name="nb2_sb")
        junk_sb = ccp.tile([128, 1], FP, name="junk_sb")
        nc.vector.memset(eps_sb[:], EPS)
        nc.vector.memset(nb2_sb[:], -2.0)
        # touch the Sqrt act table now so its load isn't on the phase-C chain
        nc.scalar.activation(junk_sb[:], eps_sb[:], func=Sqrt,
                             bias=eps_sb[:], scale=1.0)
        if not triv_ln:
            gam_sb = ccp.tile([128, H], BF, name="gam_sb")
            bet_sb = ccp.tile([128, H], BF, name="bet_sb")

        ctx_sb = ctxp.tile([128, 16, TOK], F8, name="ctx")
        # qt/kt hold the DoubleRow K-pair on dim1; slice [:,1,:] is a zero
        # pad (memset once per buffer below) so K=64 contractions are legal.
        qT_sb = [qtp.tile([64, 2, TOK], F8, name="qt") for _ in range(NH)]
        kT_sb = [ktp.tile([64, 2, S], F8, name="kt") for _ in range(KVH)]
        # V: [s-part, sc-in-quad, kv-head, 64+1(rowsum)+7(pad to 16B)]
        vv_sb = [vvp.tile([128, 4, KVH, 72], F8, name="vv") for _ in range(2)]

        for t in qT_sb:
            nc.gpsimd.memset(t[:, 1, :], 0.0)
        for t in kT_sb:
            nc.gpsimd.memset(t[:, 1, :], 0.0)
        for t in vv_sb:
            nc.gpsimd.memset(t[:, :, :, 64:65], 1.0)

        _stk = ExitStack()
        psA = _stk.enter_context(tc.tile_pool(name="psA", bufs=2, space="PSUM"))
        psS = _stk.enter_context(tc.tile_pool(name="psS", bufs=2, space="PSUM"))

        id_sb = constp.tile([128, 128], BF, name="id_sb")
        if not triv_bias:
            bias_sb = constp.tile([128, 532], FP, name="bias_sb")
            bq_sb = bias_sb[:, 0:16]
            bk_sb = bias_sb[:, 16:20]
            bv_sb = bias_sb[:, 20:532]

        def q_proj(m):
            blk, base = wq_blk[0]
            q = m - base
            assert 0 <= q < 4, (m, base)
            ps = psA.tile([128, TOK], FP, tag="psA", name="psA")
            for c in range(KC):
                nc.tensor.matmul(
                    ps[:],
                    blk[:, c, :, q * 128:(q + 1) * 128],
                    xt_t[:, c],
                    start=(c == 0),
                    stop=(c == KC - 1),
                    perf_mode=DR,
                )
            with nc.allow_low_precision(reason="q cast to fp8 for scores"):
                for hi in range(2):
                    if triv_bias:
                        nc.vector.tensor_copy(
                            qT_sb[2 * m + hi][:, 0, :],
                            ps[hi * 64:hi * 64 + 64, :],
                        )
                    else:
                        nc.vector.tensor_scalar_add(
                            qT_sb[2 * m + hi][:, 0, :],
                            ps[hi * 64:hi * 64 + 64, :],
                            bq_sb[hi * 64:hi * 64 + 64, m:m + 1],
                        )

        def scores_quad(hh, qd):
            h = hh // G
            ps = psS.tile([128, 2, TOK], FP, tag="psS", name="psS")
            for i in range(2):
                sc = 2 * qd + i
                nc.tensor.matmul(
                    ps[:, i, :],
                    kT_sb[h][:, :, sc * 128:(sc + 1) * 128],
                    qT_sb[hh][:],
                    start=True,
                    stop=True,
                    perf_mode=DR,
                )
                if use_mask:
                    nc.vector.tensor_add(ps[:, i, :], ps[:, i, :],
                                         mask_sb[sc][:])
            ex = expp.tile([128, 2, TOK], F8, tag="ex", name="ex")
            nc.scalar.activation(ex[:], ps[:], func=Exp,
                                 scale=1.0 / 8192.0, bias=nb2_sb[:])
            return ex

        def attn_v(hh, exs):
            h = hh // G
            po = psO.tile([128, TOK], FP, tag="psO", name="psO")
            for qd in range(4):
                vq = vv_sb[qd // 2][:, (qd % 2) * 2:(qd % 2) * 2 + 2, h, 0:65]
                nc.tensor.matmul(
                    po[0:65, :],
                    vq,
                    exs[qd][:],
                    start=(qd == 0),
                    stop=(qd == 3),
                    perf_mode=DR,
                )
            recb = recp.tile([1, TOK], BF, tag="recb", name="recb")
            with nc.allow_low_precision(reason="softmax recip rounds to bf16"):
                nc.vector.reciprocal(recb[:], po[64:65, :])
            # broadcast recip across 64 partitions on the idle Pool engine
            rb = bcp.tile([64, TOK], BF, tag="rb", name="rb")
            nc.gpsimd.partition_broadcast(rb[:], recb[:])
            with nc.allow_low_precision(reason="ctx cast to fp8 for O-proj"):
                nc.vector.tensor_mul(
                    ctx_sb[(hh % 2) * 64:(hh % 2) * 64 + 64, hh // 2, :],
                    po[0:64, :],
                    rb[:],
                )

        # ---- Phase A: input DMAs, K/V/Q projections, s-lo scores ---------
        with (
            tc.tile_pool(name="eTp", bufs=1) as etp,
            tc.tile_pool(name="wk", bufs=1) as wkp,
            tc.tile_pool(name="wv", bufs=1) as wvp,
            tc.tile_pool(name="psKV", bufs=2, space="PSUM") as psKV,
        ):
            wk_t = wkp.tile([128, KC, 2, 512], F8, name="wkt")
            et_t = etp.tile([128, KC, 2, S], F8, name="et")
            wv_t = wvp.tile([128, KC, 2, 512], F8, name="wvt")
            xt_t = xtp.tile([128, KC, 2, TOK], F8, name="xt")
            # few big DMAs: descriptor-gen (HWDGE) is a serial shared
            # resource (~0.63us per DMA instruction), so chunking loses
            nc.sync.dma_start(wk_t[:], wk[:])
            nc.sync.dma_start(et_t[:, :, :, 0:512], eT[:, :, :, 0:512])
            if not triv_bias:
                nc.sync.dma_start(bias_sb[:], bias_all[:])
            nc.sync.dma_start(xt_t[:], xT[:])
            t = wqp.tile([128, KC, 2, 512], F8, tag="wq", name="wqt")
            nc.scalar.dma_start(t[:, :, :, 0:256], wq[:, :, :, 0:256])
            nc.scalar.dma_start(t[:, :, :, 256:512], wq[:, :, :, 256:512])
            nc.scalar.dma_start(id_sb[:], ident[:])
            wq_blk = [(t, 0)]
            nc.sync.dma_start(et_t[:, :, :, 512:S], eT[:, :, :, 512:S])
            nc.sync.dma_start(wv_t[:], wv[:])
            if use_mask:
                mask_sb = []
                for sc in range(SC):
                    mt = mkp.tile([128, TOK], BF, tag="mk", name="mk")
                    nc.sync.dma_start(mt[:], maskT[sc * 128:(sc + 1) * 128, :])
                    mask_sb.append(mt)

            def k_proj(sh, ms):
                pss = [psKV.tile([128, 512], FP, tag="psKV", name="psKV")
                       for _ in ms]
                for c in range(KC):
                    for i, m in enumerate(ms):
                        nc.tensor.matmul(
                            pss[i][:],
                            wk_t[:, c, :, m * 128:(m + 1) * 128],
                            et_t[:, c, :, sh * 512:(sh + 1) * 512],
                            start=(c == 0),
                            stop=(c == KC - 1),
                            perf_mode=DR,
                        )
                with nc.allow_low_precision(reason="k cast to fp8"):
                    for i, m in enumerate(ms):
                        for hi in range(2):
                            dst = kT_sb[2 * m + hi][:, 0,
                                                    sh * 512:(sh + 1) * 512]
                            if triv_bias:
                                nc.vector.tensor_copy(
                                    dst, pss[i][hi * 64:hi * 64 + 64, :])
                            else:
                                nc.vector.tensor_scalar_add(
                                    dst,
                                    pss[i][hi * 64:hi * 64 + 64, :],
                                    bk_sb[hi * 64:hi * 64 + 64, m:m + 1],
                                )

            def v_proj(scs):
                pss = [psKV.tile([128, 512], FP, tag="psKV", name="psKV")
                       for _ in scs]
                for c in range(KC):
                    for i, sc in enumerate(scs):
                        nc.tensor.matmul(
                            pss[i][:],
                            et_t[:, c, :, sc * 128:(sc + 1) * 128],
                            wv_t[:, c],
                            start=(c == 0),
                            stop=(c == KC - 1),
                            perf_mode=DR,
                        )
                with nc.allow_low_precision(reason="v cast to fp8"):
                    for i, sc in enumerate(scs):
                        if triv_bias:
                            nc.vector.tensor_copy(
                                vv_sb[sc // 4][:, sc % 4, :, 0:64],
                                pss[i].rearrange("p (h d) -> p h d", d=HD),
                            )
                        else:
                            nc.vector.tensor_add(
                                vv_sb[sc // 4][:, sc % 4, :, 0:64],
                                pss[i].rearrange("p (h d) -> p h d", d=HD),
                                bv_sb.rearrange("p (h d) -> p h d", d=HD),
                            )

            # interleave s-lo scores with the projections so ACT (the pole)
            # starts as soon as kT h0/h1 and qT 0-3 exist
            stash = {}
            k_proj(0, (0, 1))
            q_proj(0)
            q_proj(1)
            stash[(0, 0)] = scores_quad(0, 0)
            stash[(0, 1)] = scores_quad(0, 1)
            k_proj(0, (2, 3))
            stash[(1, 0)] = scores_quad(1, 0)
            stash[(1, 1)] = scores_quad(1, 1)
            v_proj((0, 1))
            q_proj(2)
            q_proj(3)
            stash[(2, 0)] = scores_quad(2, 0)
            stash[(2, 1)] = scores_quad(2, 1)
            v_proj((2, 3))
            stash[(3, 0)] = scores_quad(3, 0)
            stash[(3, 1)] = scores_quad(3, 1)
            k_proj(1, (0, 1))
            k_proj(1, (2, 3))
            v_proj((4, 5))
            v_proj((6, 7))

        # psKV released; open attention output pool in its banks
        psO = _stk.enter_context(tc.tile_pool(name="psO", bufs=2, space="PSUM"))

        wo_blks = []
        xres_sb = []

        # ---- Phase B: per-head scores+exp / lagged attn@V ----------------
        pending = None
        for hh in range(NH):
            if hh % 2 == 0 and 2 <= hh <= 24:
                m = 4 + (hh - 2) // 2
                if m % 4 == 0:  # new 512-col Wq block
                    t = wqp.tile([128, KC, 2, 512], F8, tag="wq", name="wqt")
                    nc.scalar.dma_start(
                        t[:], wq[:, :, :, (m // 4) * 512:(m // 4 + 1) * 512])
                    wq_blk[0] = (t, m)
                q_proj(m)
            if hh in (24, 28):  # prefetch Wo halves
                half = (hh - 24) // 4
                t = wop.tile([128, KC, 2, 1024], F8, tag="wo", name="wot")
                nc.scalar.dma_start(
                    t[:], wo[:, :, :, half * 1024:(half + 1) * 1024])
                wo_blks.append(t)
            if hh in (25, 27, 29, 31):  # prefetch residual rows
                tt = (hh - 25) // 2
                xt4 = xrp.tile([128, H], BF, tag="xr", name="xr")
                nc.sync.dma_start(xt4[:], xres[tt * 128:(tt + 1) * 128, :])
                xres_sb.append(xt4)
            exs = []
            for qd in range(4):
                if qd < 2 and (hh, qd) in stash:
                    exs.append(stash.pop((hh, qd)))
                else:
                    exs.append(scores_quad(hh, qd))
            if pending is not None:
                attn_v(*pending)
            pending = (hh, exs)
        attn_v(*pending)

        _stk.close()

        # ---- Phase C: output projection + residual + layernorm -----------
        with (
            tc.tile_pool(name="psC", bufs=8, space="PSUM") as psC,
            tc.tile_pool(name="outp", bufs=2) as outp,
        ):
            if not triv_ln:
                nc.sync.dma_start(gam_sb[:], gamr[:])
                nc.sync.dma_start(bet_sb[:], betr[:])

            for tt in range(4):
                ob = outp.tile([128, H], FP, tag="ob", name="ob")
                stats = lnp.tile([128, 4, 6], FP, tag="st", name="st")
                pss = []
                for nb in range(4):
                    ps = psC.tile([128, 512], FP, tag="psC", name="psC")
                    pss.append(ps)
                    for c in range(KC):
                        nc.tensor.matmul(
                            ps[:],
                            ctx_sb[:, 2 * c:2 * c + 2, tt * 128:(tt + 1) * 128],
                            wo_blks[nb // 2][:, c, :,
                                             (nb % 2) * 512:(nb % 2 + 1) * 512],
                            start=(c == 0),
                            stop=False,
                            perf_mode=DR,
                        )
                    sl = slice(nb * 512, (nb + 1) * 512)
                    # residual add on the PE: += I.T @ xres (bf16)
                    nc.tensor.matmul(
                        ps[:],
                        id_sb[:],
                        xres_sb[tt][:, sl],
                        start=False,
                        stop=True,
                    )
                    nc.vector.bn_stats(stats[:, nb, :], ps[:])
                mv = lnp.tile([128, 2], FP, tag="mv", name="mv")
                nc.vector.bn_aggr(mv[:], stats[:])
                std = lnp.tile([128, 1], FP, tag="sd", name="sd")
                nc.scalar.activation(
                    std[:], mv[:, 1:2], func=Sqrt, bias=eps_sb[:], scale=1.0
                )
                rstd = lnp.tile([128, 1], FP, tag="rs", name="rs")
                nc.vector.reciprocal(rstd[:], std[:])
                nmr = lnp.tile([128, 1], FP, tag="nm", name="nm")
                nc.vector.scalar_tensor_tensor(
                    nmr[:], mv[:, 0:1], -1.0, rstd[:],
                    op0=mybir.AluOpType.mult, op1=mybir.AluOpType.mult,
                )
                for nb in range(4):
                    sl = slice(nb * 512, (nb + 1) * 512)
                    # (ps - mu) * rstd, alternating DVE / ACT (= ps*rstd
                    # + (-mu*rstd) via Identity) -- Pool cannot read PSUM
                    if nb % 2 == 0:
                        nc.vector.tensor_scalar(
                            ob[:, sl],
                            pss[nb][:],
                            scalar1=mv[:, 0:1],
                            scalar2=rstd[:],
                            op0=mybir.AluOpType.subtract,
                            op1=mybir.AluOpType.mult,
                        )
                    else:
                        nc.scalar.activation(
                            ob[:, sl], pss[nb][:], func=Ident,
                            bias=nmr[:], scale=rstd[:],
                        )
                    if not triv_ln:
                        eng = nc.vector if nb % 2 == 0 else nc.gpsimd
                        eng.tensor_mul(ob[:, sl], ob[:, sl], gam_sb[:, sl])
                        eng.tensor_add(ob[:, sl], ob[:, sl], bet_sb[:, sl])
                    nc.sync.dma_start(out[tt * 128:(tt + 1) * 128, sl],
                                      ob[:, sl])

    nc.compile()
    return nc


def _get_nc(use_mask: bool, triv_ln: bool | None = None,
            triv_bias: bool | None = None):
    if triv_ln is None:
        triv_ln = _LAST_TRIV[0]
    if triv_bias is None:
        triv_bias = _LAST_TRIV[1]
    key = (use_mask, triv_ln, triv_bias)
    if key not in _CACHE:
        _CACHE[key] = _build(use_mask, triv_ln, triv_bias)
    return _CACHE[key]


_LAST_TRIV = [True, True]


def _pack_w(w, cols):
    """[2048, cols] fp32 -> [128, 8, 2, cols] fp8 with K-pair layout."""
    return np.ascontiguousarray(
        (w * WS).reshape(KC, 2, 128, cols).transpose(2, 0, 1, 3)
    ).astype(FP8)


def _pack_act(xT, cols):
    """[2048, cols] fp32 (feature-major) -> [128, 8, 2, cols] fp8."""
    return np.ascontiguousarray(
        xT.reshape(KC, 2, 128, cols).transpose(2, 0, 1, 3)
    ).astype(FP8)


def kernel(
    hidden_state,
    encoder_hidden_state,
    encoder_attention_mask,
    Wq, bq, Wk, bk, Wv, bv, Wo, bo, gamma, beta,
):
    hidden_state = np.asarray(hidden_state, dtype=np.float32)
    encoder_hidden_state = np.asarray(encoder_hidden_state, dtype=np.float32)
    encoder_attention_mask = np.asarray(encoder_attention_mask, dtype=np.float32)
    Wq = np.asarray(Wq, dtype=np.float32)
    bq = np.asarray(bq, dtype=np.float32)
    Wk = np.asarray(Wk, dtype=np.float32)
    bk = np.asarray(bk, dtype=np.float32)
    Wv = np.asarray(Wv, dtype=np.float32)
    bv = np.asarray(bv, dtype=np.float32)
    Wo = np.asarray(Wo, dtype=np.float32)
    bo = np.asarray(bo, dtype=np.float32)
    gamma = np.asarray(gamma, dtype=np.float32)
    beta = np.asarray(beta, dtype=np.float32)

    use_mask = bool(np.any(encoder_attention_mask))
    triv_ln = bool(np.all(gamma == 1.0) and np.all(beta == 0.0))
    triv_bias = not (np.any(bq) or np.any(bk) or np.any(bv))
    _LAST_TRIV[0] = triv_ln
    _LAST_TRIV[1] = triv_bias
    nc = _get_nc(use_mask, triv_ln, triv_bias)
    in_maps = _prepare_in_maps(
        hidden_state, encoder_hidden_state, encoder_attention_mask,
        Wq, bq, Wk, bk, Wv, bv, Wo, bo, gamma, beta, use_mask,
    )

    res = run_bass_kernel_spmd(nc, in_maps, core_ids=list(range(8)))
    kernel._last_results = res

    output = np.empty((B, L, H), dtype=np.float32)
    for c in range(8):
        b, lh = c // 2, c % 2
        output[b, lh * TOK:(lh + 1) * TOK, :] = res.results[c]["out"]
    return output


def _prepare_in_maps(
    hidden_state, encoder_hidden_state, encoder_attention_mask,
    Wq, bq, Wk, bk, Wv, bv, Wo, bo, gamma, beta, use_mask,
):
    triv_ln = bool(np.all(gamma == 1.0) and np.all(beta == 0.0))
    triv_bias = not (np.any(bq) or np.any(bk) or np.any(bv))
    wq_p = _pack_w(Wq, H)
    wk_p = _pack_w(Wk, KVH * HD)
    wv_p = _pack_w(Wv, KVH * HD)
    wo_p = _pack_w(Wo, H)
    if not triv_bias:
        bias_all = np.ascontiguousarray(np.concatenate(
            [
                bq.reshape(16, 128).T * WS,
                bk.reshape(4, 128).T * WS,
                np.tile(bv[None, :], (128, 1)) * WS,
            ],
            axis=1,
        ).astype(np.float32))
    ident = np.ascontiguousarray(np.eye(128, dtype=np.float32).astype(BF16))
    if not triv_ln:
        gamr = np.ascontiguousarray(
            np.tile(gamma[None, :].astype(BF16), (128, 1)))
        betr = np.ascontiguousarray(
            np.tile(beta[None, :].astype(BF16), (128, 1)))

    eT_by_b = [
        _pack_act(encoder_hidden_state[b].T, S) for b in range(B)
    ]

    in_maps = []
    for c in range(8):
        b, lh = c // 2, c % 2
        rows = hidden_state[b, lh * TOK:(lh + 1) * TOK, :]
        m = {
            "xT": _pack_act(rows.T, TOK),
            "xres": ((rows + bo[None, :]) * (WS * WS)).astype(BF16),
            "eT": eT_by_b[b],
            "wq": wq_p, "wk": wk_p, "wv": wv_p, "wo": wo_p,
            "ident": ident,
        }
        if not triv_bias:
            m["bias_all"] = bias_all
        if not triv_ln:
            m["gamr"] = gamr
            m["betr"] = betr
        if use_mask:
            mslice = encoder_attention_mask[b, 0, lh * TOK:(lh + 1) * TOK, :]
            m["maskT"] = np.ascontiguousarray(
                (mslice.T * (8.0 * WS * WS)).astype(BF16))
        in_maps.append(m)
    return in_maps


# revision 38
# speedup vs baseline: 1.6137x; 1.0108x over previous
"""Encoder-decoder GQA attention block (B=4, L=S=1024, H=2048, 32 Q heads,
8 KV heads, head_dim 64) + output projection + residual + layernorm, on 8
Trainium2 NeuronCores.

Sharding: rows. Core c handles batch c//2, L-half c%2 (512 query rows).
K/V projections are computed per-batch (duplicated on the 2 cores sharing a
batch), attention over all 32 heads for the core's rows, output projection,
residual + LN. No collectives.

v2: all large matmuls run fp8(e4m3) with perf_mode=DoubleRow (0.5 cyc/row,
two 128-deep K slices per instruction -> 4x the bf16 MAC rate). Weights are
pre-scaled x32 on the host so their ~N(0, 0.02) entries land in e4m3's
normal range; activations stay unscaled (~N(0,1)). Scale bookkeeping:
  q,k carry x32 each -> scores PSUM x1024 -> exp scale 2^-13 (=0.125/1024)
  exp is shifted by bias=-2 (values ~e^[-9,3]) to sit in e4m3 range; the
  V "ones column" is 1/32 so rowsum carries x(1/32) while ctx carries v's
  x32 -> ctx = 32*ctx_true, exactly what fp8 O-proj wants.
  O-proj PSUM = 1024*(ctx@Wo); residual is pre-scaled x1024 on the host and
  layernorm is scale-invariant (eps shift negligible), so no unscale op.
Scores keep K=64 contraction: DoubleRow's second K-slice is a zero pad in
kT (j=1 memset once per buffer), so the pair contributes k.T@q + 0*junk.
attn@V pairs s-chunks: exp tiles are [128s, 2sc, 512t], V is [128s, 4sc,
8h, 72] (72 = 64+1 rowsum col + 7 pad so the DoubleRow pair stride is
16B-aligned).

Engine budget (cost model): ACT exp is the pole (~133us: 128 ops x
[128,2,512]); PE ~90us total (fp8-DR everywhere, bf16 only for the K=1
recip broadcast); DVE ~90us (PSUM->SBUF casts, recip, ctx mul, LN stats);
Pool takes the zero-pad memsets, residual adds and the LN normalize mul;
DMA ~60us serialized. Schedule: scores for heads 0-7 (s-lo) interleave
with the K/V projections in phase A so ACT starts ~12us in; attn@V lags
one head behind exp; Wq streams in 512-col blocks, Wo prefetches late in
phase B; phase C is dense PE (O-proj) with per-row-block LN + store.

LayerNorm gamma/beta ops and their DMAs are skipped when gamma==1 and
beta==0 (detected at runtime; separate build variant), same for the mask.
"""

from contextlib import ExitStack

import numpy as np
import ml_dtypes

import concourse.bass as bass  # noqa: F401  (bass.AP used via handles)
import concourse.mybir as mybir
import concourse.tile as tile
from concourse import bacc
from concourse.bass_utils import run_bass_kernel_spmd

BF16 = ml_dtypes.bfloat16
FP8 = ml_dtypes.float8_e4m3fn

H = 2048
NH = 32
KVH = 8
G = 4           # query-head groups per kv head
HD = 64
B, L, S = 4, 1024, 1024
TOK = 512       # decoder rows per core
KC = 8          # contraction chunk-pairs (8 x (2x128) = 2048)
SC = S // 128   # 8 s chunks
EPS = 1e-6
WS = 32.0       # host-side fp8 weight scale

FP = mybir.dt.float32
BF = mybir.dt.bfloat16
F8 = mybir.dt.float8e4
DR = mybir.MatmulPerfMode.DoubleRow

_CACHE: dict = {}


def _build(use_mask: bool, triv_ln: bool, triv_bias: bool):
    nc = bacc.Bacc("TRN2", target_bir_lowering=False)

    xT = nc.dram_tensor("xT", [128, KC, 2, TOK], F8, kind="ExternalInput")
    xres = nc.dram_tensor("xres", [TOK, H], BF, kind="ExternalInput")
    eT = nc.dram_tensor("eT", [128, KC, 2, S], F8, kind="ExternalInput")
    wq = nc.dram_tensor("wq", [128, KC, 2, H], F8, kind="ExternalInput")
    wk = nc.dram_tensor("wk", [128, KC, 2, 512], F8, kind="ExternalInput")
    wv = nc.dram_tensor("wv", [128, KC, 2, 512], F8, kind="ExternalInput")
    wo = nc.dram_tensor("wo", [128, KC, 2, H], F8, kind="ExternalInput")
    # packed biases: [bq(16) | bk(4) | bv(512)] -> one DMA
    if not triv_bias:
        bias_all = nc.dram_tensor("bias_all", [128, 532], FP,
                                  kind="ExternalInput")
    ident = nc.dram_tensor("ident", [128, 128], BF, kind="ExternalInput")
    if not triv_ln:
        gamr = nc.dram_tensor("gamr", [128, H], BF, kind="ExternalInput")
        betr = nc.dram_tensor("betr", [128, H], BF, kind="ExternalInput")
    if use_mask:
        maskT = nc.dram_tensor("maskT", [S, TOK], BF, kind="ExternalInput")
    out = nc.dram_tensor("out", [TOK, H], FP, kind="ExternalOutput")

    Exp = mybir.ActivationFunctionType.Exp
    Ident = mybir.ActivationFunctionType.Identity
    Sqrt = mybir.ActivationFunctionType.Sqrt

    with tile.TileContext(nc) as tc:
      with (
          tc.tile_pool(name="ctxT", bufs=1) as ctxp,
          tc.tile_pool(name="cc", bufs=1) as ccp,
          tc.tile_pool(name="ln", bufs=10) as lnp,
          tc.tile_pool(name="qT", bufs=NH) as qtp,
          tc.tile_pool(name="kT", bufs=KVH) as ktp,
          tc.tile_pool(name="vv", bufs=2) as vvp,
          tc.tile_pool(name="expp", bufs=14) as expp,
          tc.tile_pool(name="rec", bufs=4) as recp,
          tc.tile_pool(name="bc", bufs=4) as bcp,
          tc.tile_pool(name="const", bufs=1) as constp,
          tc.tile_pool(name="wq", bufs=2) as wqp,
          tc.tile_pool(name="xTp", bufs=1) as xtp,
          tc.tile_pool(name="wo", bufs=2) as wop,
          tc.tile_pool(name="xr", bufs=4) as xrp,
          tc.tile_pool(name="maskp", bufs=SC if use_mask else 1) as mkp,
      ):
        eps_sb = ccp.tile([128, 1], FP, name="eps_sb")
        nb2_sb = ccp.tile([128, 1], FP, name="nb2_sb")
        junk_sb = ccp.tile([128, 1], FP, name="junk_sb")
        nc.vector.memset(eps_sb[:], EPS)
        nc.vector.memset(nb2_sb[:], -2.0)
        # touch the Sqrt act table now so its load isn't on the phase-C chain
        nc.scalar.activation(junk_sb[:], eps_sb[:], func=Sqrt,
                             bias=eps_sb[:], scale=1.0)
        if not triv_ln:
            gam_sb = ccp.tile([128, H], BF, name="gam_sb")
            bet_sb = ccp.tile([128, H], BF, name="bet_sb")

        ctx_sb = ctxp.tile([128, 16, TOK], F8, name="ctx")
        # qt/kt hold the DoubleRow K-pair on dim1; slice [:,1,:] is a zero
        # pad (memset once per buffer below) so K=64 contractions are legal.
        qT_sb = [qtp.tile([64, 2, TOK], F8, name="qt") for _ in range(NH)]
        kT_sb = [ktp.tile([64, 2, S], F8, name="kt") for _ in range(KVH)]
        # V: [s-part, sc-in-quad, kv-head, 64+1(rowsum)+7(pad to 16B)]
        vv_sb = [vvp.tile([128, 4, KVH, 72], F8, name="vv") for _ in range(2)]

        # zero the DoubleRow pads: early-needed ones on DVE (idle until
        # the first PSUM copies ~17us), the rest on Pool
        for t in kT_sb:
            nc.vector.memset(t[:, 1, :], 0.0)
        for t in qT_sb[:8]:
            nc.vector.memset(t[:, 1, :], 0.0)
        for t in vv_sb:
            nc.gpsimd.memset(t[:, :, :, 64:65], 1.0)
        for t in qT_sb[8:]:
            nc.gpsimd.memset(t[:, 1, :], 0.0)

        _stk = ExitStack()
        psA = _stk.enter_context(tc.tile_pool(name="psA", bufs=2, space="PSUM"))
        psS = _stk.enter_context(tc.tile_pool(name="psS", bufs=2, space="PSUM"))

        id_sb = constp.tile([128, 128], BF, name="id_sb")
        if not triv_bias:
            bias_sb = constp.tile([128, 532], FP, name="bias_sb")
            bq_sb = bias_sb[:, 0:16]
            bk_sb = bias_sb[:, 16:20]
            bv_sb = bias_sb[:, 20:532]

        def q_proj(m):
            blk, base = wq_blk[0]
            q = m - base
            assert 0 <= q < 4, (m, base)
            ps = psA.tile([128, TOK], FP, tag="psA", name="psA")
            for c in range(KC):
                nc.tensor.matmul(
                    ps[:],
                    blk[:, c, :, q * 128:(q + 1) * 128],
                    xt_t[:, c],
                    start=(c == 0),
                    stop=(c == KC - 1),
                    perf_mode=DR,
                )
            with nc.allow_low_precision(reason="q cast to fp8 for scores"):
                for hi in range(2):
                    if triv_bias:
                        nc.vector.tensor_copy(
                            qT_sb[2 * m + hi][:, 0, :],
                            ps[hi * 64:hi * 64 + 64, :],
                        )
                    else:
                        nc.vector.tensor_scalar_add(
                            qT_sb[2 * m + hi][:, 0, :],
                            ps[hi * 64:hi * 64 + 64, :],
                            bq_sb[hi * 64:hi * 64 + 64, m:m + 1],
                        )

        def scores_quad(hh, qd):
            h = hh // G
            ps = psS.tile([128, 2, TOK], FP, tag="psS", name="psS")
            for i in range(2):
                sc = 2 * qd + i
                nc.tensor.matmul(
                    ps[:, i, :],
                    kT_sb[h][:, :, sc * 128:(sc + 1) * 128],
                    qT_sb[hh][:],
                    start=True,
                    stop=True,
                    perf_mode=DR,
                )
                if use_mask:
                    nc.vector.tensor_add(ps[:, i, :], ps[:, i, :],
                                         mask_sb[sc][:])
            ex = expp.tile([128, 2, TOK], F8, tag="ex", name="ex")
            nc.scalar.activation(ex[:], ps[:], func=Exp,
                                 scale=1.0 / 8192.0, bias=nb2_sb[:])
            return ex

        def attn_v(hh, exs):
            h = hh // G
            po = psO.tile([128, TOK], FP, tag="psO", name="psO")
            for qd in range(4):
                vq = vv_sb[qd // 2][:, (qd % 2) * 2:(qd % 2) * 2 + 2, h, 0:65]
                nc.tensor.matmul(
                    po[0:65, :],
                    vq,
                    exs[qd][:],
                    start=(qd == 0),
                    stop=(qd == 3),
                    perf_mode=DR,
                )
            recb = recp.tile([1, TOK], BF, tag="recb", name="recb")
            with nc.allow_low_precision(reason="softmax recip rounds to bf16"):
                nc.vector.reciprocal(recb[:], po[64:65, :])
            # broadcast recip across 64 partitions on the idle Pool engine
            rb = bcp.tile([64, TOK], BF, tag="rb", name="rb")
            nc.gpsimd.partition_broadcast(rb[:], recb[:])
            with nc.allow_low_precision(reason="ctx cast to fp8 for O-proj"):
                nc.vector.tensor_mul(
                    ctx_sb[(hh % 2) * 64:(hh % 2) * 64 + 64, hh // 2, :],
                    po[0:64, :],
                    rb[:],
                )

        # ---- Phase A: input DMAs, K/V/Q projections, s-lo scores ---------
        with (
            tc.tile_pool(name="eTp", bufs=1) as etp,
            tc.tile_pool(name="wk", bufs=1) as wkp,
            tc.tile_pool(name="wv", bufs=1) as wvp,
            tc.tile_pool(name="psKV", bufs=2, space="PSUM") as psKV,
        ):
            wk_t = wkp.tile([128, KC, 2, 512], F8, name="wkt")
            et_t = etp.tile([128, KC, 2, S], F8, name="et")
            wv_t = wvp.tile([128, KC, 2, 512], F8, name="wvt")
            xt_t = xtp.tile([128, KC, 2, TOK], F8, name="xt")
            # few big DMAs: descriptor-gen (HWDGE) is a serial shared
            # resource (~0.63us per DMA instruction), so chunking loses
            # critical-path loads on SP; decoder-side on ACT queue;
            # late-needed on DVE queue (DMA engine is shared+serial, queue
            # split biases acquisition order toward the critical path)
            nc.sync.dma_start(wk_t[:], wk[:])
            nc.sync.dma_start(et_t[:, :, :, 0:512], eT[:, :, :, 0:512])
            if not triv_bias:
                nc.sync.dma_start(bias_sb[:], bias_all[:])
            t = wqp.tile([128, KC, 2, 512], F8, tag="wq", name="wqt")
            nc.scalar.dma_start(xt_t[:], xT[:])
            nc.scalar.dma_start(t[:, :, :, 0:256], wq[:, :, :, 0:256])
            nc.scalar.dma_start(t[:, :, :, 256:512], wq[:, :, :, 256:512])
            nc.scalar.dma_start(id_sb[:], ident[:])
            wq_blk = [(t, 0)]
            nc.sync.dma_start(et_t[:, :, :, 512:S], eT[:, :, :, 512:S])
            nc.sync.dma_start(wv_t[:], wv[:])
            if use_mask:
                mask_sb = []
                for sc in range(SC):
                    mt = mkp.tile([128, TOK], BF, tag="mk", name="mk")
                    nc.sync.dma_start(mt[:], maskT[sc * 128:(sc + 1) * 128, :])
                    mask_sb.append(mt)

            def k_proj(sh, ms):
                pss = [psKV.tile([128, 512], FP, tag="psKV", name="psKV")
                       for _ in ms]
                for c in range(KC):
                    for i, m in enumerate(ms):
                        nc.tensor.matmul(
                            pss[i][:],
                            wk_t[:, c, :, m * 128:(m + 1) * 128],
                            et_t[:, c, :, sh * 512:(sh + 1) * 512],
                            start=(c == 0),
                            stop=(c == KC - 1),
                            perf_mode=DR,
                        )
                with nc.allow_low_precision(reason="k cast to fp8"):
                    for i, m in enumerate(ms):
                        for hi in range(2):
                            dst = kT_sb[2 * m + hi][:, 0,
                                                    sh * 512:(sh + 1) * 512]
                            if triv_bias:
                                nc.vector.tensor_copy(
                                    dst, pss[i][hi * 64:hi * 64 + 64, :])
                            else:
                                nc.vector.tensor_scalar_add(
                                    dst,
                                    pss[i][hi * 64:hi * 64 + 64, :],
                                    bk_sb[hi * 64:hi * 64 + 64, m:m + 1],
                                )

            def v_proj(scs):
                pss = [psKV.tile([128, 512], FP, tag="psKV", name="psKV")
                       for _ in scs]
                for c in range(KC):
                    for i, sc in enumerate(scs):
                        nc.tensor.matmul(
                            pss[i][:],
                            et_t[:, c, :, sc * 128:(sc + 1) * 128],
                            wv_t[:, c],
                            start=(c == 0),
                            stop=(c == KC - 1),
                            perf_mode=DR,
                        )
                with nc.allow_low_precision(reason="v cast to fp8"):
                    for i, sc in enumerate(scs):
                        if triv_bias:
                            nc.vector.tensor_copy(
                                vv_sb[sc // 4][:, sc % 4, :, 0:64],
                                pss[i].rearrange("p (h d) -> p h d", d=HD),
                            )
                        else:
                            nc.vector.tensor_add(
                                vv_sb[sc // 4][:, sc % 4, :, 0:64],
                                pss[i].rearrange("p (h d) -> p h d", d=HD),
                                bv_sb.rearrange("p (h d) -> p h d", d=HD),
                            )

            # interleave s-lo scores with the projections so ACT (the pole)
            # starts as soon as kT h0/h1 and qT 0-3 exist
            stash = {}
            k_proj(0, (0, 1))
            q_proj(0)
            q_proj(1)
            stash[(0, 0)] = scores_quad(0, 0)
            stash[(0, 1)] = scores_quad(0, 1)
            k_proj(0, (2, 3))
            stash[(1, 0)] = scores_quad(1, 0)
            stash[(1, 1)] = scores_quad(1, 1)
            v_proj((0, 1))
            q_proj(2)
            q_proj(3)
            stash[(2, 0)] = scores_quad(2, 0)
            stash[(2, 1)] = scores_quad(2, 1)
            v_proj((2, 3))
            stash[(3, 0)] = scores_quad(3, 0)
            stash[(3, 1)] = scores_quad(3, 1)
            k_proj(1, (0, 1))
            stash[(4, 0)] = scores_quad(4, 0)
            stash[(4, 1)] = scores_quad(4, 1)
            k_proj(1, (2, 3))
            v_proj((4, 5))
            v_proj((6, 7))

        # psKV released; open attention output pool in its banks
        psO = _stk.enter_context(tc.tile_pool(name="psO", bufs=2, space="PSUM"))

        wo_blks = []
        xres_sb = []

        # ---- Phase B: per-head scores+exp / lagged attn@V ----------------
        pending = None
        for hh in range(NH):
            if hh % 2 == 0 and 2 <= hh <= 24:
                m = 4 + (hh - 2) // 2
                if m % 4 == 0:  # new 512-col Wq block
                    t = wqp.tile([128, KC, 2, 512], F8, tag="wq", name="wqt")
                    nc.scalar.dma_start(
                        t[:], wq[:, :, :, (m // 4) * 512:(m // 4 + 1) * 512])
                    wq_blk[0] = (t, m)
                q_proj(m)
            if hh in (24, 28):  # prefetch Wo halves
                half = (hh - 24) // 4
                t = wop.tile([128, KC, 2, 1024], F8, tag="wo", name="wot")
                nc.scalar.dma_start(
                    t[:], wo[:, :, :, half * 1024:(half + 1) * 1024])
                wo_blks.append(t)
            if hh in (25, 27, 29, 31):  # prefetch residual rows
                tt = (hh - 25) // 2
                xt4 = xrp.tile([128, H], BF, tag="xr", name="xr")
                nc.sync.dma_start(xt4[:], xres[tt * 128:(tt + 1) * 128, :])
                xres_sb.append(xt4)
            exs = []
            for qd in range(4):
                if qd < 2 and (hh, qd) in stash:
                    exs.append(stash.pop((hh, qd)))
                else:
                    exs.append(scores_quad(hh, qd))
            if pending is not None:
                attn_v(*pending)
            pending = (hh, exs)
        attn_v(*pending)

        _stk.close()

        # ---- Phase C: output projection + residual + layernorm -----------
        with (
            tc.tile_pool(name="psC", bufs=8, space="PSUM") as psC,
            tc.tile_pool(name="outp", bufs=2) as outp,
        ):
            if not triv_ln:
                nc.sync.dma_start(gam_sb[:], gamr[:])
                nc.sync.dma_start(bet_sb[:], betr[:])

            for tt in range(4):
                ob = outp.tile([128, H], FP, tag="ob", name="ob")
                stats = lnp.tile([128, 4, 6], FP, tag="st", name="st")
                pss = []
                for nb in range(4):
                    ps = psC.tile([128, 512], FP, tag="psC", name="psC")
                    pss.append(ps)
                    for c in range(KC):
                        nc.tensor.matmul(
                            ps[:],
                            ctx_sb[:, 2 * c:2 * c + 2, tt * 128:(tt + 1) * 128],
                            wo_blks[nb // 2][:, c, :,
                                             (nb % 2) * 512:(nb % 2 + 1) * 512],
                            start=(c == 0),
                            stop=False,
                            perf_mode=DR,
                        )
                    sl = slice(nb * 512, (nb + 1) * 512)
                    # residual add on the PE: += I.T @ xres (bf16)
                    nc.tensor.matmul(
                        ps[:],
                        id_sb[:],
                        xres_sb[tt][:, sl],
                        start=False,
                        stop=True,
                    )
                    nc.vector.bn_stats(stats[:, nb, :], ps[:])
                mv = lnp.tile([128, 2], FP, tag="mv", name="mv")
                nc.vector.bn_aggr(mv[:], stats[:])
                std = lnp.tile([128, 1], FP, tag="sd", name="sd")
                nc.scalar.activation(
                    std[:], mv[:, 1:2], func=Sqrt, bias=eps_sb[:], scale=1.0
                )
                rstd = lnp.tile([128, 1], FP, tag="rs", name="rs")
                nc.vector.reciprocal(rstd[:], std[:])
                nmr = lnp.tile([128, 1], FP, tag="nm", name="nm")
                nc.vector.scalar_tensor_tensor(
                    nmr[:], mv[:, 0:1], -1.0, rstd[:],
                    op0=mybir.AluOpType.mult, op1=mybir.AluOpType.mult,
                )
                for nb in range(4):
                    sl = slice(nb * 512, (nb + 1) * 512)
                    # (ps - mu) * rstd, alternating DVE / ACT (= ps*rstd
                    # + (-mu*rstd) via Identity) -- Pool cannot read PSUM
                    if nb % 2 == 0:
                        nc.vector.tensor_scalar(
                            ob[:, sl],
                            pss[nb][:],
                            scalar1=mv[:, 0:1],
                            scalar2=rstd[:],
                            op0=mybir.AluOpType.subtract,
                            op1=mybir.AluOpType.mult,
                        )
                    else:
                        nc.scalar.activation(
                            ob[:, sl], pss[nb][:], func=Ident,
                            bias=nmr[:], scale=rstd[:],
                        )
                    if not triv_ln:
                        eng = nc.vector if nb % 2 == 0 else nc.gpsimd
                        eng.tensor_mul(ob[:, sl], ob[:, sl], gam_sb[:, sl])
                        eng.tensor_add(ob[:, sl], ob[:, sl], bet_sb[:, sl])
                    nc.sync.dma_start(out[tt * 128:(tt + 1) * 128, sl],
                                      ob[:, sl])

    nc.compile()
    return nc


def _get_nc(use_mask: bool, triv_ln: bool | None = None,
            triv_bias: bool | None = None):
    if triv_ln is None:
        triv_ln = _LAST_TRIV[0]
    if triv_bias is None:
        triv_bias = _LAST_TRIV[1]
    key = (use_mask, triv_ln, triv_bias)
    if key not in _CACHE:
        _CACHE[key] = _build(use_mask, triv_ln, triv_bias)
    return _CACHE[key]


_LAST_TRIV = [True, True]


def _pack_w(w, cols):
    """[2048, cols] fp32 -> [128, 8, 2, cols] fp8 with K-pair layout."""
    return np.ascontiguousarray(
        (w * WS).reshape(KC, 2, 128, cols).transpose(2, 0, 1, 3)
    ).astype(FP8)


def _pack_act(xT, cols):
    """[2048, cols] fp32 (feature-major) -> [128, 8, 2, cols] fp8."""
    return np.ascontiguousarray(
        xT.reshape(KC, 2, 128, cols).transpose(2, 0, 1, 3)
    ).astype(FP8)


def kernel(
    hidden_state,
    encoder_hidden_state,
    encoder_attention_mask,
    Wq, bq, Wk, bk, Wv, bv, Wo, bo, gamma, beta,
):
    hidden_state = np.asarray(hidden_state, dtype=np.float32)
    encoder_hidden_state = np.asarray(encoder_hidden_state, dtype=np.float32)
    encoder_attention_mask = np.asarray(encoder_attention_mask, dtype=np.float32)
    Wq = np.asarray(Wq, dtype=np.float32)
    bq = np.asarray(bq, dtype=np.float32)
    Wk = np.asarray(Wk, dtype=np.float32)
    bk = np.asarray(bk, dtype=np.float32)
    Wv = np.asarray(Wv, dtype=np.float32)
    bv = np.asarray(bv, dtype=np.float32)
    Wo = np.asarray(Wo, dtype=np.float32)
    bo = np.asarray(bo, dtype=np.float32)
    gamma = np.asarray(gamma, dtype=np.float32)
    beta = np.asarray(beta, dtype=np.float32)

    use_mask = bool(np.any(encoder_attention_mask))
    triv_ln = bool(np.all(gamma == 1.0) and np.all(beta == 0.0))
    triv_bias = not (np.any(bq) or np.any(bk) or np.any(bv))
    _LAST_TRIV[0] = triv_ln
    _LAST_TRIV[1] = triv_bias
    nc = _get_nc(use_mask, triv_ln, triv_bias)
    in_maps = _prepare_in_maps(
        hidden_state, encoder_hidden_state, encoder_attention_mask,
        Wq, bq, Wk, bk, Wv, bv, Wo, bo, gamma, beta, use_mask,
    )

    res = run_bass_kernel_spmd(nc, in_maps, core_ids=list(range(8)))
    kernel._last_results = res

    output = np.empty((B, L, H), dtype=np.float32)
    for c in range(8):
        b, lh = c // 2, c % 2
        output[b, lh * TOK:(lh + 1) * TOK, :] = res.results[c]["out"]
    return output


def _prepare_in_maps(
    hidden_state, encoder_hidden_state, encoder_attention_mask,
    Wq, bq, Wk, bk, Wv, bv, Wo, bo, gamma, beta, use_mask,
):
    triv_ln = bool(np.all(gamma == 1.0) and np.all(beta == 0.0))
    triv_bias = not (np.any(bq) or np.any(bk) or np.any(bv))
    wq_p = _pack_w(Wq, H)
    wk_p = _pack_w(Wk, KVH * HD)
    wv_p = _pack_w(Wv, KVH * HD)
    wo_p = _pack_w(Wo, H)
    if not triv_bias:
        bias_all = np.ascontiguousarray(np.concatenate(
            [
                bq.reshape(16, 128).T * WS,
                bk.reshape(4, 128).T * WS,
                np.tile(bv[None, :], (128, 1)) * WS,
            ],
            axis=1,
        ).astype(np.float32))
    ident = np.ascontiguousarray(np.eye(128, dtype=np.float32).astype(BF16))
    if not triv_ln:
        gamr = np.ascontiguousarray(
            np.tile(gamma[None, :].astype(BF16), (128, 1)))
        betr = np.ascontiguousarray(
            np.tile(beta[None, :].astype(BF16), (128, 1)))

    eT_by_b = [
        _pack_act(encoder_hidden_state[b].T, S) for b in range(B)
    ]

    in_maps = []
    for c in range(8):
        b, lh = c // 2, c % 2
        rows = hidden_state[b, lh * TOK:(lh + 1) * TOK, :]
        m = {
            "xT": _pack_act(rows.T, TOK),
            "xres": ((rows + bo[None, :]) * (WS * WS)).astype(BF16),
            "eT": eT_by_b[b],
            "wq": wq_p, "wk": wk_p, "wv": wv_p, "wo": wo_p,
            "ident": ident,
        }
        if not triv_bias:
            m["bias_all"] = bias_all
        if not triv_ln:
            m["gamr"] = gamr
            m["betr"] = betr
        if use_mask:
            mslice = encoder_attention_mask[b, 0, lh * TOK:(lh + 1) * TOK, :]
            m["maskT"] = np.ascontiguousarray(
                (mslice.T * (8.0 * WS * WS)).astype(BF16))
        in_maps.append(m)
    return in_maps


# revision 45
# speedup vs baseline: 1.6298x; 1.0100x over previous
"""Encoder-decoder GQA attention block (B=4, L=S=1024, H=2048, 32 Q heads,
8 KV heads, head_dim 64) + output projection + residual + layernorm, on 8
Trainium2 NeuronCores.

Sharding: rows. Core c handles batch c//2, L-half c%2 (512 query rows).
K/V projections are computed per-batch (duplicated on the 2 cores sharing a
batch), attention over all 32 heads for the core's rows, output projection,
residual + LN. No collectives.

v2: all large matmuls run fp8(e4m3) with perf_mode=DoubleRow (0.5 cyc/row,
two 128-deep K slices per instruction -> 4x the bf16 MAC rate). Weights are
pre-scaled x32 on the host so their ~N(0, 0.02) entries land in e4m3's
normal range; activations stay unscaled (~N(0,1)). Scale bookkeeping:
  q,k carry x32 each -> scores PSUM x1024 -> exp scale 2^-13 (=0.125/1024)
  exp is shifted by bias=-2 (values ~e^[-9,3]) to sit in e4m3 range; the
  V "ones column" is 1/32 so rowsum carries x(1/32) while ctx carries v's
  x32 -> ctx = 32*ctx_true, exactly what fp8 O-proj wants.
  O-proj PSUM = 1024*(ctx@Wo); residual is pre-scaled x1024 on the host and
  layernorm is scale-invariant (eps shift negligible), so no unscale op.
Scores keep K=64 contraction: DoubleRow's second K-slice is a zero pad in
kT (j=1 memset once per buffer), so the pair contributes k.T@q + 0*junk.
attn@V pairs s-chunks: exp tiles are [128s, 2sc, 512t], V is [128s, 4sc,
8h, 72] (72 = 64+1 rowsum col + 7 pad so the DoubleRow pair stride is
16B-aligned).

Engine budget (cost model): ACT exp is the pole (~133us: 128 ops x
[128,2,512]); PE ~90us total (fp8-DR everywhere, bf16 only for the K=1
recip broadcast); DVE ~90us (PSUM->SBUF casts, recip, ctx mul, LN stats);
Pool takes the zero-pad memsets, residual adds and the LN normalize mul;
DMA ~60us serialized. Schedule: scores for heads 0-7 (s-lo) interleave
with the K/V projections in phase A so ACT starts ~12us in; attn@V lags
one head behind exp; Wq streams in 512-col blocks, Wo prefetches late in
phase B; phase C is dense PE (O-proj) with per-row-block LN + store.

LayerNorm gamma/beta ops and their DMAs are skipped when gamma==1 and
beta==0 (detected at runtime; separate build variant), same for the mask.
"""

from contextlib import ExitStack

import numpy as np
import ml_dtypes

import concourse.bass as bass  # noqa: F401  (bass.AP used via handles)
import concourse.mybir as mybir
import concourse.tile as tile
from concourse import bacc
from concourse.bass_utils import run_bass_kernel_spmd

BF16 = ml_dtypes.bfloat16
FP8 = ml_dtypes.float8_e4m3fn

H = 2048
NH = 32
KVH = 8
G = 4           # query-head groups per kv head
HD = 64
B, L, S = 4, 1024, 1024
TOK = 512       # decoder rows per core
KC = 8          # contraction chunk-pairs (8 x (2x128) = 2048)
SC = S // 128   # 8 s chunks
EPS = 1e-6
WS = 32.0       # host-side fp8 weight scale

FP = mybir.dt.float32
BF = mybir.dt.bfloat16
F8 = mybir.dt.float8e4
DR = mybir.MatmulPerfMode.DoubleRow

_CACHE: dict = {}


def _build(use_mask: bool, triv_ln: bool, triv_bias: bool):
    nc = bacc.Bacc("TRN2", target_bir_lowering=False)

    xT = nc.dram_tensor("xT", [128, KC, 2, TOK], F8, kind="ExternalInput")
    xres = nc.dram_tensor("xres", [TOK, H], BF, kind="ExternalInput")
    eT = nc.dram_tensor("eT", [128, KC, 2, S], F8, kind="ExternalInput")
    wq = nc.dram_tensor("wq", [128, KC, 2, H], F8, kind="ExternalInput")
    wk = nc.dram_tensor("wk", [128, KC, 2, 512], F8, kind="ExternalInput")
    wv = nc.dram_tensor("wv", [128, KC, 2, 512], F8, kind="ExternalInput")
    wo = nc.dram_tensor("wo", [128, KC, 2, H], F8, kind="ExternalInput")
    # packed biases: [bq(16) | bk(4) | bv(512)] -> one DMA
    if not triv_bias:
        bias_all = nc.dram_tensor("bias_all", [128, 532], FP,
                                  kind="ExternalInput")
    ident = nc.dram_tensor("ident", [128, 128], BF, kind="ExternalInput")
    if not triv_ln:
        gamr = nc.dram_tensor("gamr", [128, H], BF, kind="ExternalInput")
        betr = nc.dram_tensor("betr", [128, H], BF, kind="ExternalInput")
    if use_mask:
        maskT = nc.dram_tensor("maskT", [S, TOK], BF, kind="ExternalInput")
    out = nc.dram_tensor("out", [TOK, H], FP, kind="ExternalOutput")

    Exp = mybir.ActivationFunctionType.Exp
    Ident = mybir.ActivationFunctionType.Identity
    Sqrt = mybir.ActivationFunctionType.Sqrt

    with tile.TileContext(nc) as tc:
      with (
          tc.tile_pool(name="ctxT", bufs=1) as ctxp,
          tc.tile_pool(name="cc", bufs=1) as ccp,
          tc.tile_pool(name="ln", bufs=10) as lnp,
          tc.tile_pool(name="qT", bufs=NH) as qtp,
          tc.tile_pool(name="kT", bufs=KVH) as ktp,
          tc.tile_pool(name="vv", bufs=2) as vvp,
          tc.tile_pool(name="expp", bufs=14) as expp,
          tc.tile_pool(name="rec", bufs=4) as recp,
          tc.tile_pool(name="bc", bufs=4) as bcp,
          tc.tile_pool(name="const", bufs=1) as constp,
          tc.tile_pool(name="wq", bufs=2) as wqp,
          tc.tile_pool(name="xTp", bufs=1) as xtp,
          tc.tile_pool(name="wo", bufs=2) as wop,
          tc.tile_pool(name="xr", bufs=4) as xrp,
          tc.tile_pool(name="maskp", bufs=SC if use_mask else 1) as mkp,
      ):
        eps_sb = ccp.tile([128, 1], FP, name="eps_sb")
        nb2_sb = ccp.tile([128, 1], FP, name="nb2_sb")
        junk_sb = ccp.tile([128, 1], FP, name="junk_sb")
        nc.vector.memset(eps_sb[:], EPS)
        nc.vector.memset(nb2_sb[:], -2.0)
        # touch the Sqrt act table now so its load isn't on the phase-C chain
        nc.scalar.activation(junk_sb[:], eps_sb[:], func=Sqrt,
                             bias=eps_sb[:], scale=1.0)
        if not triv_ln:
            gam_sb = ccp.tile([128, H], BF, name="gam_sb")
            bet_sb = ccp.tile([128, H], BF, name="bet_sb")

        ctx_sb = ctxp.tile([128, 16, TOK], F8, name="ctx")
        # qt/kt hold the DoubleRow K-pair on dim1; slice [:,1,:] is a zero
        # pad (memset once per buffer below) so K=64 contractions are legal.
        qT_sb = [qtp.tile([64, 2, TOK], F8, name="qt") for _ in range(NH)]
        kT_sb = [ktp.tile([64, 2, S], F8, name="kt") for _ in range(KVH)]
        # V: [s-part, sc-in-quad, kv-head, 64+1(rowsum)+7(pad to 16B)]
        vv_sb = [vvp.tile([128, 4, KVH, 72], F8, name="vv") for _ in range(2)]

        # zero the DoubleRow pads: early-needed ones on DVE (idle until
        # the first PSUM copies ~17us), the rest on Pool
        for t in kT_sb:
            nc.vector.memset(t[:, 1, :], 0.0)
        for t in qT_sb[:8]:
            nc.vector.memset(t[:, 1, :], 0.0)
        for t in vv_sb:
            nc.gpsimd.memset(t[:, :, :, 64:65], 1.0)
        for t in qT_sb[8:]:
            nc.gpsimd.memset(t[:, 1, :], 0.0)

        _stk = ExitStack()
        psA = _stk.enter_context(tc.tile_pool(name="psA", bufs=2, space="PSUM"))
        psS = _stk.enter_context(tc.tile_pool(name="psS", bufs=2, space="PSUM"))

        id_sb = constp.tile([128, 128], BF, name="id_sb")
        if not triv_bias:
            bias_sb = constp.tile([128, 532], FP, name="bias_sb")
            bq_sb = bias_sb[:, 0:16]
            bk_sb = bias_sb[:, 16:20]
            bv_sb = bias_sb[:, 20:532]

        def q_proj(m):
            blk, base = wq_blk[0]
            q = m - base
            assert 0 <= q < 4, (m, base)
            ps = psA.tile([128, TOK], FP, tag="psA", name="psA")
            for c in range(KC):
                nc.tensor.matmul(
                    ps[:],
                    blk[:, c, :, q * 128:(q + 1) * 128],
                    xt_t[:, c],
                    start=(c == 0),
                    stop=(c == KC - 1),
                    perf_mode=DR,
                )
            with nc.allow_low_precision(reason="q cast to fp8 for scores"):
                for hi in range(2):
                    if triv_bias:
                        nc.vector.tensor_copy(
                            qT_sb[2 * m + hi][:, 0, :],
                            ps[hi * 64:hi * 64 + 64, :],
                        )
                    else:
                        nc.vector.tensor_scalar_add(
                            qT_sb[2 * m + hi][:, 0, :],
                            ps[hi * 64:hi * 64 + 64, :],
                            bq_sb[hi * 64:hi * 64 + 64, m:m + 1],
                        )

        def scores_quad(hh, qd):
            h = hh // G
            ps = psS.tile([128, 2, TOK], FP, tag="psS", name="psS")
            for i in range(2):
                sc = 2 * qd + i
                nc.tensor.matmul(
                    ps[:, i, :],
                    kT_sb[h][:, :, sc * 128:(sc + 1) * 128],
                    qT_sb[hh][:],
                    start=True,
                    stop=True,
                    perf_mode=DR,
                )
                if use_mask:
                    nc.vector.tensor_add(ps[:, i, :], ps[:, i, :],
                                         mask_sb[sc][:])
            ex = expp.tile([128, 2, TOK], F8, tag="ex", name="ex")
            nc.scalar.activation(ex[:], ps[:], func=Exp,
                                 scale=1.0 / 8192.0, bias=nb2_sb[:])
            return ex

        def attn_v(hh, exs):
            h = hh // G
            po = psO.tile([128, TOK], FP, tag="psO", name="psO")
            for qd in range(4):
                vq = vv_sb[qd // 2][:, (qd % 2) * 2:(qd % 2) * 2 + 2, h, 0:65]
                nc.tensor.matmul(
                    po[0:65, :],
                    vq,
                    exs[qd][:],
                    start=(qd == 0),
                    stop=(qd == 3),
                    perf_mode=DR,
                )
            recb = recp.tile([1, TOK], BF, tag="recb", name="recb")
            with nc.allow_low_precision(reason="softmax recip rounds to bf16"):
                nc.vector.reciprocal(recb[:], po[64:65, :])
            # broadcast recip across 64 partitions on the idle Pool engine
            rb = bcp.tile([64, TOK], BF, tag="rb", name="rb")
            nc.gpsimd.partition_broadcast(rb[:], recb[:])
            with nc.allow_low_precision(reason="ctx cast to fp8 for O-proj"):
                nc.vector.tensor_mul(
                    ctx_sb[(hh % 2) * 64:(hh % 2) * 64 + 64, hh // 2, :],
                    po[0:64, :],
                    rb[:],
                )

        # ---- Phase A: input DMAs, K/V/Q projections, s-lo scores ---------
        with (
            tc.tile_pool(name="eTp", bufs=1) as etp,
            tc.tile_pool(name="wk", bufs=1) as wkp,
            tc.tile_pool(name="wv", bufs=1) as wvp,
            tc.tile_pool(name="psKV", bufs=2, space="PSUM") as psKV,
        ):
            wk_t = wkp.tile([128, KC, 2, 512], F8, name="wkt")
            et_t = etp.tile([128, KC, 2, S], F8, name="et")
            wv_t = wvp.tile([128, KC, 2, 512], F8, name="wvt")
            xt_t = xtp.tile([128, KC, 2, TOK], F8, name="xt")
            # few big DMAs: descriptor-gen (HWDGE) is a serial shared
            # resource (~0.63us per DMA instruction), so chunking loses
            # PE p-state warm-up during the initial DMA wait: dummy
            # matmuls on the already-zeroed kT pads keep pe_busy_start
            # anchored early so the first real chains run at full clock
            wps = psA.tile([128, TOK], FP, tag="psA", name="psA")
            for _ in range(40):
                nc.tensor.matmul(
                    wps[:, 0:512], kT_sb[0][:, 1, 0:128], kT_sb[0][:, 1, 0:512],
                    start=True, stop=True,
                )
            # critical-path loads on SP; decoder-side on ACT queue;
            # late-needed on DVE queue (DMA engine is shared+serial, queue
            # split biases acquisition order toward the critical path)
            nc.sync.dma_start(wk_t[:], wk[:])
            nc.sync.dma_start(et_t[:, :, :, 0:512], eT[:, :, :, 0:512])
            if not triv_bias:
                nc.sync.dma_start(bias_sb[:], bias_all[:])
            nc.sync.dma_start(xt_t[:], xT[:])
            t = wqp.tile([128, KC, 2, 512], F8, tag="wq", name="wqt")
            nc.scalar.dma_start(t[:, :, :, 0:256], wq[:, :, :, 0:256])
            nc.scalar.dma_start(t[:, :, :, 256:512], wq[:, :, :, 256:512])
            nc.scalar.dma_start(id_sb[:], ident[:])
            wq_blk = [(t, 0)]
            nc.sync.dma_start(wv_t[:], wv[:])
            nc.sync.dma_start(et_t[:, :, :, 512:S], eT[:, :, :, 512:S])
            if use_mask:
                mask_sb = []
                for sc in range(SC):
                    mt = mkp.tile([128, TOK], BF, tag="mk", name="mk")
                    nc.sync.dma_start(mt[:], maskT[sc * 128:(sc + 1) * 128, :])
                    mask_sb.append(mt)

            def k_proj(sh, ms):
                pss = [psKV.tile([128, 512], FP, tag="psKV", name="psKV")
                       for _ in ms]
                for c in range(KC):
                    for i, m in enumerate(ms):
                        nc.tensor.matmul(
                            pss[i][:],
                            wk_t[:, c, :, m * 128:(m + 1) * 128],
                            et_t[:, c, :, sh * 512:(sh + 1) * 512],
                            start=(c == 0),
                            stop=(c == KC - 1),
                            perf_mode=DR,
                        )
                with nc.allow_low_precision(reason="k cast to fp8"):
                    for i, m in enumerate(ms):
                        for hi in range(2):
                            dst = kT_sb[2 * m + hi][:, 0,
                                                    sh * 512:(sh + 1) * 512]
                            if triv_bias:
                                nc.vector.tensor_copy(
                                    dst, pss[i][hi * 64:hi * 64 + 64, :])
                            else:
                                nc.vector.tensor_scalar_add(
                                    dst,
                                    pss[i][hi * 64:hi * 64 + 64, :],
                                    bk_sb[hi * 64:hi * 64 + 64, m:m + 1],
                                )

            def v_proj(scs):
                pss = [psKV.tile([128, 512], FP, tag="psKV", name="psKV")
                       for _ in scs]
                for c in range(KC):
                    for i, sc in enumerate(scs):
                        nc.tensor.matmul(
                            pss[i][:],
                            et_t[:, c, :, sc * 128:(sc + 1) * 128],
                            wv_t[:, c],
                            start=(c == 0),
                            stop=(c == KC - 1),
                            perf_mode=DR,
                        )
                with nc.allow_low_precision(reason="v cast to fp8"):
                    for i, sc in enumerate(scs):
                        if triv_bias:
                            nc.vector.tensor_copy(
                                vv_sb[sc // 4][:, sc % 4, :, 0:64],
                                pss[i].rearrange("p (h d) -> p h d", d=HD),
                            )
                        else:
                            nc.vector.tensor_add(
                                vv_sb[sc // 4][:, sc % 4, :, 0:64],
                                pss[i].rearrange("p (h d) -> p h d", d=HD),
                                bv_sb.rearrange("p (h d) -> p h d", d=HD),
                            )

            # interleave s-lo scores with the projections so ACT (the pole)
            # starts as soon as kT h0/h1 and qT 0-3 exist
            stash = {}
            k_proj(0, (0, 1))
            q_proj(0)
            q_proj(1)
            stash[(0, 0)] = scores_quad(0, 0)
            stash[(0, 1)] = scores_quad(0, 1)
            k_proj(0, (2, 3))
            stash[(1, 0)] = scores_quad(1, 0)
            stash[(1, 1)] = scores_quad(1, 1)
            v_proj((0, 1))
            q_proj(2)
            q_proj(3)
            stash[(2, 0)] = scores_quad(2, 0)
            stash[(2, 1)] = scores_quad(2, 1)
            v_proj((2, 3))
            stash[(3, 0)] = scores_quad(3, 0)
            stash[(3, 1)] = scores_quad(3, 1)
            k_proj(1, (0, 1))
            stash[(4, 0)] = scores_quad(4, 0)
            stash[(4, 1)] = scores_quad(4, 1)
            k_proj(1, (2, 3))
            v_proj((4, 5))
            v_proj((6, 7))

        # psKV released; open attention output pool in its banks
        psO = _stk.enter_context(tc.tile_pool(name="psO", bufs=2, space="PSUM"))

        wo_blks = []
        xres_sb = []

        # ---- Phase B: per-head scores+exp / lagged attn@V ----------------
        pending = []
        for hh in range(NH):
            if hh % 2 == 0 and 2 <= hh <= 24:
                m = 4 + (hh - 2) // 2
                if m % 4 == 0:  # new 512-col Wq block
                    t = wqp.tile([128, KC, 2, 512], F8, tag="wq", name="wqt")
                    nc.scalar.dma_start(
                        t[:], wq[:, :, :, (m // 4) * 512:(m // 4 + 1) * 512])
                    wq_blk[0] = (t, m)
                q_proj(m)
            if hh in (24, 28):  # prefetch Wo halves
                half = (hh - 24) // 4
                t = wop.tile([128, KC, 2, 1024], F8, tag="wo", name="wot")
                nc.scalar.dma_start(
                    t[:], wo[:, :, :, half * 1024:(half + 1) * 1024])
                wo_blks.append(t)
            if hh in (25, 27, 29, 31):  # prefetch residual rows
                tt = (hh - 25) // 2
                xt4 = xrp.tile([128, H], BF, tag="xr", name="xr")
                nc.sync.dma_start(xt4[:], xres[tt * 128:(tt + 1) * 128, :])
                xres_sb.append(xt4)
            exs = []
            for qd in range(4):
                if qd < 2 and (hh, qd) in stash:
                    exs.append(stash.pop((hh, qd)))
                else:
                    exs.append(scores_quad(hh, qd))
            pending.append((hh, exs))
            if len(pending) > 1:
                attn_v(*pending.pop(0))
        for p in pending:
            attn_v(*p)

        _stk.close()

        # ---- Phase C: output projection + residual + layernorm -----------
        with (
            tc.tile_pool(name="psC", bufs=8, space="PSUM") as psC,
            tc.tile_pool(name="outp", bufs=2) as outp,
        ):
            if not triv_ln:
                nc.sync.dma_start(gam_sb[:], gamr[:])
                nc.sync.dma_start(bet_sb[:], betr[:])

            for tt in range(4):
                ob = outp.tile([128, H], FP, tag="ob", name="ob")
                stats = lnp.tile([128, 4, 6], FP, tag="st", name="st")
                pss = []
                for nb in range(4):
                    ps = psC.tile([128, 512], FP, tag="psC", name="psC")
                    pss.append(ps)
                    for c in range(KC):
                        nc.tensor.matmul(
                            ps[:],
                            ctx_sb[:, 2 * c:2 * c + 2, tt * 128:(tt + 1) * 128],
                            wo_blks[nb // 2][:, c, :,
                                             (nb % 2) * 512:(nb % 2 + 1) * 512],
                            start=(c == 0),
                            stop=False,
                            perf_mode=DR,
                        )
                    sl = slice(nb * 512, (nb + 1) * 512)
                    # residual add on the PE: += I.T @ xres (bf16)
                    nc.tensor.matmul(
                        ps[:],
                        id_sb[:],
                        xres_sb[tt][:, sl],
                        start=False,
                        stop=True,
                    )
                    nc.vector.bn_stats(stats[:, nb, :], ps[:])
                mv = lnp.tile([128, 2], FP, tag="mv", name="mv")
                nc.vector.bn_aggr(mv[:], stats[:])
                std = lnp.tile([128, 1], FP, tag="sd", name="sd")
                nc.scalar.activation(
                    std[:], mv[:, 1:2], func=Sqrt, bias=eps_sb[:], scale=1.0
                )
                rstd = lnp.tile([128, 1], FP, tag="rs", name="rs")
                nc.vector.reciprocal(rstd[:], std[:])
                nmr = lnp.tile([128, 1], FP, tag="nm", name="nm")
                nc.vector.scalar_tensor_tensor(
                    nmr[:], mv[:, 0:1], -1.0, rstd[:],
                    op0=mybir.AluOpType.mult, op1=mybir.AluOpType.mult,
                )
                for nb in range(4):
                    sl = slice(nb * 512, (nb + 1) * 512)
                    # (ps - mu) * rstd, alternating DVE / ACT (= ps*rstd
                    # + (-mu*rstd) via Identity) -- Pool cannot read PSUM
                    if nb % 2 == 0:
                        nc.vector.tensor_scalar(
                            ob[:, sl],
                            pss[nb][:],
                            scalar1=mv[:, 0:1],
                            scalar2=rstd[:],
                            op0=mybir.AluOpType.subtract,
                            op1=mybir.AluOpType.mult,
                        )
                    else:
                        nc.scalar.activation(
                            ob[:, sl], pss[nb][:], func=Ident,
                            bias=nmr[:], scale=rstd[:],
                        )
                    if not triv_ln:
                        eng = nc.vector if nb % 2 == 0 else nc.gpsimd
                        eng.tensor_mul(ob[:, sl], ob[:, sl], gam_sb[:, sl])
                        eng.tensor_add(ob[:, sl], ob[:, sl], bet_sb[:, sl])
                    nc.sync.dma_start(out[tt * 128:(tt + 1) * 128, sl],
                                      ob[:, sl])

    nc.compile()
    return nc


def _get_nc(use_mask: bool, triv_ln: bool | None = None,
            triv_bias: bool | None = None):
    if triv_ln is None:
        triv_ln = _LAST_TRIV[0]
    if triv_bias is None:
        triv_bias = _LAST_TRIV[1]
    key = (use_mask, triv_ln, triv_bias)
    if key not in _CACHE:
        _CACHE[key] = _build(use_mask, triv_ln, triv_bias)
    return _CACHE[key]


_LAST_TRIV = [True, True]


def _pack_w(w, cols):
    """[2048, cols] fp32 -> [128, 8, 2, cols] fp8 with K-pair layout."""
    return np.ascontiguousarray(
        (w * WS).reshape(KC, 2, 128, cols).transpose(2, 0, 1, 3)
    ).astype(FP8)


def _pack_act(xT, cols):
    """[2048, cols] fp32 (feature-major) -> [128, 8, 2, cols] fp8."""
    return np.ascontiguousarray(
        xT.reshape(KC, 2, 128, cols).transpose(2, 0, 1, 3)
    ).astype(FP8)


def kernel(
    hidden_state,
    encoder_hidden_state,
    encoder_attention_mask,
    Wq, bq, Wk, bk, Wv, bv, Wo, bo, gamma, beta,
):
    hidden_state = np.asarray(hidden_state, dtype=np.float32)
    encoder_hidden_state = np.asarray(encoder_hidden_state, dtype=np.float32)
    encoder_attention_mask = np.asarray(encoder_attention_mask, dtype=np.float32)
    Wq = np.asarray(Wq, dtype=np.float32)
    bq = np.asarray(bq, dtype=np.float32)
    Wk = np.asarray(Wk, dtype=np.float32)
    bk = np.asarray(bk, dtype=np.float32)
    Wv = np.asarray(Wv, dtype=np.float32)
    bv = np.asarray(bv, dtype=np.float32)
    Wo = np.asarray(Wo, dtype=np.float32)
    bo = np.asarray(bo, dtype=np.float32)
    gamma = np.asarray(gamma, dtype=np.float32)
    beta = np.asarray(beta, dtype=np.float32)

    use_mask = bool(np.any(encoder_attention_mask))
    triv_ln = bool(np.all(gamma == 1.0) and np.all(beta == 0.0))
    triv_bias = not (np.any(bq) or np.any(bk) or np.any(bv))
    _LAST_TRIV[0] = triv_ln
    _LAST_TRIV[1] = triv_bias
    nc = _get_nc(use_mask, triv_ln, triv_bias)
    in_maps = _prepare_in_maps(
        hidden_state, encoder_hidden_state, encoder_attention_mask,
        Wq, bq, Wk, bk, Wv, bv, Wo, bo, gamma, beta, use_mask,
    )

    res = run_bass_kernel_spmd(nc, in_maps, core_ids=list(range(8)))
    kernel._last_results = res

    output = np.empty((B, L, H), dtype=np.float32)
    for c in range(8):
        b, lh = c // 2, c % 2
        output[b, lh * TOK:(lh + 1) * TOK, :] = res.results[c]["out"]
    return output


def _prepare_in_maps(
    hidden_state, encoder_hidden_state, encoder_attention_mask,
    Wq, bq, Wk, bk, Wv, bv, Wo, bo, gamma, beta, use_mask,
):
    triv_ln = bool(np.all(gamma == 1.0) and np.all(beta == 0.0))
    triv_bias = not (np.any(bq) or np.any(bk) or np.any(bv))
    wq_p = _pack_w(Wq, H)
    wk_p = _pack_w(Wk, KVH * HD)
    wv_p = _pack_w(Wv, KVH * HD)
    wo_p = _pack_w(Wo, H)
    if not triv_bias:
        bias_all = np.ascontiguousarray(np.concatenate(
            [
                bq.reshape(16, 128).T * WS,
                bk.reshape(4, 128).T * WS,
                np.tile(bv[None, :], (128, 1)) * WS,
            ],
            axis=1,
        ).astype(np.float32))
    ident = np.ascontiguousarray(np.eye(128, dtype=np.float32).astype(BF16))
    if not triv_ln:
        gamr = np.ascontiguousarray(
            np.tile(gamma[None, :].astype(BF16), (128, 1)))
        betr = np.ascontiguousarray(
            np.tile(beta[None, :].astype(BF16), (128, 1)))

    eT_by_b = [
        _pack_act(encoder_hidden_state[b].T, S) for b in range(B)
    ]

    in_maps = []
    for c in range(8):
        b, lh = c // 2, c % 2
        rows = hidden_state[b, lh * TOK:(lh + 1) * TOK, :]
        m = {
            "xT": _pack_act(rows.T, TOK),
            "xres": ((rows + bo[None, :]) * (WS * WS)).astype(BF16),
            "eT": eT_by_b[b],
            "wq": wq_p, "wk": wk_p, "wv": wv_p, "wo": wo_p,
            "ident": ident,
        }
        if not triv_bias:
            m["bias_all"] = bias_all
        if not triv_ln:
            m["gamr"] = gamr
            m["betr"] = betr
        if use_mask:
            mslice = encoder_attention_mask[b, 0, lh * TOK:(lh + 1) * TOK, :]
            m["maskT"] = np.ascontiguousarray(
                (mslice.T * (8.0 * WS * WS)).astype(BF16))
        in_maps.append(m)
    return in_maps


# revision 46
# speedup vs baseline: 1.6613x; 1.0194x over previous
"""Encoder-decoder GQA attention block (B=4, L=S=1024, H=2048, 32 Q heads,
8 KV heads, head_dim 64) + output projection + residual + layernorm, on 8
Trainium2 NeuronCores.

Sharding: rows. Core c handles batch c//2, L-half c%2 (512 query rows).
K/V projections are computed per-batch (duplicated on the 2 cores sharing a
batch), attention over all 32 heads for the core's rows, output projection,
residual + LN. No collectives.

v2: all large matmuls run fp8(e4m3) with perf_mode=DoubleRow (0.5 cyc/row,
two 128-deep K slices per instruction -> 4x the bf16 MAC rate). Weights are
pre-scaled x32 on the host so their ~N(0, 0.02) entries land in e4m3's
normal range; activations stay unscaled (~N(0,1)). Scale bookkeeping:
  q,k carry x32 each -> scores PSUM x1024 -> exp scale 2^-13 (=0.125/1024)
  exp is shifted by bias=-2 (values ~e^[-9,3]) to sit in e4m3 range; the
  V "ones column" is 1/32 so rowsum carries x(1/32) while ctx carries v's
  x32 -> ctx = 32*ctx_true, exactly what fp8 O-proj wants.
  O-proj PSUM = 1024*(ctx@Wo); residual is pre-scaled x1024 on the host and
  layernorm is scale-invariant (eps shift negligible), so no unscale op.
Scores keep K=64 contraction: DoubleRow's second K-slice is a zero pad in
kT (j=1 memset once per buffer), so the pair contributes k.T@q + 0*junk.
attn@V pairs s-chunks: exp tiles are [128s, 2sc, 512t], V is [128s, 4sc,
8h, 72] (72 = 64+1 rowsum col + 7 pad so the DoubleRow pair stride is
16B-aligned).

Engine budget (cost model): ACT exp is the pole (~133us: 128 ops x
[128,2,512]); PE ~90us total (fp8-DR everywhere, bf16 only for the K=1
recip broadcast); DVE ~90us (PSUM->SBUF casts, recip, ctx mul, LN stats);
Pool takes the zero-pad memsets, residual adds and the LN normalize mul;
DMA ~60us serialized. Schedule: scores for heads 0-7 (s-lo) interleave
with the K/V projections in phase A so ACT starts ~12us in; attn@V lags
one head behind exp; Wq streams in 512-col blocks, Wo prefetches late in
phase B; phase C is dense PE (O-proj) with per-row-block LN + store.

LayerNorm gamma/beta ops and their DMAs are skipped when gamma==1 and
beta==0 (detected at runtime; separate build variant), same for the mask.
"""

from contextlib import ExitStack

import numpy as np
import ml_dtypes

import concourse.bass as bass  # noqa: F401  (bass.AP used via handles)
import concourse.mybir as mybir
import concourse.tile as tile
from concourse import bacc
from concourse.bass_utils import run_bass_kernel_spmd

BF16 = ml_dtypes.bfloat16
FP8 = ml_dtypes.float8_e4m3fn

H = 2048
NH = 32
KVH = 8
G = 4           # query-head groups per kv head
HD = 64
B, L, S = 4, 1024, 1024
TOK = 512       # decoder rows per core
KC = 8          # contraction chunk-pairs (8 x (2x128) = 2048)
SC = S // 128   # 8 s chunks
EPS = 1e-6
WS = 32.0       # host-side fp8 weight scale

FP = mybir.dt.float32
BF = mybir.dt.bfloat16
F8 = mybir.dt.float8e4
DR = mybir.MatmulPerfMode.DoubleRow

_CACHE: dict = {}


def _build(use_mask: bool, triv_ln: bool, triv_bias: bool):
    nc = bacc.Bacc("TRN2", target_bir_lowering=False)

    xT = nc.dram_tensor("xT", [128, KC, 2, TOK], F8, kind="ExternalInput")
    xres = nc.dram_tensor("xres", [TOK, H], BF, kind="ExternalInput")
    eT = nc.dram_tensor("eT", [128, KC, 2, S], F8, kind="ExternalInput")
    wq = nc.dram_tensor("wq", [128, KC, 2, H], F8, kind="ExternalInput")
    wk = nc.dram_tensor("wk", [128, KC, 2, 512], F8, kind="ExternalInput")
    wv = nc.dram_tensor("wv", [128, KC, 2, 512], F8, kind="ExternalInput")
    wo = nc.dram_tensor("wo", [128, KC, 2, H], F8, kind="ExternalInput")
    # packed biases: [bq(16) | bk(4) | bv(512)] -> one DMA
    if not triv_bias:
        bias_all = nc.dram_tensor("bias_all", [128, 532], FP,
                                  kind="ExternalInput")
    ident = nc.dram_tensor("ident", [128, 128], BF, kind="ExternalInput")
    if not triv_ln:
        gamr = nc.dram_tensor("gamr", [128, H], BF, kind="ExternalInput")
        betr = nc.dram_tensor("betr", [128, H], BF, kind="ExternalInput")
    if use_mask:
        maskT = nc.dram_tensor("maskT", [S, TOK], BF, kind="ExternalInput")
    out = nc.dram_tensor("out", [TOK, H], FP, kind="ExternalOutput")

    Exp = mybir.ActivationFunctionType.Exp
    Ident = mybir.ActivationFunctionType.Identity
    Sqrt = mybir.ActivationFunctionType.Sqrt

    with tile.TileContext(nc) as tc:
      with (
          tc.tile_pool(name="ctxT", bufs=1) as ctxp,
          tc.tile_pool(name="cc", bufs=1) as ccp,
          tc.tile_pool(name="ln", bufs=10) as lnp,
          tc.tile_pool(name="qT", bufs=NH) as qtp,
          tc.tile_pool(name="kT", bufs=KVH) as ktp,
          tc.tile_pool(name="vv", bufs=2) as vvp,
          tc.tile_pool(name="expp", bufs=16) as expp,
          tc.tile_pool(name="rec", bufs=4) as recp,
          tc.tile_pool(name="bc", bufs=4) as bcp,
          tc.tile_pool(name="const", bufs=1) as constp,
          tc.tile_pool(name="wq", bufs=2) as wqp,
          tc.tile_pool(name="xTp", bufs=1) as xtp,
          tc.tile_pool(name="wo", bufs=2) as wop,
          tc.tile_pool(name="xr", bufs=4) as xrp,
          tc.tile_pool(name="maskp", bufs=SC if use_mask else 1) as mkp,
      ):
        eps_sb = ccp.tile([128, 1], FP, name="eps_sb")
        nb2_sb = ccp.tile([128, 1], FP, name="nb2_sb")
        junk_sb = ccp.tile([128, 1], FP, name="junk_sb")
        nc.vector.memset(eps_sb[:], EPS)
        nc.vector.memset(nb2_sb[:], -2.0)
        # touch the Sqrt act table now so its load isn't on the phase-C chain
        nc.scalar.activation(junk_sb[:], eps_sb[:], func=Sqrt,
                             bias=eps_sb[:], scale=1.0)
        if not triv_ln:
            gam_sb = ccp.tile([128, H], BF, name="gam_sb")
            bet_sb = ccp.tile([128, H], BF, name="bet_sb")

        ctx_sb = ctxp.tile([128, 16, TOK], F8, name="ctx")
        # qt/kt hold the DoubleRow K-pair on dim1; slice [:,1,:] is a zero
        # pad (memset once per buffer below) so K=64 contractions are legal.
        qT_sb = [qtp.tile([64, 2, TOK], F8, name="qt") for _ in range(NH)]
        kT_sb = [ktp.tile([64, 2, S], F8, name="kt") for _ in range(KVH)]
        # V: [s-part, sc-in-quad, kv-head, 64+1(rowsum)+7(pad to 16B)]
        vv_sb = [vvp.tile([128, 4, KVH, 72], F8, name="vv") for _ in range(2)]

        # zero the DoubleRow pads: early-needed ones on DVE (idle until
        # the first PSUM copies ~17us), the rest on Pool
        for t in kT_sb:
            nc.vector.memset(t[:, 1, :], 0.0)
        for t in qT_sb[:8]:
            nc.vector.memset(t[:, 1, :], 0.0)
        for t in vv_sb:
            nc.gpsimd.memset(t[:, :, :, 64:65], 1.0)
        for t in qT_sb[8:]:
            nc.gpsimd.memset(t[:, 1, :], 0.0)

        _stk = ExitStack()
        psA = _stk.enter_context(tc.tile_pool(name="psA", bufs=2, space="PSUM"))
        psS = _stk.enter_context(tc.tile_pool(name="psS", bufs=2, space="PSUM"))

        id_sb = constp.tile([128, 128], BF, name="id_sb")
        if not triv_bias:
            bias_sb = constp.tile([128, 532], FP, name="bias_sb")
            bq_sb = bias_sb[:, 0:16]
            bk_sb = bias_sb[:, 16:20]
            bv_sb = bias_sb[:, 20:532]

        def q_proj(m):
            blk, base = wq_blk[0]
            q = m - base
            assert 0 <= q < 4, (m, base)
            ps = psA.tile([128, TOK], FP, tag="psA", name="psA")
            for c in range(KC):
                nc.tensor.matmul(
                    ps[:],
                    blk[:, c, :, q * 128:(q + 1) * 128],
                    xt_t[:, c],
                    start=(c == 0),
                    stop=(c == KC - 1),
                    perf_mode=DR,
                )
            with nc.allow_low_precision(reason="q cast to fp8 for scores"):
                for hi in range(2):
                    if triv_bias:
                        nc.vector.tensor_copy(
                            qT_sb[2 * m + hi][:, 0, :],
                            ps[hi * 64:hi * 64 + 64, :],
                        )
                    else:
                        nc.vector.tensor_scalar_add(
                            qT_sb[2 * m + hi][:, 0, :],
                            ps[hi * 64:hi * 64 + 64, :],
                            bq_sb[hi * 64:hi * 64 + 64, m:m + 1],
                        )

        def scores_quad(hh, qd):
            h = hh // G
            ps = psS.tile([128, 2, TOK], FP, tag="psS", name="psS")
            for i in range(2):
                sc = 2 * qd + i
                nc.tensor.matmul(
                    ps[:, i, :],
                    kT_sb[h][:, :, sc * 128:(sc + 1) * 128],
                    qT_sb[hh][:],
                    start=True,
                    stop=True,
                    perf_mode=DR,
                )
                if use_mask:
                    nc.vector.tensor_add(ps[:, i, :], ps[:, i, :],
                                         mask_sb[sc][:])
            ex = expp.tile([128, 2, TOK], F8, tag="ex", name="ex")
            nc.scalar.activation(ex[:], ps[:], func=Exp,
                                 scale=1.0 / 8192.0, bias=nb2_sb[:])
            return ex

        def attn_v(hh, exs):
            h = hh // G
            po = psO.tile([128, TOK], FP, tag="psO", name="psO")
            for qd in range(4):
                vq = vv_sb[qd // 2][:, (qd % 2) * 2:(qd % 2) * 2 + 2, h, 0:65]
                nc.tensor.matmul(
                    po[0:65, :],
                    vq,
                    exs[qd][:],
                    start=(qd == 0),
                    stop=(qd == 3),
                    perf_mode=DR,
                )
            recb = recp.tile([1, TOK], BF, tag="recb", name="recb")
            with nc.allow_low_precision(reason="softmax recip rounds to bf16"):
                nc.vector.reciprocal(recb[:], po[64:65, :])
            # broadcast recip across 64 partitions on the idle Pool engine
            rb = bcp.tile([64, TOK], BF, tag="rb", name="rb")
            nc.gpsimd.partition_broadcast(rb[:], recb[:])
            with nc.allow_low_precision(reason="ctx cast to fp8 for O-proj"):
                nc.vector.tensor_mul(
                    ctx_sb[(hh % 2) * 64:(hh % 2) * 64 + 64, hh // 2, :],
                    po[0:64, :],
                    rb[:],
                )

        # ---- Phase A: input DMAs, K/V/Q projections, s-lo scores ---------
        with (
            tc.tile_pool(name="eTp", bufs=1) as etp,
            tc.tile_pool(name="wk", bufs=1) as wkp,
            tc.tile_pool(name="wv", bufs=1) as wvp,
            tc.tile_pool(name="psKV", bufs=2, space="PSUM") as psKV,
        ):
            wk_t = wkp.tile([128, KC, 2, 512], F8, name="wkt")
            et_t = etp.tile([128, KC, 2, S], F8, name="et")
            wv_t = wvp.tile([128, KC, 2, 512], F8, name="wvt")
            xt_t = xtp.tile([128, KC, 2, TOK], F8, name="xt")
            # few big DMAs: descriptor-gen (HWDGE) is a serial shared
            # resource (~0.63us per DMA instruction), so chunking loses
            # PE p-state warm-up during the initial DMA wait: dummy
            # matmuls on the already-zeroed kT pads keep pe_busy_start
            # anchored early so the first real chains run at full clock
            wps = psA.tile([128, TOK], FP, tag="psA", name="psA")
            for _ in range(40):
                nc.tensor.matmul(
                    wps[:, 0:512], kT_sb[0][:, 1, 0:128], kT_sb[0][:, 1, 0:512],
                    start=True, stop=True,
                )
            # critical-path loads on SP; decoder-side on ACT queue;
            # late-needed on DVE queue (DMA engine is shared+serial, queue
            # split biases acquisition order toward the critical path)
            nc.sync.dma_start(wk_t[:], wk[:])
            nc.sync.dma_start(et_t[:, :, :, 0:512], eT[:, :, :, 0:512])
            if not triv_bias:
                nc.sync.dma_start(bias_sb[:], bias_all[:])
            nc.sync.dma_start(xt_t[:], xT[:])
            t = wqp.tile([128, KC, 2, 512], F8, tag="wq", name="wqt")
            nc.scalar.dma_start(t[:, :, :, 0:256], wq[:, :, :, 0:256])
            nc.scalar.dma_start(t[:, :, :, 256:512], wq[:, :, :, 256:512])
            nc.scalar.dma_start(id_sb[:], ident[:])
            wq_blk = [(t, 0)]
            nc.sync.dma_start(wv_t[:], wv[:])
            nc.sync.dma_start(et_t[:, :, :, 512:S], eT[:, :, :, 512:S])
            if use_mask:
                mask_sb = []
                for sc in range(SC):
                    mt = mkp.tile([128, TOK], BF, tag="mk", name="mk")
                    nc.sync.dma_start(mt[:], maskT[sc * 128:(sc + 1) * 128, :])
                    mask_sb.append(mt)

            def k_proj(sh, ms):
                pss = [psKV.tile([128, 512], FP, tag="psKV", name="psKV")
                       for _ in ms]
                for c in range(KC):
                    for i, m in enumerate(ms):
                        nc.tensor.matmul(
                            pss[i][:],
                            wk_t[:, c, :, m * 128:(m + 1) * 128],
                            et_t[:, c, :, sh * 512:(sh + 1) * 512],
                            start=(c == 0),
                            stop=(c == KC - 1),
                            perf_mode=DR,
                        )
                with nc.allow_low_precision(reason="k cast to fp8"):
                    for i, m in enumerate(ms):
                        for hi in range(2):
                            dst = kT_sb[2 * m + hi][:, 0,
                                                    sh * 512:(sh + 1) * 512]
                            if triv_bias:
                                nc.vector.tensor_copy(
                                    dst, pss[i][hi * 64:hi * 64 + 64, :])
                            else:
                                nc.vector.tensor_scalar_add(
                                    dst,
                                    pss[i][hi * 64:hi * 64 + 64, :],
                                    bk_sb[hi * 64:hi * 64 + 64, m:m + 1],
                                )

            def v_proj(scs):
                pss = [psKV.tile([128, 512], FP, tag="psKV", name="psKV")
                       for _ in scs]
                for c in range(KC):
                    for i, sc in enumerate(scs):
                        nc.tensor.matmul(
                            pss[i][:],
                            et_t[:, c, :, sc * 128:(sc + 1) * 128],
                            wv_t[:, c],
                            start=(c == 0),
                            stop=(c == KC - 1),
                            perf_mode=DR,
                        )
                with nc.allow_low_precision(reason="v cast to fp8"):
                    for i, sc in enumerate(scs):
                        if triv_bias:
                            nc.vector.tensor_copy(
                                vv_sb[sc // 4][:, sc % 4, :, 0:64],
                                pss[i].rearrange("p (h d) -> p h d", d=HD),
                            )
                        else:
                            nc.vector.tensor_add(
                                vv_sb[sc // 4][:, sc % 4, :, 0:64],
                                pss[i].rearrange("p (h d) -> p h d", d=HD),
                                bv_sb.rearrange("p (h d) -> p h d", d=HD),
                            )

            # interleave s-lo scores with the projections so ACT (the pole)
            # starts as soon as kT h0/h1 and qT 0-3 exist
            stash = {}
            k_proj(0, (0, 1))
            q_proj(0)
            q_proj(1)
            stash[(0, 0)] = scores_quad(0, 0)
            stash[(0, 1)] = scores_quad(0, 1)
            k_proj(0, (2, 3))
            stash[(1, 0)] = scores_quad(1, 0)
            stash[(1, 1)] = scores_quad(1, 1)
            v_proj((0, 1))
            q_proj(2)
            q_proj(3)
            stash[(2, 0)] = scores_quad(2, 0)
            stash[(2, 1)] = scores_quad(2, 1)
            v_proj((2, 3))
            stash[(3, 0)] = scores_quad(3, 0)
            stash[(3, 1)] = scores_quad(3, 1)
            k_proj(1, (0, 1))
            stash[(4, 0)] = scores_quad(4, 0)
            stash[(4, 1)] = scores_quad(4, 1)
            stash[(5, 0)] = scores_quad(5, 0)
            stash[(5, 1)] = scores_quad(5, 1)
            k_proj(1, (2, 3))
            v_proj((4, 5))
            v_proj((6, 7))

        # psKV released; open attention output pool in its banks
        psO = _stk.enter_context(tc.tile_pool(name="psO", bufs=2, space="PSUM"))

        wo_blks = []
        xres_sb = []

        # ---- Phase B: per-head scores+exp / lagged attn@V ----------------
        pending = []
        for hh in range(NH):
            if hh % 2 == 0 and 2 <= hh <= 24:
                m = 4 + (hh - 2) // 2
                if m % 4 == 0:  # new 512-col Wq block
                    t = wqp.tile([128, KC, 2, 512], F8, tag="wq", name="wqt")
                    nc.scalar.dma_start(
                        t[:], wq[:, :, :, (m // 4) * 512:(m // 4 + 1) * 512])
                    wq_blk[0] = (t, m)
                q_proj(m)
            if hh in (24, 28):  # prefetch Wo halves
                half = (hh - 24) // 4
                t = wop.tile([128, KC, 2, 1024], F8, tag="wo", name="wot")
                nc.scalar.dma_start(
                    t[:], wo[:, :, :, half * 1024:(half + 1) * 1024])
                wo_blks.append(t)
            if hh in (25, 27, 29, 31):  # prefetch residual rows
                tt = (hh - 25) // 2
                xt4 = xrp.tile([128, H], BF, tag="xr", name="xr")
                nc.sync.dma_start(xt4[:], xres[tt * 128:(tt + 1) * 128, :])
                xres_sb.append(xt4)
            exs = []
            for qd in range(4):
                if qd < 2 and (hh, qd) in stash:
                    exs.append(stash.pop((hh, qd)))
                else:
                    exs.append(scores_quad(hh, qd))
            pending.append((hh, exs))
            if len(pending) > 1:
                attn_v(*pending.pop(0))
        for p in pending:
            attn_v(*p)

        _stk.close()

        # ---- Phase C: output projection + residual + layernorm -----------
        with (
            tc.tile_pool(name="psC", bufs=8, space="PSUM") as psC,
            tc.tile_pool(name="outp", bufs=2) as outp,
        ):
            if not triv_ln:
                nc.sync.dma_start(gam_sb[:], gamr[:])
                nc.sync.dma_start(bet_sb[:], betr[:])

            for tt in range(4):
                ob = outp.tile([128, H], FP, tag="ob", name="ob")
                stats = lnp.tile([128, 4, 6], FP, tag="st", name="st")
                pss = []
                for nb in range(4):
                    ps = psC.tile([128, 512], FP, tag="psC", name="psC")
                    pss.append(ps)
                    for c in range(KC):
                        nc.tensor.matmul(
                            ps[:],
                            ctx_sb[:, 2 * c:2 * c + 2, tt * 128:(tt + 1) * 128],
                            wo_blks[nb // 2][:, c, :,
                                             (nb % 2) * 512:(nb % 2 + 1) * 512],
                            start=(c == 0),
                            stop=False,
                            perf_mode=DR,
                        )
                    sl = slice(nb * 512, (nb + 1) * 512)
                    # residual add on the PE: += I.T @ xres (bf16)
                    nc.tensor.matmul(
                        ps[:],
                        id_sb[:],
                        xres_sb[tt][:, sl],
                        start=False,
                        stop=True,
                    )
                    nc.vector.bn_stats(stats[:, nb, :], ps[:])
                mv = lnp.tile([128, 2], FP, tag="mv", name="mv")
                nc.vector.bn_aggr(mv[:], stats[:])
                std = lnp.tile([128, 1], FP, tag="sd", name="sd")
                nc.scalar.activation(
                    std[:], mv[:, 1:2], func=Sqrt, bias=eps_sb[:], scale=1.0
                )
                rstd = lnp.tile([128, 1], FP, tag="rs", name="rs")
                nc.vector.reciprocal(rstd[:], std[:])
                nmr = lnp.tile([128, 1], FP, tag="nm", name="nm")
                nc.vector.scalar_tensor_tensor(
                    nmr[:], mv[:, 0:1], -1.0, rstd[:],
                    op0=mybir.AluOpType.mult, op1=mybir.AluOpType.mult,
                )
                for nb in range(4):
                    sl = slice(nb * 512, (nb + 1) * 512)
                    # (ps - mu) * rstd, alternating DVE / ACT (= ps*rstd
                    # + (-mu*rstd) via Identity) -- Pool cannot read PSUM
                    if nb % 2 == 0:
                        nc.vector.tensor_scalar(
                            ob[:, sl],
                            pss[nb][:],
                            scalar1=mv[:, 0:1],
                            scalar2=rstd[:],
                            op0=mybir.AluOpType.subtract,
                            op1=mybir.AluOpType.mult,
                        )
                    else:
                        nc.scalar.activation(
                            ob[:, sl], pss[nb][:], func=Ident,
                            bias=nmr[:], scale=rstd[:],
                        )
                    if not triv_ln:
                        eng = nc.vector if nb % 2 == 0 else nc.gpsimd
                        eng.tensor_mul(ob[:, sl], ob[:, sl], gam_sb[:, sl])
                        eng.tensor_add(ob[:, sl], ob[:, sl], bet_sb[:, sl])
                    nc.sync.dma_start(out[tt * 128:(tt + 1) * 128, sl],
                                      ob[:, sl])

    nc.compile()
    return nc


def _get_nc(use_mask: bool, triv_ln: bool | None = None,
            triv_bias: bool | None = None):
    if triv_ln is None:
        triv_ln = _LAST_TRIV[0]
    if triv_bias is None:
        triv_bias = _LAST_TRIV[1]
    key = (use_mask, triv_ln, triv_bias)
    if key not in _CACHE:
        _CACHE[key] = _build(use_mask, triv_ln, triv_bias)
    return _CACHE[key]


_LAST_TRIV = [True, True]


def _pack_w(w, cols):
    """[2048, cols] fp32 -> [128, 8, 2, cols] fp8 with K-pair layout."""
    return np.ascontiguousarray(
        (w * WS).reshape(KC, 2, 128, cols).transpose(2, 0, 1, 3)
    ).astype(FP8)


def _pack_act(xT, cols):
    """[2048, cols] fp32 (feature-major) -> [128, 8, 2, cols] fp8."""
    return np.ascontiguousarray(
        xT.reshape(KC, 2, 128, cols).transpose(2, 0, 1, 3)
    ).astype(FP8)


def kernel(
    hidden_state,
    encoder_hidden_state,
    encoder_attention_mask,
    Wq, bq, Wk, bk, Wv, bv, Wo, bo, gamma, beta,
):
    hidden_state = np.asarray(hidden_state, dtype=np.float32)
    encoder_hidden_state = np.asarray(encoder_hidden_state, dtype=np.float32)
    encoder_attention_mask = np.asarray(encoder_attention_mask, dtype=np.float32)
    Wq = np.asarray(Wq, dtype=np.float32)
    bq = np.asarray(bq, dtype=np.float32)
    Wk = np.asarray(Wk, dtype=np.float32)
    bk = np.asarray(bk, dtype=np.float32)
    Wv = np.asarray(Wv, dtype=np.float32)
    bv = np.asarray(bv, dtype=np.float32)
    Wo = np.asarray(Wo, dtype=np.float32)
    bo = np.asarray(bo, dtype=np.float32)
    gamma = np.asarray(gamma, dtype=np.float32)
    beta = np.asarray(beta, dtype=np.float32)

    use_mask = bool(np.any(encoder_attention_mask))
    triv_ln = bool(np.all(gamma == 1.0) and np.all(beta == 0.0))
    triv_bias = not (np.any(bq) or np.any(bk) or np.any(bv))
    _LAST_TRIV[0] = triv_ln
    _LAST_TRIV[1] = triv_bias
    nc = _get_nc(use_mask, triv_ln, triv_bias)
    in_maps = _prepare_in_maps(
        hidden_state, encoder_hidden_state, encoder_attention_mask,
        Wq, bq, Wk, bk, Wv, bv, Wo, bo, gamma, beta, use_mask,
    )

    res = run_bass_kernel_spmd(nc, in_maps, core_ids=list(range(8)))
    kernel._last_results = res

    output = np.empty((B, L, H), dtype=np.float32)
    for c in range(8):
        b, lh = c // 2, c % 2
        output[b, lh * TOK:(lh + 1) * TOK, :] = res.results[c]["out"]
    return output


def _prepare_in_maps(
    hidden_state, encoder_hidden_state, encoder_attention_mask,
    Wq, bq, Wk, bk, Wv, bv, Wo, bo, gamma, beta, use_mask,
):
    triv_ln = bool(np.all(gamma == 1.0) and np.all(beta == 0.0))
    triv_bias = not (np.any(bq) or np.any(bk) or np.any(bv))
    wq_p = _pack_w(Wq, H)
    wk_p = _pack_w(Wk, KVH * HD)
    wv_p = _pack_w(Wv, KVH * HD)
    wo_p = _pack_w(Wo, H)
    if not triv_bias:
        bias_all = np.ascontiguousarray(np.concatenate(
            [
                bq.reshape(16, 128).T * WS,
                bk.reshape(4, 128).T * WS,
                np.tile(bv[None, :], (128, 1)) * WS,
            ],
            axis=1,
        ).astype(np.float32))
    ident = np.ascontiguousarray(np.eye(128, dtype=np.float32).astype(BF16))
    if not triv_ln:
        gamr = np.ascontiguousarray(
            np.tile(gamma[None, :].astype(BF16), (128, 1)))
        betr = np.ascontiguousarray(
            np.tile(beta[None, :].astype(BF16), (128, 1)))

    eT_by_b = [
        _pack_act(encoder_hidden_state[b].T, S) for b in range(B)
    ]

    in_maps = []
    for c in range(8):
        b, lh = c // 2, c % 2
        rows = hidden_state[b, lh * TOK:(lh + 1) * TOK, :]
        m = {
            "xT": _pack_act(rows.T, TOK),
            "xres": ((rows + bo[None, :]) * (WS * WS)).astype(BF16),
            "eT": eT_by_b[b],
            "wq": wq_p, "wk": wk_p, "wv": wv_p, "wo": wo_p,
            "ident": ident,
        }
        if not triv_bias:
            m["bias_all"] = bias_all
        if not triv_ln:
            m["gamr"] = gamr
            m["betr"] = betr
        if use_mask:
            mslice = encoder_attention_mask[b, 0, lh * TOK:(lh + 1) * TOK, :]
            m["maskT"] = np.ascontiguousarray(
                (mslice.T * (8.0 * WS * WS)).astype(BF16))
        in_maps.append(m)
    return in_maps


# revision 52
# speedup vs baseline: 1.6624x; 1.0006x over previous
"""Encoder-decoder GQA attention block (B=4, L=S=1024, H=2048, 32 Q heads,
8 KV heads, head_dim 64) + output projection + residual + layernorm, on 8
Trainium2 NeuronCores.

Sharding: rows. Core c handles batch c//2, L-half c%2 (512 query rows).
K/V projections are computed per-batch (duplicated on the 2 cores sharing a
batch), attention over all 32 heads for the core's rows, output projection,
residual + LN. No collectives.

v2: all large matmuls run fp8(e4m3) with perf_mode=DoubleRow (0.5 cyc/row,
two 128-deep K slices per instruction -> 4x the bf16 MAC rate). Weights are
pre-scaled x32 on the host so their ~N(0, 0.02) entries land in e4m3's
normal range; activations stay unscaled (~N(0,1)). Scale bookkeeping:
  q,k carry x32 each -> scores PSUM x1024 -> exp scale 2^-13 (=0.125/1024)
  exp is shifted by bias=-2 (values ~e^[-9,3]) to sit in e4m3 range; the
  V "ones column" is 1/32 so rowsum carries x(1/32) while ctx carries v's
  x32 -> ctx = 32*ctx_true, exactly what fp8 O-proj wants.
  O-proj PSUM = 1024*(ctx@Wo); residual is pre-scaled x1024 on the host and
  layernorm is scale-invariant (eps shift negligible), so no unscale op.
Scores keep K=64 contraction: DoubleRow's second K-slice is a zero pad in
kT (j=1 memset once per buffer), so the pair contributes k.T@q + 0*junk.
attn@V pairs s-chunks: exp tiles are [128s, 2sc, 512t], V is [128s, 4sc,
8h, 72] (72 = 64+1 rowsum col + 7 pad so the DoubleRow pair stride is
16B-aligned).

Engine budget (cost model): ACT exp is the pole (~133us: 128 ops x
[128,2,512]); PE ~90us total (fp8-DR everywhere, bf16 only for the K=1
recip broadcast); DVE ~90us (PSUM->SBUF casts, recip, ctx mul, LN stats);
Pool takes the zero-pad memsets, residual adds and the LN normalize mul;
DMA ~60us serialized. Schedule: scores for heads 0-7 (s-lo) interleave
with the K/V projections in phase A so ACT starts ~12us in; attn@V lags
one head behind exp; Wq streams in 512-col blocks, Wo prefetches late in
phase B; phase C is dense PE (O-proj) with per-row-block LN + store.

LayerNorm gamma/beta ops and their DMAs are skipped when gamma==1 and
beta==0 (detected at runtime; separate build variant), same for the mask.
"""

from contextlib import ExitStack

import numpy as np
import ml_dtypes

import concourse.bass as bass  # noqa: F401  (bass.AP used via handles)
import concourse.mybir as mybir
import concourse.tile as tile
from concourse import bacc
from concourse.bass_utils import run_bass_kernel_spmd

BF16 = ml_dtypes.bfloat16
FP8 = ml_dtypes.float8_e4m3fn

H = 2048
NH = 32
KVH = 8
G = 4           # query-head groups per kv head
HD = 64
B, L, S = 4, 1024, 1024
TOK = 512       # decoder rows per core
KC = 8          # contraction chunk-pairs (8 x (2x128) = 2048)
SC = S // 128   # 8 s chunks
EPS = 1e-6
WS = 32.0       # host-side fp8 weight scale

FP = mybir.dt.float32
BF = mybir.dt.bfloat16
F8 = mybir.dt.float8e4
DR = mybir.MatmulPerfMode.DoubleRow

_CACHE: dict = {}


def _build(use_mask: bool, triv_ln: bool, triv_bias: bool):
    nc = bacc.Bacc("TRN2", target_bir_lowering=False)

    xT = nc.dram_tensor("xT", [128, KC, 2, TOK], F8, kind="ExternalInput")
    xres = nc.dram_tensor("xres", [TOK, H], BF, kind="ExternalInput")
    eT = nc.dram_tensor("eT", [128, KC, 2, S], F8, kind="ExternalInput")
    wq = nc.dram_tensor("wq", [128, KC, 2, H], F8, kind="ExternalInput")
    wk = nc.dram_tensor("wk", [128, KC, 2, 512], F8, kind="ExternalInput")
    wv = nc.dram_tensor("wv", [128, KC, 2, 512], F8, kind="ExternalInput")
    wo = nc.dram_tensor("wo", [128, KC, 2, H], F8, kind="ExternalInput")
    # packed biases: [bq(16) | bk(4) | bv(512)] -> one DMA
    if not triv_bias:
        bias_all = nc.dram_tensor("bias_all", [128, 532], FP,
                                  kind="ExternalInput")
    ident = nc.dram_tensor("ident", [128, 128], BF, kind="ExternalInput")
    if not triv_ln:
        gamr = nc.dram_tensor("gamr", [128, H], BF, kind="ExternalInput")
        betr = nc.dram_tensor("betr", [128, H], BF, kind="ExternalInput")
    if use_mask:
        maskT = nc.dram_tensor("maskT", [S, TOK], BF, kind="ExternalInput")
    out = nc.dram_tensor("out", [TOK, H], FP, kind="ExternalOutput")

    Exp = mybir.ActivationFunctionType.Exp
    Ident = mybir.ActivationFunctionType.Identity
    Sqrt = mybir.ActivationFunctionType.Sqrt

    with tile.TileContext(nc) as tc:
      with (
          tc.tile_pool(name="ctxT", bufs=1) as ctxp,
          tc.tile_pool(name="cc", bufs=1) as ccp,
          tc.tile_pool(name="ln", bufs=10) as lnp,
          tc.tile_pool(name="qT", bufs=NH) as qtp,
          tc.tile_pool(name="kT", bufs=KVH) as ktp,
          tc.tile_pool(name="vv", bufs=2) as vvp,
          tc.tile_pool(name="expp", bufs=16) as expp,
          tc.tile_pool(name="rec", bufs=6) as recp,
          tc.tile_pool(name="bc", bufs=6) as bcp,
          tc.tile_pool(name="const", bufs=1) as constp,
          tc.tile_pool(name="wq", bufs=2) as wqp,
          tc.tile_pool(name="xTp", bufs=1) as xtp,
          tc.tile_pool(name="wo", bufs=2) as wop,
          tc.tile_pool(name="xr", bufs=4) as xrp,
          tc.tile_pool(name="maskp", bufs=SC if use_mask else 1) as mkp,
      ):
        eps_sb = ccp.tile([128, 1], FP, name="eps_sb")
        nb2_sb = ccp.tile([128, 1], FP, name="nb2_sb")
        junk_sb = ccp.tile([128, 1], FP, name="junk_sb")
        nc.vector.memset(eps_sb[:], EPS)
        nc.vector.memset(nb2_sb[:], -2.0)
        # touch the Sqrt act table now so its load isn't on the phase-C chain
        nc.scalar.activation(junk_sb[:], eps_sb[:], func=Sqrt,
                             bias=eps_sb[:], scale=1.0)
        if not triv_ln:
            gam_sb = ccp.tile([128, H], BF, name="gam_sb")
            bet_sb = ccp.tile([128, H], BF, name="bet_sb")

        ctx_sb = ctxp.tile([128, 16, TOK], F8, name="ctx")
        # qt/kt hold the DoubleRow K-pair on dim1; slice [:,1,:] is a zero
        # pad (memset once per buffer below) so K=64 contractions are legal.
        qT_sb = [qtp.tile([64, 2, TOK], F8, name="qt") for _ in range(NH)]
        kT_sb = [ktp.tile([64, 2, S], F8, name="kt") for _ in range(KVH)]
        # V: [s-part, sc-in-quad, kv-head, 64+1(rowsum)+7(pad to 16B)]
        vv_sb = [vvp.tile([128, 4, KVH, 72], F8, name="vv") for _ in range(2)]

        # zero the DoubleRow pads: early-needed ones on DVE (idle until
        # the first PSUM copies ~17us), the rest on Pool
        for t in kT_sb:
            nc.vector.memset(t[:, 1, :], 0.0)
        for t in qT_sb[:8]:
            nc.vector.memset(t[:, 1, :], 0.0)
        for t in vv_sb:
            nc.gpsimd.memset(t[:, :, :, 64:65], 1.0)
        for t in qT_sb[8:]:
            nc.gpsimd.memset(t[:, 1, :], 0.0)

        _stk = ExitStack()
        psA = _stk.enter_context(tc.tile_pool(name="psA", bufs=2, space="PSUM"))
        psS = _stk.enter_context(tc.tile_pool(name="psS", bufs=2, space="PSUM"))

        id_sb = constp.tile([128, 128], BF, name="id_sb")
        if not triv_bias:
            bias_sb = constp.tile([128, 532], FP, name="bias_sb")
            bq_sb = bias_sb[:, 0:16]
            bk_sb = bias_sb[:, 16:20]
            bv_sb = bias_sb[:, 20:532]

        def q_proj(m):
            blk, base = wq_blk[0]
            q = m - base
            assert 0 <= q < 4, (m, base)
            ps = psA.tile([128, TOK], FP, tag="psA", name="psA")
            for c in range(KC):
                nc.tensor.matmul(
                    ps[:],
                    blk[:, c, :, q * 128:(q + 1) * 128],
                    xt_t[:, c],
                    start=(c == 0),
                    stop=(c == KC - 1),
                    perf_mode=DR,
                )
            with nc.allow_low_precision(reason="q cast to fp8 for scores"):
                for hi in range(2):
                    if triv_bias:
                        nc.vector.tensor_copy(
                            qT_sb[2 * m + hi][:, 0, :],
                            ps[hi * 64:hi * 64 + 64, :],
                        )
                    else:
                        nc.vector.tensor_scalar_add(
                            qT_sb[2 * m + hi][:, 0, :],
                            ps[hi * 64:hi * 64 + 64, :],
                            bq_sb[hi * 64:hi * 64 + 64, m:m + 1],
                        )

        def scores_quad(hh, qd):
            h = hh // G
            ps = psS.tile([128, 2, TOK], FP, tag="psS", name="psS")
            for i in range(2):
                sc = 2 * qd + i
                nc.tensor.matmul(
                    ps[:, i, :],
                    kT_sb[h][:, :, sc * 128:(sc + 1) * 128],
                    qT_sb[hh][:],
                    start=True,
                    stop=True,
                    perf_mode=DR,
                )
                if use_mask:
                    nc.vector.tensor_add(ps[:, i, :], ps[:, i, :],
                                         mask_sb[sc][:])
            ex = expp.tile([128, 2, TOK], F8, tag="ex", name="ex")
            nc.scalar.activation(ex[:], ps[:], func=Exp,
                                 scale=1.0 / 8192.0, bias=nb2_sb[:])
            return ex

        def attn_v(hh, exs):
            h = hh // G
            po = psO.tile([128, TOK], FP, tag="psO", name="psO")
            for qd in range(4):
                vq = vv_sb[qd // 2][:, (qd % 2) * 2:(qd % 2) * 2 + 2, h, 0:65]
                nc.tensor.matmul(
                    po[0:65, :],
                    vq,
                    exs[qd][:],
                    start=(qd == 0),
                    stop=(qd == 3),
                    perf_mode=DR,
                )
            recb = recp.tile([1, TOK], BF, tag="recb", name="recb")
            with nc.allow_low_precision(reason="softmax recip rounds to bf16"):
                nc.vector.reciprocal(recb[:], po[64:65, :])
            # broadcast recip across 64 partitions on the idle Pool engine
            rb = bcp.tile([64, TOK], BF, tag="rb", name="rb")
            nc.gpsimd.partition_broadcast(rb[:], recb[:])
            with nc.allow_low_precision(reason="ctx cast to fp8 for O-proj"):
                nc.vector.tensor_mul(
                    ctx_sb[(hh % 2) * 64:(hh % 2) * 64 + 64, hh // 2, :],
                    po[0:64, :],
                    rb[:],
                )

        # ---- Phase A: input DMAs, K/V/Q projections, s-lo scores ---------
        with (
            tc.tile_pool(name="eTp", bufs=1) as etp,
            tc.tile_pool(name="wk", bufs=1) as wkp,
            tc.tile_pool(name="wv", bufs=1) as wvp,
            tc.tile_pool(name="psKV", bufs=2, space="PSUM") as psKV,
        ):
            wk_t = wkp.tile([128, KC, 2, 512], F8, name="wkt")
            et_t = etp.tile([128, KC, 2, S], F8, name="et")
            wv_t = wvp.tile([128, KC, 2, 512], F8, name="wvt")
            xt_t = xtp.tile([128, KC, 2, TOK], F8, name="xt")
            # few big DMAs: descriptor-gen (HWDGE) is a serial shared
            # resource (~0.63us per DMA instruction), so chunking loses
            # PE p-state warm-up during the initial DMA wait: dummy
            # matmuls on the already-zeroed kT pads keep pe_busy_start
            # anchored early so the first real chains run at full clock
            wps = psA.tile([128, TOK], FP, tag="psA", name="psA")
            for _ in range(40):
                nc.tensor.matmul(
                    wps[:, 0:512], kT_sb[0][:, 1, 0:128], kT_sb[0][:, 1, 0:512],
                    start=True, stop=True,
                )
            # critical-path loads on SP; decoder-side on ACT queue;
            # late-needed on DVE queue (DMA engine is shared+serial, queue
            # split biases acquisition order toward the critical path)
            nc.sync.dma_start(wk_t[:], wk[:])
            nc.sync.dma_start(et_t[:, :, :, 0:512], eT[:, :, :, 0:512])
            if not triv_bias:
                nc.sync.dma_start(bias_sb[:], bias_all[:])
            nc.sync.dma_start(xt_t[:], xT[:])
            t = wqp.tile([128, KC, 2, 512], F8, tag="wq", name="wqt")
            nc.scalar.dma_start(t[:, :, :, 0:256], wq[:, :, :, 0:256])
            nc.scalar.dma_start(t[:, :, :, 256:512], wq[:, :, :, 256:512])
            nc.scalar.dma_start(id_sb[:], ident[:])
            wq_blk = [(t, 0)]
            nc.sync.dma_start(wv_t[:], wv[:])
            nc.sync.dma_start(et_t[:, :, :, 512:S], eT[:, :, :, 512:S])
            if use_mask:
                mask_sb = []
                for sc in range(SC):
                    mt = mkp.tile([128, TOK], BF, tag="mk", name="mk")
                    nc.sync.dma_start(mt[:], maskT[sc * 128:(sc + 1) * 128, :])
                    mask_sb.append(mt)

            def k_proj(sh, ms):
                pss = [psKV.tile([128, 512], FP, tag="psKV", name="psKV")
                       for _ in ms]
                for c in range(KC):
                    for i, m in enumerate(ms):
                        nc.tensor.matmul(
                            pss[i][:],
                            wk_t[:, c, :, m * 128:(m + 1) * 128],
                            et_t[:, c, :, sh * 512:(sh + 1) * 512],
                            start=(c == 0),
                            stop=(c == KC - 1),
                            perf_mode=DR,
                        )
                with nc.allow_low_precision(reason="k cast to fp8"):
                    for i, m in enumerate(ms):
                        for hi in range(2):
                            dst = kT_sb[2 * m + hi][:, 0,
                                                    sh * 512:(sh + 1) * 512]
                            if triv_bias:
                                nc.vector.tensor_copy(
                                    dst, pss[i][hi * 64:hi * 64 + 64, :])
                            else:
                                nc.vector.tensor_scalar_add(
                                    dst,
                                    pss[i][hi * 64:hi * 64 + 64, :],
                                    bk_sb[hi * 64:hi * 64 + 64, m:m + 1],
                                )

            def v_proj(scs):
                pss = [psKV.tile([128, 512], FP, tag="psKV", name="psKV")
                       for _ in scs]
                for c in range(KC):
                    for i, sc in enumerate(scs):
                        nc.tensor.matmul(
                            pss[i][:],
                            et_t[:, c, :, sc * 128:(sc + 1) * 128],
                            wv_t[:, c],
                            start=(c == 0),
                            stop=(c == KC - 1),
                            perf_mode=DR,
                        )
                with nc.allow_low_precision(reason="v cast to fp8"):
                    for i, sc in enumerate(scs):
                        if triv_bias:
                            nc.vector.tensor_copy(
                                vv_sb[sc // 4][:, sc % 4, :, 0:64],
                                pss[i].rearrange("p (h d) -> p h d", d=HD),
                            )
                        else:
                            nc.vector.tensor_add(
                                vv_sb[sc // 4][:, sc % 4, :, 0:64],
                                pss[i].rearrange("p (h d) -> p h d", d=HD),
                                bv_sb.rearrange("p (h d) -> p h d", d=HD),
                            )

            # interleave s-lo scores with the projections so ACT (the pole)
            # starts as soon as kT h0/h1 and qT 0-3 exist
            stash = {}
            k_proj(0, (0, 1))
            q_proj(0)
            q_proj(1)
            stash[(0, 0)] = scores_quad(0, 0)
            stash[(0, 1)] = scores_quad(0, 1)
            k_proj(0, (2, 3))
            stash[(1, 0)] = scores_quad(1, 0)
            stash[(1, 1)] = scores_quad(1, 1)
            v_proj((0, 1))
            q_proj(2)
            q_proj(3)
            stash[(2, 0)] = scores_quad(2, 0)
            stash[(2, 1)] = scores_quad(2, 1)
            v_proj((2, 3))
            stash[(3, 0)] = scores_quad(3, 0)
            stash[(3, 1)] = scores_quad(3, 1)
            k_proj(1, (0, 1))
            stash[(4, 0)] = scores_quad(4, 0)
            stash[(4, 1)] = scores_quad(4, 1)
            stash[(5, 0)] = scores_quad(5, 0)
            stash[(5, 1)] = scores_quad(5, 1)
            k_proj(1, (2, 3))
            v_proj((4, 5))
            v_proj((6, 7))

        # psKV released; open attention output pool in its banks
        psO = _stk.enter_context(tc.tile_pool(name="psO", bufs=2, space="PSUM"))

        wo_blks = []
        xres_sb = []

        # ---- Phase B: per-head scores+exp / lagged attn@V ----------------
        pending = []
        for hh in range(NH):
            if hh % 2 == 0 and 2 <= hh <= 24:
                m = 4 + (hh - 2) // 2
                if m % 4 == 0:  # new 512-col Wq block
                    t = wqp.tile([128, KC, 2, 512], F8, tag="wq", name="wqt")
                    nc.scalar.dma_start(
                        t[:], wq[:, :, :, (m // 4) * 512:(m // 4 + 1) * 512])
                    wq_blk[0] = (t, m)
                q_proj(m)
            if hh in (24, 28):  # prefetch Wo halves
                half = (hh - 24) // 4
                t = wop.tile([128, KC, 2, 1024], F8, tag="wo", name="wot")
                nc.scalar.dma_start(
                    t[:], wo[:, :, :, half * 1024:(half + 1) * 1024])
                wo_blks.append(t)
            if hh in (25, 27, 29, 31):  # prefetch residual rows
                tt = (hh - 25) // 2
                xt4 = xrp.tile([128, H], BF, tag="xr", name="xr")
                nc.sync.dma_start(xt4[:], xres[tt * 128:(tt + 1) * 128, :])
                xres_sb.append(xt4)
            exs = []
            for qd in range(4):
                if qd < 2 and (hh, qd) in stash:
                    exs.append(stash.pop((hh, qd)))
                else:
                    exs.append(scores_quad(hh, qd))
            pending.append((hh, exs))
            if len(pending) > 1:
                attn_v(*pending.pop(0))
        for p in pending:
            attn_v(*p)

        _stk.close()

        # ---- Phase C: output projection + residual + layernorm -----------
        with (
            tc.tile_pool(name="psC", bufs=8, space="PSUM") as psC,
            tc.tile_pool(name="outp", bufs=2) as outp,
        ):
            if not triv_ln:
                nc.sync.dma_start(gam_sb[:], gamr[:])
                nc.sync.dma_start(bet_sb[:], betr[:])

            for tt in range(4):
                ob = outp.tile([128, H], FP, tag="ob", name="ob")
                stats = lnp.tile([128, 4, 6], FP, tag="st", name="st")
                pss = []
                for nb in range(4):
                    ps = psC.tile([128, 512], FP, tag="psC", name="psC")
                    pss.append(ps)
                    for c in range(KC):
                        nc.tensor.matmul(
                            ps[:],
                            ctx_sb[:, 2 * c:2 * c + 2, tt * 128:(tt + 1) * 128],
                            wo_blks[nb // 2][:, c, :,
                                             (nb % 2) * 512:(nb % 2 + 1) * 512],
                            start=(c == 0),
                            stop=False,
                            perf_mode=DR,
                        )
                    sl = slice(nb * 512, (nb + 1) * 512)
                    # residual add on the PE: += I.T @ xres (bf16)
                    nc.tensor.matmul(
                        ps[:],
                        id_sb[:],
                        xres_sb[tt][:, sl],
                        start=False,
                        stop=True,
                    )
                    nc.vector.bn_stats(stats[:, nb, :], ps[:])
                mv = lnp.tile([128, 2], FP, tag="mv", name="mv")
                nc.vector.bn_aggr(mv[:], stats[:])
                std = lnp.tile([128, 1], FP, tag="sd", name="sd")
                nc.scalar.activation(
                    std[:], mv[:, 1:2], func=Sqrt, bias=eps_sb[:], scale=1.0
                )
                rstd = lnp.tile([128, 1], FP, tag="rs", name="rs")
                nc.vector.reciprocal(rstd[:], std[:])
                nmr = lnp.tile([128, 1], FP, tag="nm", name="nm")
                nc.vector.scalar_tensor_tensor(
                    nmr[:], mv[:, 0:1], -1.0, rstd[:],
                    op0=mybir.AluOpType.mult, op1=mybir.AluOpType.mult,
                )
                for nb in range(4):
                    sl = slice(nb * 512, (nb + 1) * 512)
                    # (ps - mu) * rstd, alternating DVE / ACT (= ps*rstd
                    # + (-mu*rstd) via Identity) -- Pool cannot read PSUM
                    if nb % 2 == 0:
                        nc.vector.tensor_scalar(
                            ob[:, sl],
                            pss[nb][:],
                            scalar1=mv[:, 0:1],
                            scalar2=rstd[:],
                            op0=mybir.AluOpType.subtract,
                            op1=mybir.AluOpType.mult,
                        )
                    else:
                        nc.scalar.activation(
                            ob[:, sl], pss[nb][:], func=Ident,
                            bias=nmr[:], scale=rstd[:],
                        )
                    if not triv_ln:
                        eng = nc.vector if nb % 2 == 0 else nc.gpsimd
                        eng.tensor_mul(ob[:, sl], ob[:, sl], gam_sb[:, sl])
                        eng.tensor_add(ob[:, sl], ob[:, sl], bet_sb[:, sl])
                    nc.sync.dma_start(out[tt * 128:(tt + 1) * 128, sl],
                                      ob[:, sl])

    nc.compile()
    return nc


def _get_nc(use_mask: bool, triv_ln: bool | None = None,
            triv_bias: bool | None = None):
    if triv_ln is None:
        triv_ln = _LAST_TRIV[0]
    if triv_bias is None:
        triv_bias = _LAST_TRIV[1]
    key = (use_mask, triv_ln, triv_bias)
    if key not in _CACHE:
        _CACHE[key] = _build(use_mask, triv_ln, triv_bias)
    return _CACHE[key]


_LAST_TRIV = [True, True]


def _pack_w(w, cols):
    """[2048, cols] fp32 -> [128, 8, 2, cols] fp8 with K-pair layout."""
    return np.ascontiguousarray(
        (w * WS).reshape(KC, 2, 128, cols).transpose(2, 0, 1, 3)
    ).astype(FP8)


def _pack_act(xT, cols):
    """[2048, cols] fp32 (feature-major) -> [128, 8, 2, cols] fp8."""
    return np.ascontiguousarray(
        xT.reshape(KC, 2, 128, cols).transpose(2, 0, 1, 3)
    ).astype(FP8)


def kernel(
    hidden_state,
    encoder_hidden_state,
    encoder_attention_mask,
    Wq, bq, Wk, bk, Wv, bv, Wo, bo, gamma, beta,
):
    hidden_state = np.asarray(hidden_state, dtype=np.float32)
    encoder_hidden_state = np.asarray(encoder_hidden_state, dtype=np.float32)
    encoder_attention_mask = np.asarray(encoder_attention_mask, dtype=np.float32)
    Wq = np.asarray(Wq, dtype=np.float32)
    bq = np.asarray(bq, dtype=np.float32)
    Wk = np.asarray(Wk, dtype=np.float32)
    bk = np.asarray(bk, dtype=np.float32)
    Wv = np.asarray(Wv, dtype=np.float32)
    bv = np.asarray(bv, dtype=np.float32)
    Wo = np.asarray(Wo, dtype=np.float32)
    bo = np.asarray(bo, dtype=np.float32)
    gamma = np.asarray(gamma, dtype=np.float32)
    beta = np.asarray(beta, dtype=np.float32)

    use_mask = bool(np.any(encoder_attention_mask))
    triv_ln = bool(np.all(gamma == 1.0) and np.all(beta == 0.0))
    triv_bias = not (np.any(bq) or np.any(bk) or np.any(bv))
    _LAST_TRIV[0] = triv_ln
    _LAST_TRIV[1] = triv_bias
    nc = _get_nc(use_mask, triv_ln, triv_bias)
    in_maps = _prepare_in_maps(
        hidden_state, encoder_hidden_state, encoder_attention_mask,
        Wq, bq, Wk, bk, Wv, bv, Wo, bo, gamma, beta, use_mask,
    )

    res = run_bass_kernel_spmd(nc, in_maps, core_ids=list(range(8)))
    kernel._last_results = res

    output = np.empty((B, L, H), dtype=np.float32)
    for c in range(8):
        b, lh = c // 2, c % 2
        output[b, lh * TOK:(lh + 1) * TOK, :] = res.results[c]["out"]
    return output


def _prepare_in_maps(
    hidden_state, encoder_hidden_state, encoder_attention_mask,
    Wq, bq, Wk, bk, Wv, bv, Wo, bo, gamma, beta, use_mask,
):
    triv_ln = bool(np.all(gamma == 1.0) and np.all(beta == 0.0))
    triv_bias = not (np.any(bq) or np.any(bk) or np.any(bv))
    wq_p = _pack_w(Wq, H)
    wk_p = _pack_w(Wk, KVH * HD)
    wv_p = _pack_w(Wv, KVH * HD)
    wo_p = _pack_w(Wo, H)
    if not triv_bias:
        bias_all = np.ascontiguousarray(np.concatenate(
            [
                bq.reshape(16, 128).T * WS,
                bk.reshape(4, 128).T * WS,
                np.tile(bv[None, :], (128, 1)) * WS,
            ],
            axis=1,
        ).astype(np.float32))
    ident = np.ascontiguousarray(np.eye(128, dtype=np.float32).astype(BF16))
    if not triv_ln:
        gamr = np.ascontiguousarray(
            np.tile(gamma[None, :].astype(BF16), (128, 1)))
        betr = np.ascontiguousarray(
            np.tile(beta[None, :].astype(BF16), (128, 1)))

    eT_by_b = [
        _pack_act(encoder_hidden_state[b].T, S) for b in range(B)
    ]

    in_maps = []
    for c in range(8):
        b, lh = c // 2, c % 2
        rows = hidden_state[b, lh * TOK:(lh + 1) * TOK, :]
        m = {
            "xT": _pack_act(rows.T, TOK),
            "xres": ((rows + bo[None, :]) * (WS * WS)).astype(BF16),
            "eT": eT_by_b[b],
            "wq": wq_p, "wk": wk_p, "wv": wv_p, "wo": wo_p,
            "ident": ident,
        }
        if not triv_bias:
            m["bias_all"] = bias_all
        if not triv_ln:
            m["gamr"] = gamr
            m["betr"] = betr
        if use_mask:
            mslice = encoder_attention_mask[b, 0, lh * TOK:(lh + 1) * TOK, :]
            m["maskT"] = np.ascontiguousarray(
                (mslice.T * (8.0 * WS * WS)).astype(BF16))
        in_maps.append(m)
    return in_maps


# revision 56
# speedup vs baseline: 1.6637x; 1.0008x over previous
"""Encoder-decoder GQA attention block (B=4, L=S=1024, H=2048, 32 Q heads,
8 KV heads, head_dim 64) + output projection + residual + layernorm, on 8
Trainium2 NeuronCores.

Sharding: rows. Core c handles batch c//2, L-half c%2 (512 query rows).
K/V projections are computed per-batch (duplicated on the 2 cores sharing a
batch), attention over all 32 heads for the core's rows, output projection,
residual + LN. No collectives.

v2: all large matmuls run fp8(e4m3) with perf_mode=DoubleRow (0.5 cyc/row,
two 128-deep K slices per instruction -> 4x the bf16 MAC rate). Weights are
pre-scaled x32 on the host so their ~N(0, 0.02) entries land in e4m3's
normal range; activations stay unscaled (~N(0,1)). Scale bookkeeping:
  q,k carry x32 each -> scores PSUM x1024 -> exp scale 2^-13 (=0.125/1024)
  exp is shifted by bias=-2 (values ~e^[-9,3]) to sit in e4m3 range; the
  V "ones column" is 1/32 so rowsum carries x(1/32) while ctx carries v's
  x32 -> ctx = 32*ctx_true, exactly what fp8 O-proj wants.
  O-proj PSUM = 1024*(ctx@Wo); residual is pre-scaled x1024 on the host and
  layernorm is scale-invariant (eps shift negligible), so no unscale op.
Scores keep K=64 contraction: DoubleRow's second K-slice is a zero pad in
kT (j=1 memset once per buffer), so the pair contributes k.T@q + 0*junk.
attn@V pairs s-chunks: exp tiles are [128s, 2sc, 512t], V is [128s, 4sc,
8h, 72] (72 = 64+1 rowsum col + 7 pad so the DoubleRow pair stride is
16B-aligned).

Engine budget (cost model): ACT exp is the pole (~133us: 128 ops x
[128,2,512]); PE ~90us total (fp8-DR everywhere, bf16 only for the K=1
recip broadcast); DVE ~90us (PSUM->SBUF casts, recip, ctx mul, LN stats);
Pool takes the zero-pad memsets, residual adds and the LN normalize mul;
DMA ~60us serialized. Schedule: scores for heads 0-7 (s-lo) interleave
with the K/V projections in phase A so ACT starts ~12us in; attn@V lags
one head behind exp; Wq streams in 512-col blocks, Wo prefetches late in
phase B; phase C is dense PE (O-proj) with per-row-block LN + store.

LayerNorm gamma/beta ops and their DMAs are skipped when gamma==1 and
beta==0 (detected at runtime; separate build variant), same for the mask.
"""

from contextlib import ExitStack

import numpy as np
import ml_dtypes

import concourse.bass as bass  # noqa: F401  (bass.AP used via handles)
import concourse.mybir as mybir
import concourse.tile as tile
from concourse import bacc
from concourse.bass_utils import run_bass_kernel_spmd

BF16 = ml_dtypes.bfloat16
FP8 = ml_dtypes.float8_e4m3fn

H = 2048
NH = 32
KVH = 8
G = 4           # query-head groups per kv head
HD = 64
B, L, S = 4, 1024, 1024
TOK = 512       # decoder rows per core
KC = 8          # contraction chunk-pairs (8 x (2x128) = 2048)
SC = S // 128   # 8 s chunks
EPS = 1e-6
WS = 32.0       # host-side fp8 weight scale

FP = mybir.dt.float32
BF = mybir.dt.bfloat16
F8 = mybir.dt.float8e4
DR = mybir.MatmulPerfMode.DoubleRow

_CACHE: dict = {}


def _build(use_mask: bool, triv_ln: bool, triv_bias: bool):
    nc = bacc.Bacc("TRN2", target_bir_lowering=False)

    xT = nc.dram_tensor("xT", [128, KC, 2, TOK], F8, kind="ExternalInput")
    xres = nc.dram_tensor("xres", [TOK, H], BF, kind="ExternalInput")
    eT = nc.dram_tensor("eT", [128, KC, 2, S], F8, kind="ExternalInput")
    wq = nc.dram_tensor("wq", [128, KC, 2, H], F8, kind="ExternalInput")
    wk = nc.dram_tensor("wk", [128, KC, 2, 512], F8, kind="ExternalInput")
    wv = nc.dram_tensor("wv", [128, KC, 2, 512], F8, kind="ExternalInput")
    wo = nc.dram_tensor("wo", [128, KC, 2, H], F8, kind="ExternalInput")
    # packed biases: [bq(16) | bk(4) | bv(512)] -> one DMA
    if not triv_bias:
        bias_all = nc.dram_tensor("bias_all", [128, 532], FP,
                                  kind="ExternalInput")
    ident = nc.dram_tensor("ident", [128, 128], BF, kind="ExternalInput")
    if not triv_ln:
        gamr = nc.dram_tensor("gamr", [128, H], BF, kind="ExternalInput")
        betr = nc.dram_tensor("betr", [128, H], BF, kind="ExternalInput")
    if use_mask:
        maskT = nc.dram_tensor("maskT", [S, TOK], BF, kind="ExternalInput")
    out = nc.dram_tensor("out", [TOK, H], FP, kind="ExternalOutput")

    Exp = mybir.ActivationFunctionType.Exp
    Ident = mybir.ActivationFunctionType.Identity
    Sqrt = mybir.ActivationFunctionType.Sqrt

    with tile.TileContext(nc) as tc:
      with (
          tc.tile_pool(name="ctxT", bufs=1) as ctxp,
          tc.tile_pool(name="cc", bufs=1) as ccp,
          tc.tile_pool(name="ln", bufs=10) as lnp,
          tc.tile_pool(name="qT", bufs=NH) as qtp,
          tc.tile_pool(name="kT", bufs=KVH) as ktp,
          tc.tile_pool(name="vv", bufs=2) as vvp,
          tc.tile_pool(name="expp", bufs=20) as expp,
          tc.tile_pool(name="rec", bufs=6) as recp,
          tc.tile_pool(name="bc", bufs=6) as bcp,
          tc.tile_pool(name="const", bufs=1) as constp,
          tc.tile_pool(name="wq", bufs=2) as wqp,
          tc.tile_pool(name="xTp", bufs=1) as xtp,
          tc.tile_pool(name="wo", bufs=2) as wop,
          tc.tile_pool(name="xr", bufs=4) as xrp,
          tc.tile_pool(name="maskp", bufs=SC if use_mask else 1) as mkp,
      ):
        eps_sb = ccp.tile([128, 1], FP, name="eps_sb")
        nb2_sb = ccp.tile([128, 1], FP, name="nb2_sb")
        junk_sb = ccp.tile([128, 1], FP, name="junk_sb")
        nc.vector.memset(eps_sb[:], EPS)
        nc.vector.memset(nb2_sb[:], -2.0)
        # touch the Sqrt act table now so its load isn't on the phase-C chain
        nc.scalar.activation(junk_sb[:], eps_sb[:], func=Sqrt,
                             bias=eps_sb[:], scale=1.0)
        if not triv_ln:
            gam_sb = ccp.tile([128, H], BF, name="gam_sb")
            bet_sb = ccp.tile([128, H], BF, name="bet_sb")

        ctx_sb = ctxp.tile([128, 16, TOK], F8, name="ctx")
        # qt/kt hold the DoubleRow K-pair on dim1; slice [:,1,:] is a zero
        # pad (memset once per buffer below) so K=64 contractions are legal.
        qT_sb = [qtp.tile([64, 2, TOK], F8, name="qt") for _ in range(NH)]
        kT_sb = [ktp.tile([64, 2, S], F8, name="kt") for _ in range(KVH)]
        # V: [s-part, sc-in-quad, kv-head, 64+1(rowsum)+7(pad to 16B)]
        vv_sb = [vvp.tile([128, 4, KVH, 72], F8, name="vv") for _ in range(2)]

        # zero the DoubleRow pads: early-needed ones on DVE (idle until
        # the first PSUM copies ~17us), the rest on Pool
        for t in kT_sb:
            nc.vector.memset(t[:, 1, :], 0.0)
        for t in qT_sb[:8]:
            nc.vector.memset(t[:, 1, :], 0.0)
        for t in vv_sb:
            nc.gpsimd.memset(t[:, :, :, 64:65], 1.0)
        for t in qT_sb[8:]:
            nc.gpsimd.memset(t[:, 1, :], 0.0)

        _stk = ExitStack()
        psA = _stk.enter_context(tc.tile_pool(name="psA", bufs=2, space="PSUM"))
        psS = _stk.enter_context(tc.tile_pool(name="psS", bufs=2, space="PSUM"))

        id_sb = constp.tile([128, 128], BF, name="id_sb")
        if not triv_bias:
            bias_sb = constp.tile([128, 532], FP, name="bias_sb")
            bq_sb = bias_sb[:, 0:16]
            bk_sb = bias_sb[:, 16:20]
            bv_sb = bias_sb[:, 20:532]

        def q_proj(m):
            blk, base = wq_blk[0]
            q = m - base
            assert 0 <= q < 4, (m, base)
            ps = psA.tile([128, TOK], FP, tag="psA", name="psA")
            for c in range(KC):
                nc.tensor.matmul(
                    ps[:],
                    blk[:, c, :, q * 128:(q + 1) * 128],
                    xt_t[:, c],
                    start=(c == 0),
                    stop=(c == KC - 1),
                    perf_mode=DR,
                )
            with nc.allow_low_precision(reason="q cast to fp8 for scores"):
                for hi in range(2):
                    if triv_bias:
                        nc.vector.tensor_copy(
                            qT_sb[2 * m + hi][:, 0, :],
                            ps[hi * 64:hi * 64 + 64, :],
                        )
                    else:
                        nc.vector.tensor_scalar_add(
                            qT_sb[2 * m + hi][:, 0, :],
                            ps[hi * 64:hi * 64 + 64, :],
                            bq_sb[hi * 64:hi * 64 + 64, m:m + 1],
                        )

        def scores_quad(hh, qd):
            h = hh // G
            ps = psS.tile([128, 2, TOK], FP, tag="psS", name="psS")
            for i in range(2):
                sc = 2 * qd + i
                nc.tensor.matmul(
                    ps[:, i, :],
                    kT_sb[h][:, :, sc * 128:(sc + 1) * 128],
                    qT_sb[hh][:],
                    start=True,
                    stop=True,
                    perf_mode=DR,
                )
                if use_mask:
                    nc.vector.tensor_add(ps[:, i, :], ps[:, i, :],
                                         mask_sb[sc][:])
            ex = expp.tile([128, 2, TOK], F8, tag="ex", name="ex")
            nc.scalar.activation(ex[:], ps[:], func=Exp,
                                 scale=1.0 / 8192.0, bias=nb2_sb[:])
            return ex

        def attn_v(hh, exs):
            h = hh // G
            po = psO.tile([128, TOK], FP, tag="psO", name="psO")
            for qd in range(4):
                vq = vv_sb[qd // 2][:, (qd % 2) * 2:(qd % 2) * 2 + 2, h, 0:65]
                nc.tensor.matmul(
                    po[0:65, :],
                    vq,
                    exs[qd][:],
                    start=(qd == 0),
                    stop=(qd == 3),
                    perf_mode=DR,
                )
            recb = recp.tile([1, TOK], BF, tag="recb", name="recb")
            with nc.allow_low_precision(reason="softmax recip rounds to bf16"):
                nc.vector.reciprocal(recb[:], po[64:65, :])
            # broadcast recip across 64 partitions on the idle Pool engine
            rb = bcp.tile([64, TOK], BF, tag="rb", name="rb")
            nc.gpsimd.partition_broadcast(rb[:], recb[:])
            with nc.allow_low_precision(reason="ctx cast to fp8 for O-proj"):
                nc.vector.tensor_mul(
                    ctx_sb[(hh % 2) * 64:(hh % 2) * 64 + 64, hh // 2, :],
                    po[0:64, :],
                    rb[:],
                )

        # ---- Phase A: input DMAs, K/V/Q projections, s-lo scores ---------
        with (
            tc.tile_pool(name="eTp", bufs=1) as etp,
            tc.tile_pool(name="wk", bufs=1) as wkp,
            tc.tile_pool(name="wv", bufs=1) as wvp,
            tc.tile_pool(name="psKV", bufs=2, space="PSUM") as psKV,
        ):
            wk_t = wkp.tile([128, KC, 2, 512], F8, name="wkt")
            et_t = etp.tile([128, KC, 2, S], F8, name="et")
            wv_t = wvp.tile([128, KC, 2, 512], F8, name="wvt")
            xt_t = xtp.tile([128, KC, 2, TOK], F8, name="xt")
            # few big DMAs: descriptor-gen (HWDGE) is a serial shared
            # resource (~0.63us per DMA instruction), so chunking loses
            # PE p-state warm-up during the initial DMA wait: dummy
            # matmuls on the already-zeroed kT pads keep pe_busy_start
            # anchored early so the first real chains run at full clock
            wps = psA.tile([128, TOK], FP, tag="psA", name="psA")
            for _ in range(40):
                nc.tensor.matmul(
                    wps[:, 0:512], kT_sb[0][:, 1, 0:128], kT_sb[0][:, 1, 0:512],
                    start=True, stop=True,
                )
            # critical-path loads on SP; decoder-side on ACT queue;
            # late-needed on DVE queue (DMA engine is shared+serial, queue
            # split biases acquisition order toward the critical path)
            nc.sync.dma_start(wk_t[:], wk[:])
            nc.sync.dma_start(et_t[:, :, :, 0:512], eT[:, :, :, 0:512])
            if not triv_bias:
                nc.sync.dma_start(bias_sb[:], bias_all[:])
            nc.sync.dma_start(xt_t[:], xT[:])
            t = wqp.tile([128, KC, 2, 512], F8, tag="wq", name="wqt")
            nc.scalar.dma_start(t[:, :, :, 0:256], wq[:, :, :, 0:256])
            nc.scalar.dma_start(t[:, :, :, 256:512], wq[:, :, :, 256:512])
            nc.scalar.dma_start(id_sb[:], ident[:])
            wq_blk = [(t, 0)]
            nc.sync.dma_start(wv_t[:], wv[:])
            nc.sync.dma_start(et_t[:, :, :, 512:S], eT[:, :, :, 512:S])
            if use_mask:
                mask_sb = []
                for sc in range(SC):
                    mt = mkp.tile([128, TOK], BF, tag="mk", name="mk")
                    nc.sync.dma_start(mt[:], maskT[sc * 128:(sc + 1) * 128, :])
                    mask_sb.append(mt)

            def k_proj(sh, ms):
                pss = [psKV.tile([128, 512], FP, tag="psKV", name="psKV")
                       for _ in ms]
                for c in range(KC):
                    for i, m in enumerate(ms):
                        nc.tensor.matmul(
                            pss[i][:],
                            wk_t[:, c, :, m * 128:(m + 1) * 128],
                            et_t[:, c, :, sh * 512:(sh + 1) * 512],
                            start=(c == 0),
                            stop=(c == KC - 1),
                            perf_mode=DR,
                        )
                with nc.allow_low_precision(reason="k cast to fp8"):
                    for i, m in enumerate(ms):
                        for hi in range(2):
                            dst = kT_sb[2 * m + hi][:, 0,
                                                    sh * 512:(sh + 1) * 512]
                            if triv_bias:
                                nc.vector.tensor_copy(
                                    dst, pss[i][hi * 64:hi * 64 + 64, :])
                            else:
                                nc.vector.tensor_scalar_add(
                                    dst,
                                    pss[i][hi * 64:hi * 64 + 64, :],
                                    bk_sb[hi * 64:hi * 64 + 64, m:m + 1],
                                )

            def v_proj(scs):
                pss = [psKV.tile([128, 512], FP, tag="psKV", name="psKV")
                       for _ in scs]
                for c in range(KC):
                    for i, sc in enumerate(scs):
                        nc.tensor.matmul(
                            pss[i][:],
                            et_t[:, c, :, sc * 128:(sc + 1) * 128],
                            wv_t[:, c],
                            start=(c == 0),
                            stop=(c == KC - 1),
                            perf_mode=DR,
                        )
                with nc.allow_low_precision(reason="v cast to fp8"):
                    for i, sc in enumerate(scs):
                        if triv_bias:
                            nc.vector.tensor_copy(
                                vv_sb[sc // 4][:, sc % 4, :, 0:64],
                                pss[i].rearrange("p (h d) -> p h d", d=HD),
                            )
                        else:
                            nc.vector.tensor_add(
                                vv_sb[sc // 4][:, sc % 4, :, 0:64],
                                pss[i].rearrange("p (h d) -> p h d", d=HD),
                                bv_sb.rearrange("p (h d) -> p h d", d=HD),
                            )

            # interleave s-lo scores with the projections so ACT (the pole)
            # starts as soon as kT h0/h1 and qT 0-3 exist
            stash = {}
            k_proj(0, (0, 1))
            q_proj(0)
            q_proj(1)
            stash[(0, 0)] = scores_quad(0, 0)
            stash[(0, 1)] = scores_quad(0, 1)
            k_proj(0, (2, 3))
            stash[(1, 0)] = scores_quad(1, 0)
            stash[(1, 1)] = scores_quad(1, 1)
            v_proj((0, 1))
            q_proj(2)
            q_proj(3)
            stash[(2, 0)] = scores_quad(2, 0)
            stash[(2, 1)] = scores_quad(2, 1)
            v_proj((2, 3))
            stash[(3, 0)] = scores_quad(3, 0)
            stash[(3, 1)] = scores_quad(3, 1)
            k_proj(1, (0, 1))
            stash[(4, 0)] = scores_quad(4, 0)
            stash[(4, 1)] = scores_quad(4, 1)
            stash[(5, 0)] = scores_quad(5, 0)
            stash[(5, 1)] = scores_quad(5, 1)
            stash[(6, 0)] = scores_quad(6, 0)
            stash[(6, 1)] = scores_quad(6, 1)
            stash[(7, 0)] = scores_quad(7, 0)
            stash[(7, 1)] = scores_quad(7, 1)
            k_proj(1, (2, 3))
            v_proj((4, 5))
            v_proj((6, 7))

        # psKV released; open attention output pool in its banks
        psO = _stk.enter_context(tc.tile_pool(name="psO", bufs=2, space="PSUM"))

        wo_blks = []
        xres_sb = []

        # ---- Phase B: per-head scores+exp / lagged attn@V ----------------
        pending = []
        for hh in range(NH):
            if hh % 2 == 0 and 2 <= hh <= 24:
                m = 4 + (hh - 2) // 2
                if m % 4 == 0:  # new 512-col Wq block
                    t = wqp.tile([128, KC, 2, 512], F8, tag="wq", name="wqt")
                    nc.scalar.dma_start(
                        t[:], wq[:, :, :, (m // 4) * 512:(m // 4 + 1) * 512])
                    wq_blk[0] = (t, m)
                q_proj(m)
            if hh in (24, 28):  # prefetch Wo halves
                half = (hh - 24) // 4
                t = wop.tile([128, KC, 2, 1024], F8, tag="wo", name="wot")
                nc.scalar.dma_start(
                    t[:], wo[:, :, :, half * 1024:(half + 1) * 1024])
                wo_blks.append(t)
            if hh in (25, 27, 29, 31):  # prefetch residual rows
                tt = (hh - 25) // 2
                xt4 = xrp.tile([128, H], BF, tag="xr", name="xr")
                nc.sync.dma_start(xt4[:], xres[tt * 128:(tt + 1) * 128, :])
                xres_sb.append(xt4)
            exs = []
            for qd in range(4):
                if qd < 2 and (hh, qd) in stash:
                    exs.append(stash.pop((hh, qd)))
                else:
                    exs.append(scores_quad(hh, qd))
            pending.append((hh, exs))
            if len(pending) > 1:
                attn_v(*pending.pop(0))
        for p in pending:
            attn_v(*p)

        _stk.close()

        # ---- Phase C: output projection + residual + layernorm -----------
        with (
            tc.tile_pool(name="psC", bufs=8, space="PSUM") as psC,
            tc.tile_pool(name="outp", bufs=2) as outp,
        ):
            if not triv_ln:
                nc.sync.dma_start(gam_sb[:], gamr[:])
                nc.sync.dma_start(bet_sb[:], betr[:])

            for tt in range(4):
                ob = outp.tile([128, H], FP, tag="ob", name="ob")
                stats = lnp.tile([128, 4, 6], FP, tag="st", name="st")
                pss = []
                for nb in range(4):
                    ps = psC.tile([128, 512], FP, tag="psC", name="psC")
                    pss.append(ps)
                    for c in range(KC):
                        nc.tensor.matmul(
                            ps[:],
                            ctx_sb[:, 2 * c:2 * c + 2, tt * 128:(tt + 1) * 128],
                            wo_blks[nb // 2][:, c, :,
                                             (nb % 2) * 512:(nb % 2 + 1) * 512],
                            start=(c == 0),
                            stop=False,
                            perf_mode=DR,
                        )
                    sl = slice(nb * 512, (nb + 1) * 512)
                    # residual add on the PE: += I.T @ xres (bf16)
                    nc.tensor.matmul(
                        ps[:],
                        id_sb[:],
                        xres_sb[tt][:, sl],
                        start=False,
                        stop=True,
                    )
                    nc.vector.bn_stats(stats[:, nb, :], ps[:])
                mv = lnp.tile([128, 2], FP, tag="mv", name="mv")
                nc.vector.bn_aggr(mv[:], stats[:])
                std = lnp.tile([128, 1], FP, tag="sd", name="sd")
                nc.scalar.activation(
                    std[:], mv[:, 1:2], func=Sqrt, bias=eps_sb[:], scale=1.0
                )
                rstd = lnp.tile([128, 1], FP, tag="rs", name="rs")
                nc.vector.reciprocal(rstd[:], std[:])
                nmr = lnp.tile([128, 1], FP, tag="nm", name="nm")
                nc.vector.scalar_tensor_tensor(
                    nmr[:], mv[:, 0:1], -1.0, rstd[:],
                    op0=mybir.AluOpType.mult, op1=mybir.AluOpType.mult,
                )
                for nb in range(4):
                    sl = slice(nb * 512, (nb + 1) * 512)
                    # (ps - mu) * rstd, alternating DVE / ACT (= ps*rstd
                    # + (-mu*rstd) via Identity) -- Pool cannot read PSUM
                    if nb % 2 == 0:
                        nc.vector.tensor_scalar(
                            ob[:, sl],
                            pss[nb][:],
                            scalar1=mv[:, 0:1],
                            scalar2=rstd[:],
                            op0=mybir.AluOpType.subtract,
                            op1=mybir.AluOpType.mult,
                        )
                    else:
                        nc.scalar.activation(
                            ob[:, sl], pss[nb][:], func=Ident,
                            bias=nmr[:], scale=rstd[:],
                        )
                    if not triv_ln:
                        eng = nc.vector if nb % 2 == 0 else nc.gpsimd
                        eng.tensor_mul(ob[:, sl], ob[:, sl], gam_sb[:, sl])
                        eng.tensor_add(ob[:, sl], ob[:, sl], bet_sb[:, sl])
                    nc.sync.dma_start(out[tt * 128:(tt + 1) * 128, sl],
                                      ob[:, sl])

    nc.compile()
    return nc


def _get_nc(use_mask: bool, triv_ln: bool | None = None,
            triv_bias: bool | None = None):
    if triv_ln is None:
        triv_ln = _LAST_TRIV[0]
    if triv_bias is None:
        triv_bias = _LAST_TRIV[1]
    key = (use_mask, triv_ln, triv_bias)
    if key not in _CACHE:
        _CACHE[key] = _build(use_mask, triv_ln, triv_bias)
    return _CACHE[key]


_LAST_TRIV = [True, True]


def _pack_w(w, cols):
    """[2048, cols] fp32 -> [128, 8, 2, cols] fp8 with K-pair layout."""
    return np.ascontiguousarray(
        (w * WS).reshape(KC, 2, 128, cols).transpose(2, 0, 1, 3)
    ).astype(FP8)


def _pack_act(xT, cols):
    """[2048, cols] fp32 (feature-major) -> [128, 8, 2, cols] fp8."""
    return np.ascontiguousarray(
        xT.reshape(KC, 2, 128, cols).transpose(2, 0, 1, 3)
    ).astype(FP8)


def kernel(
    hidden_state,
    encoder_hidden_state,
    encoder_attention_mask,
    Wq, bq, Wk, bk, Wv, bv, Wo, bo, gamma, beta,
):
    hidden_state = np.asarray(hidden_state, dtype=np.float32)
    encoder_hidden_state = np.asarray(encoder_hidden_state, dtype=np.float32)
    encoder_attention_mask = np.asarray(encoder_attention_mask, dtype=np.float32)
    Wq = np.asarray(Wq, dtype=np.float32)
    bq = np.asarray(bq, dtype=np.float32)
    Wk = np.asarray(Wk, dtype=np.float32)
    bk = np.asarray(bk, dtype=np.float32)
    Wv = np.asarray(Wv, dtype=np.float32)
    bv = np.asarray(bv, dtype=np.float32)
    Wo = np.asarray(Wo, dtype=np.float32)
    bo = np.asarray(bo, dtype=np.float32)
    gamma = np.asarray(gamma, dtype=np.float32)
    beta = np.asarray(beta, dtype=np.float32)

    use_mask = bool(np.any(encoder_attention_mask))
    triv_ln = bool(np.all(gamma == 1.0) and np.all(beta == 0.0))
    triv_bias = not (np.any(bq) or np.any(bk) or np.any(bv))
    _LAST_TRIV[0] = triv_ln
    _LAST_TRIV[1] = triv_bias
    nc = _get_nc(use_mask, triv_ln, triv_bias)
    in_maps = _prepare_in_maps(
        hidden_state, encoder_hidden_state, encoder_attention_mask,
        Wq, bq, Wk, bk, Wv, bv, Wo, bo, gamma, beta, use_mask,
    )

    res = run_bass_kernel_spmd(nc, in_maps, core_ids=list(range(8)))
    kernel._last_results = res

    output = np.empty((B, L, H), dtype=np.float32)
    for c in range(8):
        b, lh = c // 2, c % 2
        output[b, lh * TOK:(lh + 1) * TOK, :] = res.results[c]["out"]
    return output


def _prepare_in_maps(
    hidden_state, encoder_hidden_state, encoder_attention_mask,
    Wq, bq, Wk, bk, Wv, bv, Wo, bo, gamma, beta, use_mask,
):
    triv_ln = bool(np.all(gamma == 1.0) and np.all(beta == 0.0))
    triv_bias = not (np.any(bq) or np.any(bk) or np.any(bv))
    wq_p = _pack_w(Wq, H)
    wk_p = _pack_w(Wk, KVH * HD)
    wv_p = _pack_w(Wv, KVH * HD)
    wo_p = _pack_w(Wo, H)
    if not triv_bias:
        bias_all = np.ascontiguousarray(np.concatenate(
            [
                bq.reshape(16, 128).T * WS,
                bk.reshape(4, 128).T * WS,
                np.tile(bv[None, :], (128, 1)) * WS,
            ],
            axis=1,
        ).astype(np.float32))
    ident = np.ascontiguousarray(np.eye(128, dtype=np.float32).astype(BF16))
    if not triv_ln:
        gamr = np.ascontiguousarray(
            np.tile(gamma[None, :].astype(BF16), (128, 1)))
        betr = np.ascontiguousarray(
            np.tile(beta[None, :].astype(BF16), (128, 1)))

    eT_by_b = [
        _pack_act(encoder_hidden_state[b].T, S) for b in range(B)
    ]

    in_maps = []
    for c in range(8):
        b, lh = c // 2, c % 2
        rows = hidden_state[b, lh * TOK:(lh + 1) * TOK, :]
        m = {
            "xT": _pack_act(rows.T, TOK),
            "xres": ((rows + bo[None, :]) * (WS * WS)).astype(BF16),
            "eT": eT_by_b[b],
            "wq": wq_p, "wk": wk_p, "wv": wv_p, "wo": wo_p,
            "ident": ident,
        }
        if not triv_bias:
            m["bias_all"] = bias_all
        if not triv_ln:
            m["gamr"] = gamr
            m["betr"] = betr
        if use_mask:
            mslice = encoder_attention_mask[b, 0, lh * TOK:(lh + 1) * TOK, :]
            m["maskT"] = np.ascontiguousarray(
                (mslice.T * (8.0 * WS * WS)).astype(BF16))
        in_maps.append(m)
    return in_maps


# revision 57
# speedup vs baseline: 1.6676x; 1.0023x over previous
"""Encoder-decoder GQA attention block (B=4, L=S=1024, H=2048, 32 Q heads,
8 KV heads, head_dim 64) + output projection + residual + layernorm, on 8
Trainium2 NeuronCores.

Sharding: rows. Core c handles batch c//2, L-half c%2 (512 query rows).
K/V projections are computed per-batch (duplicated on the 2 cores sharing a
batch), attention over all 32 heads for the core's rows, output projection,
residual + LN. No collectives.

v2: all large matmuls run fp8(e4m3) with perf_mode=DoubleRow (0.5 cyc/row,
two 128-deep K slices per instruction -> 4x the bf16 MAC rate). Weights are
pre-scaled x32 on the host so their ~N(0, 0.02) entries land in e4m3's
normal range; activations stay unscaled (~N(0,1)). Scale bookkeeping:
  q,k carry x32 each -> scores PSUM x1024 -> exp scale 2^-13 (=0.125/1024)
  exp is shifted by bias=-2 (values ~e^[-9,3]) to sit in e4m3 range; the
  V "ones column" is 1/32 so rowsum carries x(1/32) while ctx carries v's
  x32 -> ctx = 32*ctx_true, exactly what fp8 O-proj wants.
  O-proj PSUM = 1024*(ctx@Wo); residual is pre-scaled x1024 on the host and
  layernorm is scale-invariant (eps shift negligible), so no unscale op.
Scores keep K=64 contraction: DoubleRow's second K-slice is a zero pad in
kT (j=1 memset once per buffer), so the pair contributes k.T@q + 0*junk.
attn@V pairs s-chunks: exp tiles are [128s, 2sc, 512t], V is [128s, 4sc,
8h, 72] (72 = 64+1 rowsum col + 7 pad so the DoubleRow pair stride is
16B-aligned).

Engine budget (cost model): ACT exp is the pole (~133us: 128 ops x
[128,2,512]); PE ~90us total (fp8-DR everywhere, bf16 only for the K=1
recip broadcast); DVE ~90us (PSUM->SBUF casts, recip, ctx mul, LN stats);
Pool takes the zero-pad memsets, residual adds and the LN normalize mul;
DMA ~60us serialized. Schedule: scores for heads 0-7 (s-lo) interleave
with the K/V projections in phase A so ACT starts ~12us in; attn@V lags
one head behind exp; Wq streams in 512-col blocks, Wo prefetches late in
phase B; phase C is dense PE (O-proj) with per-row-block LN + store.

LayerNorm gamma/beta ops and their DMAs are skipped when gamma==1 and
beta==0 (detected at runtime; separate build variant), same for the mask.
"""

from contextlib import ExitStack

import numpy as np
import ml_dtypes

import concourse.bass as bass  # noqa: F401  (bass.AP used via handles)
import concourse.mybir as mybir
import concourse.tile as tile
from concourse import bacc
from concourse.bass_utils import run_bass_kernel_spmd

BF16 = ml_dtypes.bfloat16
FP8 = ml_dtypes.float8_e4m3fn

H = 2048
NH = 32
KVH = 8
G = 4           # query-head groups per kv head
HD = 64
B, L, S = 4, 1024, 1024
TOK = 512       # decoder rows per core
KC = 8          # contraction chunk-pairs (8 x (2x128) = 2048)
SC = S // 128   # 8 s chunks
EPS = 1e-6
WS = 32.0       # host-side fp8 weight scale

FP = mybir.dt.float32
BF = mybir.dt.bfloat16
F8 = mybir.dt.float8e4
DR = mybir.MatmulPerfMode.DoubleRow

_CACHE: dict = {}


def _build(use_mask: bool, triv_ln: bool, triv_bias: bool):
    nc = bacc.Bacc("TRN2", target_bir_lowering=False)

    xT = nc.dram_tensor("xT", [128, KC, 2, TOK], F8, kind="ExternalInput")
    xres = nc.dram_tensor("xres", [TOK, H], BF, kind="ExternalInput")
    eT = nc.dram_tensor("eT", [128, KC, 2, S], F8, kind="ExternalInput")
    wq = nc.dram_tensor("wq", [128, KC, 2, H], F8, kind="ExternalInput")
    wk = nc.dram_tensor("wk", [128, KC, 2, 512], F8, kind="ExternalInput")
    wv = nc.dram_tensor("wv", [128, KC, 2, 512], F8, kind="ExternalInput")
    wo = nc.dram_tensor("wo", [128, KC, 2, H], F8, kind="ExternalInput")
    # packed biases: [bq(16) | bk(4) | bv(512)] -> one DMA
    if not triv_bias:
        bias_all = nc.dram_tensor("bias_all", [128, 532], FP,
                                  kind="ExternalInput")
    ident = nc.dram_tensor("ident", [128, 128], BF, kind="ExternalInput")
    if not triv_ln:
        gamr = nc.dram_tensor("gamr", [128, H], BF, kind="ExternalInput")
        betr = nc.dram_tensor("betr", [128, H], BF, kind="ExternalInput")
    if use_mask:
        maskT = nc.dram_tensor("maskT", [S, TOK], BF, kind="ExternalInput")
    out = nc.dram_tensor("out", [TOK, H], BF, kind="ExternalOutput")

    Exp = mybir.ActivationFunctionType.Exp
    Ident = mybir.ActivationFunctionType.Identity
    Sqrt = mybir.ActivationFunctionType.Sqrt

    with tile.TileContext(nc) as tc:
      with (
          tc.tile_pool(name="ctxT", bufs=1) as ctxp,
          tc.tile_pool(name="cc", bufs=1) as ccp,
          tc.tile_pool(name="ln", bufs=10) as lnp,
          tc.tile_pool(name="qT", bufs=NH) as qtp,
          tc.tile_pool(name="kT", bufs=KVH) as ktp,
          tc.tile_pool(name="vv", bufs=2) as vvp,
          tc.tile_pool(name="expp", bufs=20) as expp,
          tc.tile_pool(name="rec", bufs=6) as recp,
          tc.tile_pool(name="bc", bufs=6) as bcp,
          tc.tile_pool(name="const", bufs=1) as constp,
          tc.tile_pool(name="wq", bufs=2) as wqp,
          tc.tile_pool(name="xTp", bufs=1) as xtp,
          tc.tile_pool(name="wo", bufs=2) as wop,
          tc.tile_pool(name="xr", bufs=4) as xrp,
          tc.tile_pool(name="maskp", bufs=SC if use_mask else 1) as mkp,
      ):
        eps_sb = ccp.tile([128, 1], FP, name="eps_sb")
        nb2_sb = ccp.tile([128, 1], FP, name="nb2_sb")
        junk_sb = ccp.tile([128, 1], FP, name="junk_sb")
        nc.vector.memset(eps_sb[:], EPS)
        nc.vector.memset(nb2_sb[:], -2.0)
        # touch the Sqrt act table now so its load isn't on the phase-C chain
        nc.scalar.activation(junk_sb[:], eps_sb[:], func=Sqrt,
                             bias=eps_sb[:], scale=1.0)
        if not triv_ln:
            gam_sb = ccp.tile([128, H], BF, name="gam_sb")
            bet_sb = ccp.tile([128, H], BF, name="bet_sb")

        ctx_sb = ctxp.tile([128, 16, TOK], F8, name="ctx")
        # qt/kt hold the DoubleRow K-pair on dim1; slice [:,1,:] is a zero
        # pad (memset once per buffer below) so K=64 contractions are legal.
        qT_sb = [qtp.tile([64, 2, TOK], F8, name="qt") for _ in range(NH)]
        kT_sb = [ktp.tile([64, 2, S], F8, name="kt") for _ in range(KVH)]
        # V: [s-part, sc-in-quad, kv-head, 64+1(rowsum)+7(pad to 16B)]
        vv_sb = [vvp.tile([128, 4, KVH, 72], F8, name="vv") for _ in range(2)]

        # zero the DoubleRow pads: early-needed ones on DVE (idle until
        # the first PSUM copies ~17us), the rest on Pool
        for t in kT_sb:
            nc.vector.memset(t[:, 1, :], 0.0)
        for t in qT_sb[:8]:
            nc.vector.memset(t[:, 1, :], 0.0)
        for t in vv_sb:
            nc.gpsimd.memset(t[:, :, :, 64:65], 1.0)
        for t in qT_sb[8:]:
            nc.gpsimd.memset(t[:, 1, :], 0.0)

        _stk = ExitStack()
        psA = _stk.enter_context(tc.tile_pool(name="psA", bufs=2, space="PSUM"))
        psS = _stk.enter_context(tc.tile_pool(name="psS", bufs=2, space="PSUM"))

        id_sb = constp.tile([128, 128], BF, name="id_sb")
        if not triv_bias:
            bias_sb = constp.tile([128, 532], FP, name="bias_sb")
            bq_sb = bias_sb[:, 0:16]
            bk_sb = bias_sb[:, 16:20]
            bv_sb = bias_sb[:, 20:532]

        def q_proj(m):
            blk, base = wq_blk[0]
            q = m - base
            assert 0 <= q < 4, (m, base)
            ps = psA.tile([128, TOK], FP, tag="psA", name="psA")
            for c in range(KC):
                nc.tensor.matmul(
                    ps[:],
                    blk[:, c, :, q * 128:(q + 1) * 128],
                    xt_t[:, c],
                    start=(c == 0),
                    stop=(c == KC - 1),
                    perf_mode=DR,
                )
            with nc.allow_low_precision(reason="q cast to fp8 for scores"):
                for hi in range(2):
                    if triv_bias:
                        nc.vector.tensor_copy(
                            qT_sb[2 * m + hi][:, 0, :],
                            ps[hi * 64:hi * 64 + 64, :],
                        )
                    else:
                        nc.vector.tensor_scalar_add(
                            qT_sb[2 * m + hi][:, 0, :],
                            ps[hi * 64:hi * 64 + 64, :],
                            bq_sb[hi * 64:hi * 64 + 64, m:m + 1],
                        )

        def scores_quad(hh, qd):
            h = hh // G
            ps = psS.tile([128, 2, TOK], FP, tag="psS", name="psS")
            for i in range(2):
                sc = 2 * qd + i
                nc.tensor.matmul(
                    ps[:, i, :],
                    kT_sb[h][:, :, sc * 128:(sc + 1) * 128],
                    qT_sb[hh][:],
                    start=True,
                    stop=True,
                    perf_mode=DR,
                )
                if use_mask:
                    nc.vector.tensor_add(ps[:, i, :], ps[:, i, :],
                                         mask_sb[sc][:])
            ex = expp.tile([128, 2, TOK], F8, tag="ex", name="ex")
            nc.scalar.activation(ex[:], ps[:], func=Exp,
                                 scale=1.0 / 8192.0, bias=nb2_sb[:])
            return ex

        def attn_v(hh, exs):
            h = hh // G
            po = psO.tile([128, TOK], FP, tag="psO", name="psO")
            for qd in range(4):
                vq = vv_sb[qd // 2][:, (qd % 2) * 2:(qd % 2) * 2 + 2, h, 0:65]
                nc.tensor.matmul(
                    po[0:65, :],
                    vq,
                    exs[qd][:],
                    start=(qd == 0),
                    stop=(qd == 3),
                    perf_mode=DR,
                )
            recb = recp.tile([1, TOK], BF, tag="recb", name="recb")
            with nc.allow_low_precision(reason="softmax recip rounds to bf16"):
                nc.vector.reciprocal(recb[:], po[64:65, :])
            # broadcast recip across 64 partitions on the idle Pool engine
            rb = bcp.tile([64, TOK], BF, tag="rb", name="rb")
            nc.gpsimd.partition_broadcast(rb[:], recb[:])
            with nc.allow_low_precision(reason="ctx cast to fp8 for O-proj"):
                nc.vector.tensor_mul(
                    ctx_sb[(hh % 2) * 64:(hh % 2) * 64 + 64, hh // 2, :],
                    po[0:64, :],
                    rb[:],
                )

        # ---- Phase A: input DMAs, K/V/Q projections, s-lo scores ---------
        with (
            tc.tile_pool(name="eTp", bufs=1) as etp,
            tc.tile_pool(name="wk", bufs=1) as wkp,
            tc.tile_pool(name="wv", bufs=1) as wvp,
            tc.tile_pool(name="psKV", bufs=2, space="PSUM") as psKV,
        ):
            wk_t = wkp.tile([128, KC, 2, 512], F8, name="wkt")
            et_t = etp.tile([128, KC, 2, S], F8, name="et")
            wv_t = wvp.tile([128, KC, 2, 512], F8, name="wvt")
            xt_t = xtp.tile([128, KC, 2, TOK], F8, name="xt")
            # few big DMAs: descriptor-gen (HWDGE) is a serial shared
            # resource (~0.63us per DMA instruction), so chunking loses
            # PE p-state warm-up during the initial DMA wait: dummy
            # matmuls on the already-zeroed kT pads keep pe_busy_start
            # anchored early so the first real chains run at full clock
            wps = psA.tile([128, TOK], FP, tag="psA", name="psA")
            for _ in range(40):
                nc.tensor.matmul(
                    wps[:, 0:512], kT_sb[0][:, 1, 0:128], kT_sb[0][:, 1, 0:512],
                    start=True, stop=True,
                )
            # critical-path loads on SP; decoder-side on ACT queue;
            # late-needed on DVE queue (DMA engine is shared+serial, queue
            # split biases acquisition order toward the critical path)
            nc.sync.dma_start(wk_t[:], wk[:])
            nc.sync.dma_start(et_t[:, :, :, 0:512], eT[:, :, :, 0:512])
            if not triv_bias:
                nc.sync.dma_start(bias_sb[:], bias_all[:])
            nc.sync.dma_start(xt_t[:], xT[:])
            t = wqp.tile([128, KC, 2, 512], F8, tag="wq", name="wqt")
            nc.scalar.dma_start(t[:, :, :, 0:256], wq[:, :, :, 0:256])
            nc.scalar.dma_start(t[:, :, :, 256:512], wq[:, :, :, 256:512])
            nc.scalar.dma_start(id_sb[:], ident[:])
            wq_blk = [(t, 0)]
            nc.sync.dma_start(wv_t[:], wv[:])
            nc.sync.dma_start(et_t[:, :, :, 512:S], eT[:, :, :, 512:S])
            if use_mask:
                mask_sb = []
                for sc in range(SC):
                    mt = mkp.tile([128, TOK], BF, tag="mk", name="mk")
                    nc.sync.dma_start(mt[:], maskT[sc * 128:(sc + 1) * 128, :])
                    mask_sb.append(mt)

            def k_proj(sh, ms):
                pss = [psKV.tile([128, 512], FP, tag="psKV", name="psKV")
                       for _ in ms]
                for c in range(KC):
                    for i, m in enumerate(ms):
                        nc.tensor.matmul(
                            pss[i][:],
                            wk_t[:, c, :, m * 128:(m + 1) * 128],
                            et_t[:, c, :, sh * 512:(sh + 1) * 512],
                            start=(c == 0),
                            stop=(c == KC - 1),
                            perf_mode=DR,
                        )
                with nc.allow_low_precision(reason="k cast to fp8"):
                    for i, m in enumerate(ms):
                        for hi in range(2):
                            dst = kT_sb[2 * m + hi][:, 0,
                                                    sh * 512:(sh + 1) * 512]
                            if triv_bias:
                                nc.vector.tensor_copy(
                                    dst, pss[i][hi * 64:hi * 64 + 64, :])
                            else:
                                nc.vector.tensor_scalar_add(
                                    dst,
                                    pss[i][hi * 64:hi * 64 + 64, :],
                                    bk_sb[hi * 64:hi * 64 + 64, m:m + 1],
                                )

            def v_proj(scs):
                pss = [psKV.tile([128, 512], FP, tag="psKV", name="psKV")
                       for _ in scs]
                for c in range(KC):
                    for i, sc in enumerate(scs):
                        nc.tensor.matmul(
                            pss[i][:],
                            et_t[:, c, :, sc * 128:(sc + 1) * 128],
                            wv_t[:, c],
                            start=(c == 0),
                            stop=(c == KC - 1),
                            perf_mode=DR,
                        )
                with nc.allow_low_precision(reason="v cast to fp8"):
                    for i, sc in enumerate(scs):
                        if triv_bias:
                            nc.vector.tensor_copy(
                                vv_sb[sc // 4][:, sc % 4, :, 0:64],
                                pss[i].rearrange("p (h d) -> p h d", d=HD),
                            )
                        else:
                            nc.vector.tensor_add(
                                vv_sb[sc // 4][:, sc % 4, :, 0:64],
                                pss[i].rearrange("p (h d) -> p h d", d=HD),
                                bv_sb.rearrange("p (h d) -> p h d", d=HD),
                            )

            # interleave s-lo scores with the projections so ACT (the pole)
            # starts as soon as kT h0/h1 and qT 0-3 exist
            stash = {}
            k_proj(0, (0, 1))
            q_proj(0)
            q_proj(1)
            stash[(0, 0)] = scores_quad(0, 0)
            stash[(0, 1)] = scores_quad(0, 1)
            k_proj(0, (2, 3))
            stash[(1, 0)] = scores_quad(1, 0)
            stash[(1, 1)] = scores_quad(1, 1)
            v_proj((0, 1))
            q_proj(2)
            q_proj(3)
            stash[(2, 0)] = scores_quad(2, 0)
            stash[(2, 1)] = scores_quad(2, 1)
            v_proj((2, 3))
            stash[(3, 0)] = scores_quad(3, 0)
            stash[(3, 1)] = scores_quad(3, 1)
            k_proj(1, (0, 1))
            stash[(4, 0)] = scores_quad(4, 0)
            stash[(4, 1)] = scores_quad(4, 1)
            stash[(5, 0)] = scores_quad(5, 0)
            stash[(5, 1)] = scores_quad(5, 1)
            stash[(6, 0)] = scores_quad(6, 0)
            stash[(6, 1)] = scores_quad(6, 1)
            stash[(7, 0)] = scores_quad(7, 0)
            stash[(7, 1)] = scores_quad(7, 1)
            k_proj(1, (2, 3))
            v_proj((4, 5))
            v_proj((6, 7))

        # psKV released; open attention output pool in its banks
        psO = _stk.enter_context(tc.tile_pool(name="psO", bufs=2, space="PSUM"))

        wo_blks = []
        xres_sb = []

        # ---- Phase B: per-head scores+exp / lagged attn@V ----------------
        pending = []
        for hh in range(NH):
            if hh % 2 == 0 and 2 <= hh <= 24:
                m = 4 + (hh - 2) // 2
                if m % 4 == 0:  # new 512-col Wq block
                    t = wqp.tile([128, KC, 2, 512], F8, tag="wq", name="wqt")
                    nc.scalar.dma_start(
                        t[:], wq[:, :, :, (m // 4) * 512:(m // 4 + 1) * 512])
                    wq_blk[0] = (t, m)
                q_proj(m)
            if hh in (24, 28):  # prefetch Wo halves
                half = (hh - 24) // 4
                t = wop.tile([128, KC, 2, 1024], F8, tag="wo", name="wot")
                nc.scalar.dma_start(
                    t[:], wo[:, :, :, half * 1024:(half + 1) * 1024])
                wo_blks.append(t)
            if hh in (25, 27, 29, 31):  # prefetch residual rows
                tt = (hh - 25) // 2
                xt4 = xrp.tile([128, H], BF, tag="xr", name="xr")
                nc.sync.dma_start(xt4[:], xres[tt * 128:(tt + 1) * 128, :])
                xres_sb.append(xt4)
            exs = []
            for qd in range(4):
                if qd < 2 and (hh, qd) in stash:
                    exs.append(stash.pop((hh, qd)))
                else:
                    exs.append(scores_quad(hh, qd))
            pending.append((hh, exs))
            if len(pending) > 1:
                attn_v(*pending.pop(0))
        for p in pending:
            attn_v(*p)

        _stk.close()

        # ---- Phase C: output projection + residual + layernorm -----------
        with (
            tc.tile_pool(name="psC", bufs=8, space="PSUM") as psC,
            tc.tile_pool(name="outp", bufs=2) as outp,
        ):
            if not triv_ln:
                nc.sync.dma_start(gam_sb[:], gamr[:])
                nc.sync.dma_start(bet_sb[:], betr[:])

            for tt in range(4):
                ob = outp.tile([128, H], BF, tag="ob", name="ob")
                stats = lnp.tile([128, 4, 6], FP, tag="st", name="st")
                pss = []
                for nb in range(4):
                    ps = psC.tile([128, 512], FP, tag="psC", name="psC")
                    pss.append(ps)
                    for c in range(KC):
                        nc.tensor.matmul(
                            ps[:],
                            ctx_sb[:, 2 * c:2 * c + 2, tt * 128:(tt + 1) * 128],
                            wo_blks[nb // 2][:, c, :,
                                             (nb % 2) * 512:(nb % 2 + 1) * 512],
                            start=(c == 0),
                            stop=False,
                            perf_mode=DR,
                        )
                    sl = slice(nb * 512, (nb + 1) * 512)
                    # residual add on the PE: += I.T @ xres (bf16)
                    nc.tensor.matmul(
                        ps[:],
                        id_sb[:],
                        xres_sb[tt][:, sl],
                        start=False,
                        stop=True,
                    )
                    nc.vector.bn_stats(stats[:, nb, :], ps[:])
                mv = lnp.tile([128, 2], FP, tag="mv", name="mv")
                nc.vector.bn_aggr(mv[:], stats[:])
                std = lnp.tile([128, 1], FP, tag="sd", name="sd")
                nc.scalar.activation(
                    std[:], mv[:, 1:2], func=Sqrt, bias=eps_sb[:], scale=1.0
                )
                rstd = lnp.tile([128, 1], FP, tag="rs", name="rs")
                nc.vector.reciprocal(rstd[:], std[:])
                nmr = lnp.tile([128, 1], FP, tag="nm", name="nm")
                nc.vector.scalar_tensor_tensor(
                    nmr[:], mv[:, 0:1], -1.0, rstd[:],
                    op0=mybir.AluOpType.mult, op1=mybir.AluOpType.mult,
                )
                for nb in range(4):
                    sl = slice(nb * 512, (nb + 1) * 512)
                    # (ps - mu) * rstd, alternating DVE / ACT (= ps*rstd
                    # + (-mu*rstd) via Identity) -- Pool cannot read PSUM
                    if nb % 2 == 0:
                        with nc.allow_low_precision(
                                reason="LN out rounds to bf16"):
                            nc.vector.tensor_scalar(
                                ob[:, sl],
                                pss[nb][:],
                                scalar1=mv[:, 0:1],
                                scalar2=rstd[:],
                                op0=mybir.AluOpType.subtract,
                                op1=mybir.AluOpType.mult,
                            )
                    else:
                        nc.scalar.activation(
                            ob[:, sl], pss[nb][:], func=Ident,
                            bias=nmr[:], scale=rstd[:],
                        )
                    if not triv_ln:
                        eng = nc.vector if nb % 2 == 0 else nc.gpsimd
                        eng.tensor_mul(ob[:, sl], ob[:, sl], gam_sb[:, sl])
                        eng.tensor_add(ob[:, sl], ob[:, sl], bet_sb[:, sl])
                    nc.sync.dma_start(out[tt * 128:(tt + 1) * 128, sl],
                                      ob[:, sl])

    nc.compile()
    return nc


def _get_nc(use_mask: bool, triv_ln: bool | None = None,
            triv_bias: bool | None = None):
    if triv_ln is None:
        triv_ln = _LAST_TRIV[0]
    if triv_bias is None:
        triv_bias = _LAST_TRIV[1]
    key = (use_mask, triv_ln, triv_bias)
    if key not in _CACHE:
        _CACHE[key] = _build(use_mask, triv_ln, triv_bias)
    return _CACHE[key]


_LAST_TRIV = [True, True]


def _pack_w(w, cols):
    """[2048, cols] fp32 -> [128, 8, 2, cols] fp8 with K-pair layout."""
    return np.ascontiguousarray(
        (w * WS).reshape(KC, 2, 128, cols).transpose(2, 0, 1, 3)
    ).astype(FP8)


def _pack_act(xT, cols):
    """[2048, cols] fp32 (feature-major) -> [128, 8, 2, cols] fp8."""
    return np.ascontiguousarray(
        xT.reshape(KC, 2, 128, cols).transpose(2, 0, 1, 3)
    ).astype(FP8)


def kernel(
    hidden_state,
    encoder_hidden_state,
    encoder_attention_mask,
    Wq, bq, Wk, bk, Wv, bv, Wo, bo, gamma, beta,
):
    hidden_state = np.asarray(hidden_state, dtype=np.float32)
    encoder_hidden_state = np.asarray(encoder_hidden_state, dtype=np.float32)
    encoder_attention_mask = np.asarray(encoder_attention_mask, dtype=np.float32)
    Wq = np.asarray(Wq, dtype=np.float32)
    bq = np.asarray(bq, dtype=np.float32)
    Wk = np.asarray(Wk, dtype=np.float32)
    bk = np.asarray(bk, dtype=np.float32)
    Wv = np.asarray(Wv, dtype=np.float32)
    bv = np.asarray(bv, dtype=np.float32)
    Wo = np.asarray(Wo, dtype=np.float32)
    bo = np.asarray(bo, dtype=np.float32)
    gamma = np.asarray(gamma, dtype=np.float32)
    beta = np.asarray(beta, dtype=np.float32)

    use_mask = bool(np.any(encoder_attention_mask))
    triv_ln = bool(np.all(gamma == 1.0) and np.all(beta == 0.0))
    triv_bias = not (np.any(bq) or np.any(bk) or np.any(bv))
    _LAST_TRIV[0] = triv_ln
    _LAST_TRIV[1] = triv_bias
    nc = _get_nc(use_mask, triv_ln, triv_bias)
    in_maps = _prepare_in_maps(
        hidden_state, encoder_hidden_state, encoder_attention_mask,
        Wq, bq, Wk, bk, Wv, bv, Wo, bo, gamma, beta, use_mask,
    )

    res = run_bass_kernel_spmd(nc, in_maps, core_ids=list(range(8)))
    kernel._last_results = res

    output = np.empty((B, L, H), dtype=np.float32)
    for c in range(8):
        b, lh = c // 2, c % 2
        output[b, lh * TOK:(lh + 1) * TOK, :] = res.results[c]["out"]
    return output


def _prepare_in_maps(
    hidden_state, encoder_hidden_state, encoder_attention_mask,
    Wq, bq, Wk, bk, Wv, bv, Wo, bo, gamma, beta, use_mask,
):
    triv_ln = bool(np.all(gamma == 1.0) and np.all(beta == 0.0))
    triv_bias = not (np.any(bq) or np.any(bk) or np.any(bv))
    wq_p = _pack_w(Wq, H)
    wk_p = _pack_w(Wk, KVH * HD)
    wv_p = _pack_w(Wv, KVH * HD)
    wo_p = _pack_w(Wo, H)
    if not triv_bias:
        bias_all = np.ascontiguousarray(np.concatenate(
            [
                bq.reshape(16, 128).T * WS,
                bk.reshape(4, 128).T * WS,
                np.tile(bv[None, :], (128, 1)) * WS,
            ],
            axis=1,
        ).astype(np.float32))
    ident = np.ascontiguousarray(np.eye(128, dtype=np.float32).astype(BF16))
    if not triv_ln:
        gamr = np.ascontiguousarray(
            np.tile(gamma[None, :].astype(BF16), (128, 1)))
        betr = np.ascontiguousarray(
            np.tile(beta[None, :].astype(BF16), (128, 1)))

    eT_by_b = [
        _pack_act(encoder_hidden_state[b].T, S) for b in range(B)
    ]

    in_maps = []
    for c in range(8):
        b, lh = c // 2, c % 2
        rows = hidden_state[b, lh * TOK:(lh + 1) * TOK, :]
        m = {
            "xT": _pack_act(rows.T, TOK),
            "xres": ((rows + bo[None, :]) * (WS * WS)).astype(BF16),
            "eT": eT_by_b[b],
            "wq": wq_p, "wk": wk_p, "wv": wv_p, "wo": wo_p,
            "ident": ident,
        }
        if not triv_bias:
            m["bias_all"] = bias_all
        if not triv_ln:
            m["gamr"] = gamr
            m["betr"] = betr
        if use_mask:
            mslice = encoder_attention_mask[b, 0, lh * TOK:(lh + 1) * TOK, :]
            m["maskT"] = np.ascontiguousarray(
                (mslice.T * (8.0 * WS * WS)).astype(BF16))
        in_maps.append(m)
    return in_maps
